# revision 16
# baseline (speedup 1.0000x reference)
"""Trainium2 Bass kernel for the gnn_message_passing block (8 NeuronCores).

Strategy (per core c, owning 512 global rows r = c*512..(c+1)*512):
  - Host rotates x rows by -r0 (owned rows first), pre-transposes the owned
    x block (raw feature-major), and converts all large tensors to 16-bit
    (bf16 weights/mats/x, f16 w2) -- halves HBM traffic and removes all
    casting DMAs (pure HWDGE byte copies).
  - rmsnorm scale-invariance tricks: the per-node inverse-rms r[n] is
    needed exactly (a) multiplied into the node-major aggregation inputs
    h_sb/e_sb and (b) as the exp() scale of the per-node softmax for the
    raw (non-aggregated) q/k sources.  The v path needs NO normalization:
    v = h@Wv for every module, so x_att is uniformly scaled by r_h[n],
    which the (scale-invariant) second rmsnorm removes exactly.  The
    feature-major q/k/v stationary sources are therefore RAW transposed x
    from the host -- no on-chip transposes for them.
  - rsqrt = exp(-0.5*ln(.)): ln+exp live in ONE activation table set
    (natural_log_exp_and_others) together with square/copy, so the whole
    kernel needs only ~4 ACT table loads (vs ~100 when mixing sqrt):
    nl_exp era -> gelu(FFN h) -> nl_exp era -> gelu(FFN e).
  - The five N x N aggregations (adj@h shared by modules 0/4, four proj@k)
    run once as bf16 matmuls (mat tiles streamed from DRAM as the moving
    operand); 512x512 projections are bf16 (stationary x / agg, moving W).
  - Per-node 8-head SDPA on DVE in f16 2x mode: broadcast-AP multiplies +
    halving-tree reduces (measured: TT=2x, TensorReduce/Pool/TTR=1x, so
    trees beat single reduces).  Softmax exp on ACT with per-node scale.
  - Module processing interleaves with aggregations so PE and DVE overlap:
    loads | mod 1,5 | agg0 | mod 0,4 | agg1,2 | mod 2 | agg3 | mod 3 |
    agg4 | FFN-h | mod 7 | mod 6 + pipelined FFN-e
  - FFN-e is fully pipelined per node-block via a tile callback inside the
    last module: each 128-node block runs rmsnorm2 -> transpose -> w1 ->
    gelu -> w2 -> DMA while the remaining SDPA tiles are still on DVE, so
    only the last block's chain (~14us) is exposed at the end.
  - DMA queues: x/mats/outputs on SP (HWDGE), weights on POOL (SWDGE);
    POOL otherwise only does the cheap per-node normalization multiplies.
  - CoreSim: 350.5us vs 474.6us for the previous kernel (DVE-bound; DVE
    busy 317us of the 350us span, PE 282us, ACT 184us).
"""
import numpy as np

N = 4096
E = 512
H = 8
D = 64
FF = 2048
P = 128
NCORES = 8
RPC = N // NCORES  # 512 rows per core
NT = N // P        # 32 tiles over all nodes
LT = RPC // P      # 4 local tiles
EPS = float(np.finfo(np.float32).eps)
# run the av broadcast-multiply on GpSimd for tiles with
# t % POOL_AV_MOD == POOL_AV_PHASE (engine-balance knob; real-HW GpSimd
# tensor_tensor is ~4x slower than the cost model says, so keep this off)
POOL_AV_MOD = 1000
POOL_AV_PHASE = 1

_PROGRAM_CACHE = {}


def _split_big_waits(nc, mybir):
    """walrus in this toolchain rejects multi-wait instructions; cap at 1
    (2 for EventSemaphore), chaining the excess as EventSemaphores."""
    for f in nc.m.functions:
        for bb in f.blocks:
            insts = list(bb.instructions)
            out = []
            changed = False
            for inst in insts:
                si = inst.sync_info
                waits = list(si.on_wait) if si and si.on_wait else []
                cap = 2 if isinstance(inst, mybir.InstEventSemaphore) else 1
                if len(waits) > cap:
                    extra, keep = waits[:-cap], waits[-cap:]
                    for ci in range(0, len(extra), 2):
                        ev = mybir.InstEventSemaphore(name=f"{inst.name}-evw{ci}")
                        ev.engine = inst.engine
                        ev.sync_info = mybir.SyncInfo(on_wait=extra[ci:ci + 2],
                                                      on_update=[])
                        out.append(ev)
                    si.on_wait = keep
                    changed = True
                out.append(inst)
            if changed:
                bb.instructions[:] = out


def _build_program(repeat=1):
    import concourse.bass as bass
    import concourse.tile as tile
    from concourse import mybir
    from concourse.masks import make_identity
    from contextlib import ExitStack

    f32 = mybir.dt.float32
    f32r = mybir.dt.float32r
    f16 = mybir.dt.float16
    bf16 = mybir.dt.bfloat16
    f8 = mybir.dt.float8e4
    AF = mybir.ActivationFunctionType
    OP = mybir.AluOpType
    AX = mybir.AxisListType
    DR = mybir.MatmulPerfMode.DoubleRow

    def bc(t, dims, off=0):
        return bass.AP(tensor=t.tensor, offset=t.offset + off,
                       ap=[list(t.ap[0])] + [[s, c] for (s, c) in dims])

    nc = bass.Bass()

    xn_d = nc.declare_dram_parameter("xn", [N, E], bf16, isOutput=False)
    xe_d = nc.declare_dram_parameter("xe", [N, E], bf16, isOutput=False)
    xnT_d = nc.declare_dram_parameter("xnT", [E, RPC], bf16, isOutput=False)
    xeT_d = nc.declare_dram_parameter("xeT", [E, RPC], bf16, isOutput=False)
    mat_d = [nc.declare_dram_parameter(f"mat{i}", [N, RPC], f8, isOutput=False)
             for i in range(5)]
    wq_d = nc.declare_dram_parameter("wqT", [H, E, E], bf16, isOutput=False)
    wk_d = nc.declare_dram_parameter("wkT", [H, E, E], bf16, isOutput=False)
    wv_d = nc.declare_dram_parameter("wvT", [H, E, E], bf16, isOutput=False)
    w1h_d = nc.declare_dram_parameter("w1hT", [E, FF], bf16, isOutput=False)
    w2h_d = nc.declare_dram_parameter("w2hT", [FF, E], f16, isOutput=False)
    w1e_d = nc.declare_dram_parameter("w1eT", [E, FF], bf16, isOutput=False)
    w2e_d = nc.declare_dram_parameter("w2eT", [FF, E], f16, isOutput=False)
    b1h_d = nc.declare_dram_parameter("b1h", [FF], f32, isOutput=False)
    b2h_d = nc.declare_dram_parameter("b2h", [E], f32, isOutput=False)
    b1e_d = nc.declare_dram_parameter("b1e", [FF], f32, isOutput=False)
    b2e_d = nc.declare_dram_parameter("b2e", [E], f32, isOutput=False)
    outh_d = nc.declare_dram_parameter("outh", [RPC, E], f32, isOutput=True)
    oute_d = nc.declare_dram_parameter("oute", [RPC, E], f32, isOutput=True)

    with tile.TileContext(nc, pool_alloc_mode="queue") as tc, ExitStack() as ctx:
        consts = ctx.enter_context(tc.tile_pool(name="consts", bufs=1))
        eps_t = consts.tile([P, 1], f32)
        nc.vector.memset(eps_t, EPS)
        lnwarm = consts.tile([P, 1], f32)
        # first ACT op: pins the natural_log_exp_and_others table set
        nc.scalar.activation(out=lnwarm[:], in_=eps_t[:], func=AF.Ln)
        ident = consts.tile([P, P], f16)
        make_identity(nc, ident)
        ones1f = consts.tile([1, P], f32)
        nc.gpsimd.memset(ones1f, 1.0)
        ones1 = consts.tile([1, P], f32r)
        nc.scalar.copy(ones1[:], ones1f[:])
        b1h_t = consts.tile([P, FF // P], f32)
        b1e_t = consts.tile([P, FF // P], f32)
        b2h_t = consts.tile([1, E], f32r)
        b2e_t = consts.tile([1, E], f32r)

        # whole-program pools
        statp = ctx.enter_context(tc.tile_pool(name="stat", bufs=1))
        rawp = ctx.enter_context(tc.tile_pool(name="raw", bufs=1))
        sqscp = ctx.enter_context(tc.tile_pool(name="sqsc", bufs=1))
        xTlp = ctx.enter_context(tc.tile_pool(name="xTl", bufs=1))
        wpool = ctx.enter_context(tc.tile_pool(name="wts", bufs=1))
        qkvp = ctx.enter_context(tc.tile_pool(name="qkv", bufs=1))
        tmpp = ctx.enter_context(tc.tile_pool(name="sdtmp", bufs=1))
        smp = ctx.enter_context(tc.tile_pool(name="sdsm", bufs=2))
        accp = ctx.enter_context(tc.tile_pool(name="acc", bufs=1))
        matgp = ctx.enter_context(tc.tile_pool(name="matg", bufs=2))
        fwts = ctx.enter_context(tc.tile_pool(name="fwts", bufs=1))
        psp = ctx.enter_context(tc.tile_pool(name="ps", bufs=1, space="PSUM"))

        def body():
            # per-iteration stat tiles
            ssq_h = statp.tile([P, NT], f32, tag="ssqh", name="ssqh")
            ssq_e = statp.tile([P, NT], f32, tag="ssqe", name="ssqe")
            lnsc = statp.tile([P, NT], f32, tag="lnsc", name="lnsc")
            rh_t = statp.tile([P, NT], f32, tag="rh", name="rh")
            re_t = statp.tile([P, NT], f32, tag="re", name="re")
            re2_t = statp.tile([P, LT], f32, tag="re2", name="re2")

            xnTl = [xTlp.tile([P, RPC], bf16, tag=f"xnT{fc}", name=f"xnT{fc}")
                    for fc in range(4)]
            xeTl = [xTlp.tile([P, RPC], bf16, tag=f"xeT{fc}", name=f"xeT{fc}")
                    for fc in range(4)]
            for fc in range(4):
                nc.sync.dma_start(out=xeTl[fc][:],
                                  in_=xeT_d[fc * P:(fc + 1) * P, :])
            for fc in range(4):
                nc.sync.dma_start(out=xnTl[fc][:],
                                  in_=xnT_d[fc * P:(fc + 1) * P, :])

            acc_h = [accp.tile([P, E], f16, tag=f"ah{t}", name=f"ah{t}")
                     for t in range(LT)]
            acc_e = [accp.tile([P, E], f16, tag=f"ae{t}", name=f"ae{t}")
                     for t in range(LT)]

            def load_group(x_d, sb8, ssq, rdst, g0, dve_stats=False):
                """Load 4 node-major tiles, square-accum stats, rsqrt the 4
                columns, normalize into the fp8 aggregation mega-tile sb8.
                dve_stats puts the squares on DVE (for the first group, while
                DVE is idle and ACT latency gates the first module)."""
                xg = rawp.tile([P, 4 * E], bf16, tag="xraw", bufs=2,
                               name="xraw")
                nc.sync.dma_start(
                    out=xg.rearrange("p (t e) -> p t e", e=E),
                    in_=x_d[g0 * P:(g0 + 4) * P, :].rearrange(
                        "(t p) e -> p t e", p=P))
                for t in range(4):
                    ti = g0 + t
                    scr = sqscp.tile([P, E], f16, tag="sq", bufs=1, name="sq")
                    if dve_stats:
                        nc.vector.scalar_tensor_tensor(
                            out=scr[:], in0=xg[:, t * E:(t + 1) * E],
                            scalar=0.0, in1=xg[:, t * E:(t + 1) * E],
                            op0=OP.add, op1=OP.mult,
                            accum_out=ssq[:, ti:ti + 1])
                    else:
                        nc.scalar.activation(out=scr[:],
                                             in_=xg[:, t * E:(t + 1) * E],
                                             func=AF.Square,
                                             accum_out=ssq[:, ti:ti + 1])
                nc.scalar.activation(out=lnsc[:, g0:g0 + 4],
                                     in_=ssq[:, g0:g0 + 4], func=AF.Ln,
                                     scale=1.0 / E, bias=eps_t[:])
                nc.scalar.activation(out=rdst[:, g0:g0 + 4],
                                     in_=lnsc[:, g0:g0 + 4], func=AF.Exp,
                                     scale=-0.5)
                for t in range(4):
                    ti = g0 + t
                    nc.gpsimd.tensor_scalar_mul(sb8[:, ti * E:(ti + 1) * E],
                                                xg[:, t * E:(t + 1) * E],
                                                rdst[:, ti:ti + 1])

            def aggregate(mi, src8, aggpool):
                """4 feature-major bf16 [128, 512] blocks of mat_mi @ x.

                fp8e4 DoubleRow matmuls: both operands fp8 (mats pre-scaled
                host-side; inverse scale folded into wq/wk), 2 node-tiles
                (256 contraction rows) per pass at 0.5 cycles/row.  Output
                partition limit is 64, so each psum bank holds a pair of
                64-feature blocks at partition offsets 0/64 and the bf16
                copy-out still reads one [128, 512] bank."""
                pss = [psp.tile([P, E], f32, tag=f"agps{b}", name=f"agps{b}")
                       for b in range(4)]
                scr = [psp.tile([64, E], f32, tag="projps", bufs=4,
                                name=f"agsc{b}") for b in range(4)]
                for g in range(8):
                    mt = matgp.tile([P, 4 * RPC], f8, tag="matg", name="matg")
                    nc.sync.dma_start(
                        out=mt.rearrange("p (t e) -> p t e", e=RPC),
                        in_=mat_d[mi][g * 4 * P:(g + 1) * 4 * P, :].rearrange(
                            "(t p) e -> p t e", p=P))
                    for tp in range(2):
                        pair = g * 2 + tp
                        for b in range(4):
                            # DoubleRow dst is ISA-limited to partitions
                            # 0-63: even feature-half accumulates in
                            # pss[b][0:64], odd half in a scratch bank and
                            # is rebased to [64:128] afterwards.
                            for half in range(2):
                                dst = pss[b][0:64, :] if half == 0 else scr[b][:]
                                nc.tensor.matmul(
                                    dst,
                                    lhsT=bc(src8, [(E, 2), (1, 64)],
                                            off=pair * 2 * E + (2 * b + half) * 64),
                                    rhs=bc(mt, [(RPC, 2), (1, RPC)],
                                           off=tp * 2 * RPC),
                                    start=(pair == 0), stop=(pair == 15),
                                    perf_mode=DR,
                                    tile_position=(0, 0),
                                    skip_group_check=True)
                outt = []
                for b in range(4):
                    t64 = sqscp.tile([64, E], f16, tag="agt64", bufs=2,
                                     name="agt64")
                    nc.scalar.copy(t64[:], scr[b][:])
                    nc.tensor.matmul(pss[b][64:128, :],
                                     lhsT=ident[0:64, 0:64], rhs=t64[:],
                                     start=True, stop=True,
                                     tile_position=(0, 64),
                                     skip_group_check=True)
                    at = aggpool.tile([P, E], bf16, tag=f"ag{mi}_{b}",
                                      name=f"ag{mi}_{b}")
                    nc.scalar.copy(at[:], pss[b][:])
                    outt.append(at)
                return outt

            def module(m, qsrcT, ksrcT, branch_att, rsc, first, warmln=False,
                       tile_cb=None):
                w_ts = {}
                for (dram, nm) in ((wq_d, "wq"), (wk_d, "wk"), (wv_d, "wv")):
                    wt = wpool.tile([P, 4 * E], bf16, tag=nm,
                                    bufs=(1 if nm == "wv" else 2),
                                    name=f"w_{nm}")
                    nc.sync.dma_start(
                        out=wt.rearrange("p (fc e) -> p fc e", e=E),
                        in_=dram[m].rearrange("(fc p) e -> p fc e", p=P))
                    w_ts[nm] = wt
                if warmln:
                    # re-pin the ln+exp table set after a gelu era
                    nc.scalar.activation(out=lnwarm[:], in_=eps_t[:],
                                         func=AF.Ln)

                # per-tile interleave (q_b, k_b, v_b) so tile 0's SDPA can
                # start after 12 matmuls instead of 36
                q_sb, k_sb, v_sb = [], [], []
                for b in range(LT):
                    for (srcT, wnm, lst) in ((qsrcT, "wq", q_sb),
                                             (ksrcT, "wk", k_sb),
                                             (xnTl, "wv", v_sb)):
                        wt = w_ts[wnm]
                        ps = psp.tile([P, E], f32, tag="projps", bufs=4,
                                      name="projps")
                        for fc in range(4):
                            nc.tensor.matmul(
                                ps[:],
                                lhsT=srcT[fc][:, b * P:(b + 1) * P],
                                rhs=wt[:, fc * E:(fc + 1) * E],
                                start=(fc == 0), stop=(fc == 3))
                        dt = qkvp.tile([P, E], f16, tag=f"{wnm}_{b}",
                                       bufs=(2 if wnm == "wq" else 1),
                                       name=f"qkv{b}")
                        if wnm == "wv":
                            nc.scalar.copy(bc(dt, [(1, 8), (8, 64)]), ps[:])
                        else:
                            nc.scalar.copy(dt[:], ps[:])
                        lst.append(dt)

                for t in range(LT):
                    q_t, k_t, v_t = q_sb[t], k_sb[t], v_sb[t]
                    tmp = tmpp.tile([P, H * H * D], f16, tag="sdpa", bufs=1,
                                    name="sdpa")
                    nc.vector.tensor_tensor(
                        out=bc(tmp, [(512, 8), (64, 8), (1, 64)]),
                        in0=bc(q_t, [(64, 8), (0, 8), (1, 64)]),
                        in1=bc(k_t, [(0, 8), (64, 8), (1, 64)]),
                        op=OP.mult)
                    for dd in (32, 16, 8, 4, 2):
                        nc.vector.tensor_tensor(
                            out=bc(tmp, [(64, 64), (1, dd)]),
                            in0=bc(tmp, [(64, 64), (1, dd)]),
                            in1=bc(tmp, [(64, 64), (1, dd)], off=dd),
                            op=OP.add)
                    s_t = smp.tile([P, H * H], f16, tag="s", name="s")
                    nc.vector.tensor_tensor(
                        out=s_t[:],
                        in0=bc(tmp, [(64, 64)]),
                        in1=bc(tmp, [(64, 64)], off=1),
                        op=OP.add)
                    ex_t = smp.tile([P, H * H], f16, tag="ex", name="ex")
                    nc.scalar.activation(out=ex_t[:], in_=s_t[:], func=AF.Exp,
                                         scale=rsc[:, t:t + 1])
                    den = smp.tile([P, H], f32, tag="den", name="den")
                    nc.vector.tensor_reduce(
                        out=den[:], in_=ex_t.rearrange("p (h g) -> p h g", g=H),
                        axis=AX.X, op=OP.add)
                    rden = smp.tile([P, H], f32, tag="rden", name="rden")
                    nc.vector.reciprocal(out=rden[:], in_=den[:])
                    a_t = smp.tile([P, H * H], f16, tag="a", name="a")
                    nc.vector.tensor_tensor(
                        out=bc(a_t, [(8, 8), (1, 8)]),
                        in0=bc(ex_t, [(8, 8), (1, 8)]),
                        in1=bc(rden, [(1, 8), (0, 8)]),
                        op=OP.mult)
                    tmp2 = tmpp.tile([P, H * H * D], f16, tag="sdpa2", bufs=2,
                                     name="sdpa2")
                    av_eng = nc.gpsimd if (t % POOL_AV_MOD == POOL_AV_PHASE) \
                        else nc.vector
                    av_eng.tensor_tensor(
                        out=bc(tmp2, [(512, 8), (8, 64), (1, 8)]),
                        in0=bc(a_t, [(8, 8), (0, 64), (1, 8)]),
                        in1=bc(v_t, [(0, 8), (8, 64), (1, 8)]),
                        op=OP.mult)
                    for gg in (4, 2):
                        nc.vector.tensor_tensor(
                            out=bc(tmp2, [(8, 512), (1, gg)]),
                            in0=bc(tmp2, [(8, 512), (1, gg)]),
                            in1=bc(tmp2, [(8, 512), (1, gg)], off=gg),
                            op=OP.add)
                    if first:
                        nc.vector.tensor_tensor(
                            out=branch_att[t][:],
                            in0=bc(tmp2, [(8, 512)]),
                            in1=bc(tmp2, [(8, 512)], off=1),
                            op=OP.add)
                    else:
                        rt = smp.tile([P, E], f16, tag="avred", bufs=1,
                                      name="avred")
                        nc.vector.tensor_tensor(
                            out=rt[:],
                            in0=bc(tmp2, [(8, 512)]),
                            in1=bc(tmp2, [(8, 512)], off=1),
                            op=OP.add)
                        nc.vector.tensor_tensor(out=branch_att[t][:],
                                                in0=branch_att[t][:],
                                                in1=rt[:], op=OP.add)
                    if tile_cb is not None:
                        tile_cb(t)

            def ffn_wload(w1_dram, w2_dram):
                """Weight tiles + DMAs; emit early to hide the transfers."""
                w1_ts = []
                for half in range(2):
                    HW1 = FF // 2
                    w1_t = fwts.tile([P, 4 * HW1], bf16, tag=f"w1_{half}",
                                     name=f"w1_{half}")
                    nc.sync.dma_start(
                        out=w1_t.rearrange("p (fc e) -> p fc e", e=HW1),
                        in_=w1_dram[:, half * HW1:(half + 1) * HW1].rearrange(
                            "(fc p) e -> p fc e", p=P))
                    w1_ts.append(w1_t)
                return (w1_ts, w2_dram)

            def ffn(branch_att, wtiles, b1_t, b2_t, out_dram):
                w1_ts, w2_dram = wtiles
                with tc.tile_pool(name="ffn_sb", bufs=1) as fsb:
                    w2_t = fsb.tile([P, 16 * E], f16, tag="w2", name="w2")
                    nc.sync.dma_start(
                        out=w2_t.rearrange("p (fc e) -> p fc e", e=E),
                        in_=w2_dram[:, :].rearrange("(fc p) e -> p fc e", p=P))
                    # rmsnorm2: stats + rsqrt + normalize (f16)
                    ssq2 = fsb.tile([P, LT], f32, tag="fss", name="fss")
                    for t in range(LT):
                        scr = sqscp.tile([P, E], f16, tag="sq", bufs=1,
                                         name="fsq")
                        nc.scalar.activation(out=scr[:], in_=branch_att[t][:],
                                             func=AF.Square,
                                             accum_out=ssq2[:, t:t + 1])
                    ln2 = fsb.tile([P, LT], f32, tag="fln", name="fln")
                    nc.scalar.activation(out=ln2[:], in_=ssq2[:], func=AF.Ln,
                                         scale=1.0 / E, bias=eps_t[:])
                    rs2 = fsb.tile([P, LT], f32, tag="frs", name="frs")
                    nc.scalar.activation(out=rs2[:], in_=ln2[:], func=AF.Exp,
                                         scale=-0.5)
                    xn_tiles = []
                    for t in range(LT):
                        xt = fsb.tile([P, E], f16, tag=f"fx{t}", name=f"fx{t}")
                        nc.gpsimd.tensor_scalar_mul(xt[:], branch_att[t][:],
                                                    rs2[:, t:t + 1])
                        xn_tiles.append(xt)
                    xnT = []
                    for fc in range(4):
                        ps = psp.tile([P, RPC], f16, tag="agps0", name="ftr")
                        for t in range(4):
                            nc.tensor.transpose(ps[:, t * P:(t + 1) * P],
                                                xn_tiles[t][:, fc * P:(fc + 1) * P],
                                                ident[:])
                        xt = fsb.tile([P, RPC], bf16, tag=f"fxT{fc}",
                                      name=f"fxT{fc}")
                        nc.scalar.copy(xt[:], ps[:])
                        xnT.append(xt)
                    g1 = []
                    HW1 = FF // 2
                    for half in range(2):
                        w1_t = w1_ts[half]
                        for fb in range(HW1 // P):
                            ffb = half * (HW1 // P) + fb
                            ps = psp.tile([P, RPC], f32, tag=f"agps{1 + ffb % 2}",
                                          name="fps1")
                            for fc in range(4):
                                nc.tensor.matmul(
                                    ps[:],
                                    lhsT=w1_t[:, fc * HW1 + fb * P:
                                              fc * HW1 + (fb + 1) * P],
                                    rhs=xnT[fc][:],
                                    start=(fc == 0), stop=(fc == 3))
                            gt = fsb.tile([P, RPC], f16, tag=f"g1_{ffb}",
                                          name=f"g1_{ffb}")
                            nc.scalar.activation(out=gt[:], in_=ps[:],
                                                 func=AF.Gelu,
                                                 bias=b1_t[:, ffb:ffb + 1],
                                                 scale=1.0)
                            g1.append(gt)
                    for b in range(LT):
                        ps = psp.tile([P, E], f32, tag="agps3", name="fps2")
                        for ffc in range(FF // P):
                            nc.tensor.matmul(
                                ps[:],
                                lhsT=g1[ffc][:, b * P:(b + 1) * P],
                                rhs=w2_t[:, ffc * E:(ffc + 1) * E],
                                start=(ffc == 0), stop=False)
                        nc.tensor.matmul(ps[:], lhsT=ones1[:], rhs=b2_t[:],
                                         start=False, stop=True)
                        ob = fsb.tile([P, E], f32, tag="fo", bufs=2, name="fo")
                        nc.scalar.copy(ob[:], ps[:])
                        nc.sync.dma_start(
                            out=out_dram[b * P:(b + 1) * P, :], in_=ob[:])

            def mk_ffn_pipe(branch_att, w1_ts, w2_dram, b1_t, b2_t, out_dram,
                            fsb):
                """Per-tile pipelined FFN: the returned callback is invoked
                inside the final module after each tile's SDPA, so nearly the
                whole FFN runs under the module's remaining SDPA work.  Node
                block t is pushed through rmsnorm2 -> transpose -> w1 -> gelu
                -> w2 as soon as its attention accumulator is final; only
                block 3's chain is exposed at the end."""
                HW1 = FF // 2
                st = {
                    "ssq2": fsb.tile([P, LT], f32, tag="fss", name="fss"),
                    "ln2": fsb.tile([P, LT], f32, tag="fln", name="fln"),
                    "rs2": fsb.tile([P, LT], f32, tag="frs", name="frs"),
                    "psT": [psp.tile([P, RPC], f16, tag=f"agps{fc}",
                                     name=f"eftr{fc}") for fc in range(4)],
                    "xnT": [fsb.tile([P, RPC], bf16, tag=f"fxT{fc}",
                                     name=f"fxT{fc}") for fc in range(4)],
                    "g1": [fsb.tile([P, RPC], f16, tag=f"g1_{ffb}",
                                    name=f"g1_{ffb}") for ffb in range(16)],
                    "w2": fsb.tile([P, 16 * E], f16, tag="w2", name="w2"),
                    "ps2": [None] * LT,
                }

                def out_block(b):
                    ob = fsb.tile([P, E], f32, tag="fo", bufs=2, name="fo")
                    nc.vector.tensor_scalar_mul(ob[:], st["ps2"][b][:], 1.0)
                    nc.sync.dma_start(out=out_dram[b * P:(b + 1) * P, :],
                                      in_=ob[:])

                def block_chain(b):
                    """w1 -> gelu -> w2 for node block b (emitted one SDPA
                    tile later so its ACT ops sit behind the next exp in the
                    queue and can't stall the softmax chain)."""
                    lo, hi = b * P, (b + 1) * P
                    if b > 0:
                        out_block(b - 1)
                    for ffb in range(16):
                        w1_t = w1_ts[ffb // 8]
                        fb = ffb % 8
                        ps = psp.tile([P, P], f32, tag="projps", bufs=4,
                                      name="efps1")
                        for fc in range(4):
                            nc.tensor.matmul(
                                ps[:],
                                lhsT=w1_t[:, fc * HW1 + fb * P:
                                          fc * HW1 + (fb + 1) * P],
                                rhs=st["xnT"][fc][:, lo:hi],
                                start=(fc == 0), stop=(fc == 3))
                        nc.scalar.activation(out=st["g1"][ffb][:, lo:hi],
                                             in_=ps[:], func=AF.Gelu,
                                             bias=b1_t[:, ffb:ffb + 1],
                                             scale=1.0)
                    ps2 = psp.tile([P, E], f32, tag="projps", bufs=4,
                                    name="efps2")
                    for ffc in range(FF // P):
                        nc.tensor.matmul(
                            ps2[:],
                            lhsT=st["g1"][ffc][:, lo:hi],
                            rhs=st["w2"][:, ffc * E:(ffc + 1) * E],
                            start=(ffc == 0), stop=False)
                    nc.tensor.matmul(ps2[:], lhsT=ones1[:], rhs=b2_t[:],
                                     start=False, stop=True)
                    st["ps2"][b] = ps2

                def cb(t):
                    if t == 0:
                        nc.sync.dma_start(
                            out=st["w2"].rearrange("p (fc e) -> p fc e", e=E),
                            in_=w2_dram[:, :].rearrange("(fc p) e -> p fc e",
                                                        p=P))
                    lo, hi = t * P, (t + 1) * P
                    scr = sqscp.tile([P, E], f16, tag="sq", bufs=1, name="esq")
                    nc.vector.scalar_tensor_tensor(
                        out=scr[:], in0=branch_att[t][:], scalar=0.0,
                        in1=branch_att[t][:], op0=OP.add, op1=OP.mult,
                        accum_out=st["ssq2"][:, t:t + 1])
                    nc.scalar.activation(out=st["ln2"][:, t:t + 1],
                                         in_=st["ssq2"][:, t:t + 1],
                                         func=AF.Ln, scale=1.0 / E,
                                         bias=eps_t[:])
                    nc.scalar.activation(out=st["rs2"][:, t:t + 1],
                                         in_=st["ln2"][:, t:t + 1],
                                         func=AF.Exp, scale=-0.5)
                    xt = fsb.tile([P, E], f16, tag="fx", bufs=2, name="fx")
                    nc.gpsimd.tensor_scalar_mul(xt[:], branch_att[t][:],
                                                st["rs2"][:, t:t + 1])
                    for fc in range(4):
                        nc.tensor.transpose(st["psT"][fc][:, lo:hi],
                                            xt[:, fc * P:(fc + 1) * P],
                                            ident[:])
                        nc.scalar.copy(st["xnT"][fc][:, lo:hi],
                                       st["psT"][fc][:, lo:hi])
                    if t > 0:
                        block_chain(t - 1)

                def tail():
                    block_chain(LT - 1)
                    out_block(LT - 1)

                return cb, tail

            # ======== emission order (the schedule) ========
            agg12_stack = ExitStack()
            agg12p = agg12_stack.enter_context(tc.tile_pool(name="agg12", bufs=1))
            agg34_stack = ExitStack()
            agg34p = agg34_stack.enter_context(tc.tile_pool(name="agg34", bufs=1))
            e_stack = ExitStack()
            epool = e_stack.enter_context(tc.tile_pool(name="epool", bufs=1))
            h_stack = ExitStack()
            hpool = h_stack.enter_context(tc.tile_pool(name="hpool", bufs=1))
            agg0_stack = ExitStack()
            agg0p = agg0_stack.enter_context(tc.tile_pool(name="agg0p", bufs=1))

            h_sb = hpool.tile([P, NT * E], f8, tag="h8", name="h8")
            e_sb = epool.tile([P, NT * E], f8, tag="e8", name="e8")

            # local tiles first (rsqrt cols 0-3 feed the exp scales);
            # xe first: modules 1/5 need re^2 + xeTl before anything else
            load_group(xe_d, e_sb, ssq_e, re_t, 0, dve_stats=True)
            nc.scalar.activation(out=re2_t[:], in_=re_t[:, 0:LT],
                                 func=AF.Square)

            # modules 1 and 5 need no aggregates - start DVE early
            # (xn group 0's ACT squares go after module 5 so they cannot
            # delay module 5's softmax exps in the ACT queue)
            module(1, xeTl, xeTl, acc_h, re2_t, first=True)
            module(5, xeTl, xeTl, acc_e, re2_t, first=True)
            load_group(xn_d, h_sb, ssq_h, rh_t, 0)

            for g0 in range(4, NT, 4):
                load_group(xn_d, h_sb, ssq_h, rh_t, g0)
            for g0 in range(4, NT // 2, 4):
                load_group(xe_d, e_sb, ssq_e, re_t, g0)

            nc.sync.dma_start(out=b1h_t,
                                in_=b1h_d[:].rearrange("(c p) -> p c", p=P))
            nc.sync.dma_start(out=b1e_t,
                                in_=b1e_d[:].rearrange("(c p) -> p c", p=P))
            nc.gpsimd.dma_start(out=b2h_t,
                                in_=b2h_d[:].rearrange("(a e) -> a e", a=1))
            nc.gpsimd.dma_start(out=b2e_t,
                                in_=b2e_d[:].rearrange("(a e) -> a e", a=1))

            agg0 = aggregate(0, h_sb, agg0p)
            module(0, agg0, xnTl, acc_h, rh_t, first=False)
            module(4, agg0, xnTl, acc_e, rh_t, first=False)

            agg1 = aggregate(1, h_sb, agg12p)
            agg2 = aggregate(2, h_sb, agg12p)
            agg0_stack.close()

            module(2, xeTl, agg1, acc_h, re_t, first=False)
            h_stack.close()

            for g0 in range(NT // 2, NT, 4):
                load_group(xe_d, e_sb, ssq_e, re_t, g0)
            agg3 = aggregate(3, e_sb, agg34p)
            wt_h = ffn_wload(w1h_d, w2h_d)
            module(3, xnTl, agg3, acc_h, rh_t, first=False)

            agg4 = aggregate(4, e_sb, agg34p)
            e_stack.close()

            ffn(acc_h, wt_h, b1h_t, b2h_t, outh_d)
            wt_e = ffn_wload(w1e_d, w2e_d)
            module(7, xnTl, agg4, acc_e, rh_t, first=False, warmln=True)
            agg34_stack.close()
            with tc.tile_pool(name="ffnE_sb", bufs=1) as fsbE:
                ecb, etail = mk_ffn_pipe(acc_e, wt_e[0], wt_e[1], b1e_t,
                                         b2e_t, oute_d, fsbE)
                module(6, xeTl, agg2, acc_e, re_t, first=False, tile_cb=ecb)
                etail()
            agg12_stack.close()

        for _ in range(repeat):
            body()

    _split_big_waits(nc, mybir)
    return nc


def _get_program():
    if "nc" not in _PROGRAM_CACHE:
        _PROGRAM_CACHE["nc"] = _build_program()
    return _PROGRAM_CACHE["nc"]


def _prep_inputs(x_node, x_edge, adj, Wq, Wk, Wv,
                 proj_he_h, proj_eh_h, proj_he_e, proj_eh_e,
                 rms1_h, rms1_e, rms2_h,
                 w1_h, b1_h, w2_h, b2_h, w1_e, b1_e, w2_e, b2_e):
    """Per-core input dicts. Weight folding + row rotation happen here."""
    from ml_dtypes import bfloat16, float8_e4m3
    f = np.float32
    bf = bfloat16
    f16 = np.float16
    f8 = float8_e4m3
    # mats quantized to fp8e4m3, pre-scaled into a good fp8 range; the
    # inverse scale is folded into the wq/wk of the module consuming the
    # aggregate (mat0=adj -> q of modules 0/4; mat1..4 -> k of 2,6,3,7).
    MSC = [4096.0, 64.0, 64.0, 64.0, 64.0]
    qsc = [1.0 / MSC[0], 1, 1, 1, 1.0 / MSC[0], 1, 1, 1]
    ksc = [1, 1, 1.0 / MSC[1], 1.0 / MSC[3], 1, 1, 1.0 / MSC[2], 1.0 / MSC[4]]
    wsrc_q = [rms1_h, rms1_e, rms1_e, rms1_h, rms1_h, rms1_e, rms1_e, rms1_h]
    wsrc_k = [rms1_h, rms1_e, rms1_h, rms1_e, rms1_h, rms1_e, rms1_h, rms1_e]
    wqT = np.stack([(Wq[m].T * wsrc_q[m][:, None]) * (0.125 * qsc[m])
                    for m in range(H)])
    wkT = np.stack([(Wk[m].T * wsrc_k[m][:, None]) * ksc[m] for m in range(H)])
    wvT = np.stack([Wv[m].T * rms1_h[:, None] for m in range(H)])
    w1hT = np.ascontiguousarray((w1_h * rms2_h[None, :]).T.astype(bf))
    w1eT = np.ascontiguousarray((w1_e * rms2_h[None, :]).T.astype(bf))
    w2hT = np.ascontiguousarray(w2_h.T.astype(f16))
    w2eT = np.ascontiguousarray(w2_e.T.astype(f16))
    mats = [adj, proj_eh_h, proj_eh_e, proj_he_h, proj_he_e]

    shared = dict(wqT=np.ascontiguousarray(wqT.astype(bf)),
                  wkT=np.ascontiguousarray(wkT.astype(bf)),
                  wvT=np.ascontiguousarray(wvT.astype(bf)),
                  w1hT=w1hT, w2hT=w2hT, w1eT=w1eT, w2eT=w2eT,
                  b1h=b1_h.astype(f), b2h=b2_h.astype(f),
                  b1e=b1_e.astype(f), b2e=b2_e.astype(f))
    xn_bf = x_node.astype(bf)
    xe_bf = x_edge.astype(bf)
    in_maps = []
    for c in range(NCORES):
        r0 = c * RPC
        m = dict(shared)
        m["xn"] = np.ascontiguousarray(np.roll(xn_bf, -r0, axis=0))
        m["xe"] = np.ascontiguousarray(np.roll(xe_bf, -r0, axis=0))
        m["xnT"] = np.ascontiguousarray(xn_bf[r0:r0 + RPC].T)
        m["xeT"] = np.ascontiguousarray(xe_bf[r0:r0 + RPC].T)
        for i, mat in enumerate(mats):
            mt = np.ascontiguousarray(
                (mat[r0:r0 + RPC].T * MSC[i]).astype(f8))  # [N, RPC]
            m[f"mat{i}"] = np.ascontiguousarray(np.roll(mt, -r0, axis=0))
        in_maps.append(m)
    return in_maps


def kernel(**inputs):
    from concourse.bass_utils import run_bass_kernel_spmd
    nc = _get_program()
    in_maps = _prep_inputs(**{k: np.asarray(v) for k, v in inputs.items()})
    res = run_bass_kernel_spmd(nc, in_maps, list(range(NCORES))).results
    x_h = np.concatenate([res[c]["outh"] for c in range(NCORES)], axis=0)
    x_e = np.concatenate([res[c]["oute"] for c in range(NCORES)], axis=0)
    return (x_h, x_e)



# revision 29
# speedup vs baseline: 1.4778x; 1.4778x over previous
"""Trainium2 Bass kernel for the gnn_message_passing block (8 NeuronCores).

Strategy (per core c, owning 512 global rows r = c*512..(c+1)*512):
  - Host rotates x rows by -r0 (owned rows first), pre-transposes the owned
    x block (raw feature-major), and converts all large tensors to 16-bit
    (bf16 weights/mats/x, f16 w2) -- halves HBM traffic and removes all
    casting DMAs (pure HWDGE byte copies).
  - rmsnorm scale-invariance tricks: the per-node inverse-rms r[n] is
    needed exactly (a) multiplied into the node-major aggregation inputs
    h_sb/e_sb and (b) as the exp() scale of the per-node softmax for the
    raw (non-aggregated) q/k sources.  The v path needs NO normalization:
    v = h@Wv for every module, so x_att is uniformly scaled by r_h[n],
    which the (scale-invariant) second rmsnorm removes exactly.  The
    feature-major q/k/v stationary sources are therefore RAW transposed x
    from the host -- no on-chip transposes for them.
  - rsqrt = exp(-0.5*ln(.)): ln+exp live in ONE activation table set
    (natural_log_exp_and_others) together with square/copy, so the whole
    kernel needs only ~4 ACT table loads (vs ~100 when mixing sqrt):
    nl_exp era -> gelu(FFN h) -> nl_exp era -> gelu(FFN e).
  - The five N x N aggregations (adj@h shared by modules 0/4, four proj@k)
    run in fp8e4m3 DoubleRow mode (0.5 PE cycles/row): mats are pre-scaled
    host-side into fp8 range (adj*4096, proj*64; inverse folded into the
    consuming module's wq/wk) and the normalized h/e aggregation inputs are
    quantized to fp8 mega-tiles.  DoubleRow dst is ISA-limited to psum
    partitions 0-63, so each 64-feature odd half accumulates in a scratch
    bank and is rebased to partitions 64-127 via a cheap identity matmul.
    Costs ~6e-3 extra rel-err (1e-2 total vs the 2e-2 gate), halves
    aggregation PE time AND mat HBM traffic.  512x512 projections bf16.
  - Per-node 8-head SDPA on DVE in f16 2x mode: broadcast-AP multiplies +
    halving-tree reduces (measured: TT=2x, TensorReduce/Pool/TTR=1x, so
    trees beat single reduces).  Softmax exp on ACT with per-node scale.
    GpSimd offload of the av-multiply is wired behind POOL_AV_MOD but OFF:
    the cost model charges Pool TT at 0.83ns/elem while real Q7 hardware
    runs 2-input ops ~2.2ns/elem, so the offload only looks good in sim.
  - Module processing interleaves with aggregations so PE and DVE overlap:
    loads | mod 1,5 | agg0 | mod 0,4 | agg1,2 | mod 2 | agg3 | mod 3 |
    agg4 | FFN-h | mod 7 | mod 6 + pipelined FFN-e
  - FFN-e is fully pipelined per node-block via a tile callback inside the
    last module: each 128-node block runs rmsnorm2 -> transpose -> w1 ->
    gelu -> w2 -> DMA while the remaining SDPA tiles are still on DVE.
    The per-ffb gelu bias is applied by one 4-deep indicator matmul per
    4-ffb group so the gelu runs 512 wide (4 ACT ops/block instead of 16).
    FFN-h stays serial (pipelining it into module 7 would thrash the
    gelu/exp ACT table sets) but its rmsnorm2 squares and out-copies run
    on DVE, which is idle there, so module 7's softmax exps aren't queued
    behind them on ACT.
  - DMA queues: x/mats/outputs/weights on SP (HWDGE), modules 1/5 weights
    on the ACT queue (ahead of the SP load burst at body start); only the
    casting bias loads remain on POOL SWDGE.  POOL otherwise does the
    cheap per-node normalization multiplies (1-input ops are line-rate on
    real Q7; 2-input ops are not).
  - CoreSim: 344.0us (DVE-bound: DVE busy ~307us, PE ~220, ACT ~200) vs
    350.8us for the bf16 predecessor; measured HW (repeat-differential)
    331us for the predecessor.
"""
import numpy as np

N = 4096
E = 512
H = 8
D = 64
FF = 2048
P = 128
NCORES = 8
RPC = N // NCORES  # 512 rows per core
NT = N // P        # 32 tiles over all nodes
LT = RPC // P      # 4 local tiles
EPS = float(np.finfo(np.float32).eps)
# run the av broadcast-multiply on GpSimd for tiles with
# t % POOL_AV_MOD == POOL_AV_PHASE (engine-balance knob; real-HW GpSimd
# tensor_tensor is ~4x slower than the cost model says, so keep this off)
POOL_AV_MOD = 1000
POOL_AV_PHASE = 1

_PROGRAM_CACHE = {}


def _split_big_waits(nc, mybir):
    """walrus in this toolchain rejects multi-wait instructions; cap at 1
    (2 for EventSemaphore), chaining the excess as EventSemaphores."""
    for f in nc.m.functions:
        for bb in f.blocks:
            insts = list(bb.instructions)
            out = []
            changed = False
            for inst in insts:
                si = inst.sync_info
                waits = list(si.on_wait) if si and si.on_wait else []
                cap = 2 if isinstance(inst, mybir.InstEventSemaphore) else 1
                if len(waits) > cap:
                    extra, keep = waits[:-cap], waits[-cap:]
                    for ci in range(0, len(extra), 2):
                        ev = mybir.InstEventSemaphore(name=f"{inst.name}-evw{ci}")
                        ev.engine = inst.engine
                        ev.sync_info = mybir.SyncInfo(on_wait=extra[ci:ci + 2],
                                                      on_update=[])
                        out.append(ev)
                    si.on_wait = keep
                    changed = True
                out.append(inst)
            if changed:
                bb.instructions[:] = out


def _build_program(repeat=1):
    import concourse.bass as bass
    import concourse.tile as tile
    from concourse import mybir
    from concourse.masks import make_identity
    from contextlib import ExitStack

    f32 = mybir.dt.float32
    f32r = mybir.dt.float32r
    f16 = mybir.dt.float16
    bf16 = mybir.dt.bfloat16
    f8 = mybir.dt.float8e4
    AF = mybir.ActivationFunctionType
    OP = mybir.AluOpType
    AX = mybir.AxisListType
    DR = mybir.MatmulPerfMode.DoubleRow

    def bc(t, dims, off=0):
        return bass.AP(tensor=t.tensor, offset=t.offset + off,
                       ap=[list(t.ap[0])] + [[s, c] for (s, c) in dims])

    nc = bass.Bass()

    xn_d = nc.declare_dram_parameter("xn", [N, E], bf16, isOutput=False)
    xe_d = nc.declare_dram_parameter("xe", [N, E], bf16, isOutput=False)
    xnT_d = nc.declare_dram_parameter("xnT", [E, RPC], bf16, isOutput=False)
    xeT_d = nc.declare_dram_parameter("xeT", [E, RPC], bf16, isOutput=False)
    mat_d = [nc.declare_dram_parameter(f"mat{i}", [N, RPC], f8, isOutput=False)
             for i in range(5)]
    wq_d = nc.declare_dram_parameter("wqT", [H, E, E], bf16, isOutput=False)
    wk_d = nc.declare_dram_parameter("wkT", [H, E, E], bf16, isOutput=False)
    wv_d = nc.declare_dram_parameter("wvT", [H, E, E], bf16, isOutput=False)
    w1h_d = nc.declare_dram_parameter("w1hT", [E, FF], bf16, isOutput=False)
    w2h_d = nc.declare_dram_parameter("w2hT", [FF, E], f16, isOutput=False)
    w1e_d = nc.declare_dram_parameter("w1eT", [E, FF], bf16, isOutput=False)
    w2e_d = nc.declare_dram_parameter("w2eT", [FF, E], f16, isOutput=False)
    b1h_d = nc.declare_dram_parameter("b1h", [FF], f32, isOutput=False)
    b2h_d = nc.declare_dram_parameter("b2h", [E], f32, isOutput=False)
    b1e_d = nc.declare_dram_parameter("b1e", [FF], f32, isOutput=False)
    b2e_d = nc.declare_dram_parameter("b2e", [E], f32, isOutput=False)
    outh_d = nc.declare_dram_parameter("outh", [RPC, E], f32, isOutput=True)
    oute_d = nc.declare_dram_parameter("oute", [RPC, E], f32, isOutput=True)

    with tile.TileContext(nc, pool_alloc_mode="queue") as tc, ExitStack() as ctx:
        consts = ctx.enter_context(tc.tile_pool(name="consts", bufs=1))
        eps_t = consts.tile([P, 1], f32)
        nc.vector.memset(eps_t, EPS)
        lnwarm = consts.tile([P, 1], f32)
        # first ACT op: pins the natural_log_exp_and_others table set
        nc.scalar.activation(out=lnwarm[:], in_=eps_t[:], func=AF.Ln)
        ident = consts.tile([P, P], f16)
        make_identity(nc, ident)
        ones1f = consts.tile([1, P], f32)
        nc.gpsimd.memset(ones1f, 1.0)
        ones1 = consts.tile([1, P], f32r)
        nc.scalar.copy(ones1[:], ones1f[:])
        b1h_t = consts.tile([P, FF // P], f32)
        b1e_t = consts.tile([P, FF // P], f32)
        b2h_t = consts.tile([1, E], f32r)
        b2e_t = consts.tile([1, E], f32r)
        # indicator eye(4) (x) ones(128) + b1 rows: lets the pipelined FFN
        # apply the per-ffb gelu bias via one 4-deep matmul so 4 ffb blocks
        # share a single wide gelu
        ind4f = consts.tile([4, E], f32)
        nc.gpsimd.memset(ind4f, 0.0)
        for q in range(4):
            # partition-offset writes need a DMA (engines can't start at
            # partition q); tiny one-time init copies
            nc.sync.dma_start(out=ind4f[q:q + 1, q * P:(q + 1) * P],
                              in_=ones1f[0:1, :])
        ind4 = consts.tile([4, E], f32r)
        nc.scalar.copy(ind4[:], ind4f[:])
        b1eq = consts.tile([4, 4 * P], f32r)

        # whole-program pools
        statp = ctx.enter_context(tc.tile_pool(name="stat", bufs=1))
        rawp = ctx.enter_context(tc.tile_pool(name="raw", bufs=1))
        sqscp = ctx.enter_context(tc.tile_pool(name="sqsc", bufs=1))
        xTlp = ctx.enter_context(tc.tile_pool(name="xTl", bufs=1))
        wpool = ctx.enter_context(tc.tile_pool(name="wts", bufs=1))
        qkvp = ctx.enter_context(tc.tile_pool(name="qkv", bufs=1))
        tmpp = ctx.enter_context(tc.tile_pool(name="sdtmp", bufs=1))
        smp = ctx.enter_context(tc.tile_pool(name="sdsm", bufs=2))
        accp = ctx.enter_context(tc.tile_pool(name="acc", bufs=1))
        matgp = ctx.enter_context(tc.tile_pool(name="matg", bufs=2))
        fwts = ctx.enter_context(tc.tile_pool(name="fwts", bufs=1))
        psp = ctx.enter_context(tc.tile_pool(name="ps", bufs=1, space="PSUM"))

        def body():
            # per-iteration stat tiles
            ssq_h = statp.tile([P, NT], f32, tag="ssqh", name="ssqh")
            ssq_e = statp.tile([P, NT], f32, tag="ssqe", name="ssqe")
            lnsc = statp.tile([P, NT], f32, tag="lnsc", name="lnsc")
            rh_t = statp.tile([P, NT], f32, tag="rh", name="rh")
            re_t = statp.tile([P, NT], f32, tag="re", name="re")
            re2_t = statp.tile([P, LT], f32, tag="re2", name="re2")

            xnTl = [xTlp.tile([P, RPC], bf16, tag=f"xnT{fc}", name=f"xnT{fc}")
                    for fc in range(4)]
            xeTl = [xTlp.tile([P, RPC], bf16, tag=f"xeT{fc}", name=f"xeT{fc}")
                    for fc in range(4)]
            for fc in range(4):
                nc.sync.dma_start(out=xeTl[fc][:],
                                  in_=xeT_d[fc * P:(fc + 1) * P, :])
            for fc in range(4):
                nc.sync.dma_start(out=xnTl[fc][:],
                                  in_=xnT_d[fc * P:(fc + 1) * P, :])

            acc_h = [accp.tile([P, E], f16, tag=f"ah{t}", name=f"ah{t}")
                     for t in range(LT)]
            acc_e = [accp.tile([P, E], f16, tag=f"ae{t}", name=f"ae{t}")
                     for t in range(LT)]

            def load_group(x_d, sb8, ssq, rdst, g0, dve_stats=False):
                """Load 4 node-major tiles, square-accum stats, rsqrt the 4
                columns, normalize into the fp8 aggregation mega-tile sb8.
                dve_stats puts the squares on DVE (for the first group, while
                DVE is idle and ACT latency gates the first module)."""
                xg = rawp.tile([P, 4 * E], bf16, tag="xraw", bufs=2,
                               name="xraw")
                nc.sync.dma_start(
                    out=xg.rearrange("p (t e) -> p t e", e=E),
                    in_=x_d[g0 * P:(g0 + 4) * P, :].rearrange(
                        "(t p) e -> p t e", p=P))
                for t in range(4):
                    ti = g0 + t
                    scr = sqscp.tile([P, E], f16, tag="sq", bufs=1, name="sq")
                    if dve_stats:
                        nc.vector.scalar_tensor_tensor(
                            out=scr[:], in0=xg[:, t * E:(t + 1) * E],
                            scalar=0.0, in1=xg[:, t * E:(t + 1) * E],
                            op0=OP.add, op1=OP.mult,
                            accum_out=ssq[:, ti:ti + 1])
                    else:
                        nc.scalar.activation(out=scr[:],
                                             in_=xg[:, t * E:(t + 1) * E],
                                             func=AF.Square,
                                             accum_out=ssq[:, ti:ti + 1])
                nc.scalar.activation(out=lnsc[:, g0:g0 + 4],
                                     in_=ssq[:, g0:g0 + 4], func=AF.Ln,
                                     scale=1.0 / E, bias=eps_t[:])
                nc.scalar.activation(out=rdst[:, g0:g0 + 4],
                                     in_=lnsc[:, g0:g0 + 4], func=AF.Exp,
                                     scale=-0.5)
                for t in range(4):
                    ti = g0 + t
                    nc.gpsimd.tensor_scalar_mul(sb8[:, ti * E:(ti + 1) * E],
                                                xg[:, t * E:(t + 1) * E],
                                                rdst[:, ti:ti + 1])

            def aggregate(mi, src8, aggpool):
                """4 feature-major bf16 [128, 512] blocks of mat_mi @ x.

                fp8e4 DoubleRow matmuls: both operands fp8 (mats pre-scaled
                host-side; inverse scale folded into wq/wk), 2 node-tiles
                (256 contraction rows) per pass at 0.5 cycles/row.  Output
                partition limit is 64, so each psum bank holds a pair of
                64-feature blocks at partition offsets 0/64 and the bf16
                copy-out still reads one [128, 512] bank."""
                pss = [psp.tile([P, E], f32, tag=f"agps{b}", name=f"agps{b}")
                       for b in range(4)]
                scr = [psp.tile([64, E], f32, tag="projps", bufs=4,
                                name=f"agsc{b}") for b in range(4)]
                for g in range(8):
                    mt = matgp.tile([P, 4 * RPC], f8, tag="matg", name="matg")
                    nc.sync.dma_start(
                        out=mt.rearrange("p (t e) -> p t e", e=RPC),
                        in_=mat_d[mi][g * 4 * P:(g + 1) * 4 * P, :].rearrange(
                            "(t p) e -> p t e", p=P))
                    for tp in range(2):
                        pair = g * 2 + tp
                        for b in range(4):
                            # DoubleRow dst is ISA-limited to partitions
                            # 0-63: even feature-half accumulates in
                            # pss[b][0:64], odd half in a scratch bank and
                            # is rebased to [64:128] afterwards.
                            for half in range(2):
                                dst = pss[b][0:64, :] if half == 0 else scr[b][:]
                                nc.tensor.matmul(
                                    dst,
                                    lhsT=bc(src8, [(E, 2), (1, 64)],
                                            off=pair * 2 * E + (2 * b + half) * 64),
                                    rhs=bc(mt, [(RPC, 2), (1, RPC)],
                                           off=tp * 2 * RPC),
                                    start=(pair == 0), stop=(pair == 15),
                                    perf_mode=DR,
                                    tile_position=(0, 0),
                                    skip_group_check=True)
                outt = []
                for b in range(4):
                    t64 = sqscp.tile([64, E], f16, tag="agt64", bufs=2,
                                     name="agt64")
                    nc.scalar.copy(t64[:], scr[b][:])
                    nc.tensor.matmul(pss[b][64:128, :],
                                     lhsT=ident[0:64, 0:64], rhs=t64[:],
                                     start=True, stop=True,
                                     tile_position=(0, 64),
                                     skip_group_check=True)
                    at = aggpool.tile([P, E], bf16, tag=f"ag{mi}_{b}",
                                      name=f"ag{mi}_{b}")
                    nc.scalar.copy(at[:], pss[b][:])
                    outt.append(at)
                return outt

            def module(m, qsrcT, ksrcT, branch_att, rsc, first, warmln=False,
                       tile_cb=None, wdma=None):
                w_ts = {}
                for (dram, nm) in ((wq_d, "wq"), (wk_d, "wk"), (wv_d, "wv")):
                    wt = wpool.tile([P, 4 * E], bf16, tag=nm,
                                    bufs=(1 if nm == "wv" else 2),
                                    name=f"w_{nm}")
                    (wdma or nc.sync).dma_start(
                        out=wt.rearrange("p (fc e) -> p fc e", e=E),
                        in_=dram[m].rearrange("(fc p) e -> p fc e", p=P))
                    w_ts[nm] = wt
                if warmln:
                    # re-pin the ln+exp table set after a gelu era
                    nc.scalar.activation(out=lnwarm[:], in_=eps_t[:],
                                         func=AF.Ln)

                # per-tile interleave (q_b, k_b, v_b) so tile 0's SDPA can
                # start after 12 matmuls instead of 36
                q_sb, k_sb, v_sb = [], [], []
                for b in range(LT):
                    for (srcT, wnm, lst) in ((qsrcT, "wq", q_sb),
                                             (ksrcT, "wk", k_sb),
                                             (xnTl, "wv", v_sb)):
                        wt = w_ts[wnm]
                        ps = psp.tile([P, E], f32, tag="projps", bufs=4,
                                      name="projps")
                        for fc in range(4):
                            nc.tensor.matmul(
                                ps[:],
                                lhsT=srcT[fc][:, b * P:(b + 1) * P],
                                rhs=wt[:, fc * E:(fc + 1) * E],
                                start=(fc == 0), stop=(fc == 3))
                        dt = qkvp.tile([P, E], f16, tag=f"{wnm}_{b}",
                                       bufs=(2 if wnm == "wq" else 1),
                                       name=f"qkv{b}")
                        if wnm == "wv":
                            nc.scalar.copy(bc(dt, [(1, 8), (8, 64)]), ps[:])
                        else:
                            nc.scalar.copy(dt[:], ps[:])
                        lst.append(dt)

                for t in range(LT):
                    q_t, k_t, v_t = q_sb[t], k_sb[t], v_sb[t]
                    tmp = tmpp.tile([P, H * H * D], f16, tag="sdpa", bufs=1,
                                    name="sdpa")
                    nc.vector.tensor_tensor(
                        out=bc(tmp, [(512, 8), (64, 8), (1, 64)]),
                        in0=bc(q_t, [(64, 8), (0, 8), (1, 64)]),
                        in1=bc(k_t, [(0, 8), (64, 8), (1, 64)]),
                        op=OP.mult)
                    for dd in (32, 16, 8, 4, 2):
                        nc.vector.tensor_tensor(
                            out=bc(tmp, [(64, 64), (1, dd)]),
                            in0=bc(tmp, [(64, 64), (1, dd)]),
                            in1=bc(tmp, [(64, 64), (1, dd)], off=dd),
                            op=OP.add)
                    s_t = smp.tile([P, H * H], f16, tag="s", name="s")
                    nc.vector.tensor_tensor(
                        out=s_t[:],
                        in0=bc(tmp, [(64, 64)]),
                        in1=bc(tmp, [(64, 64)], off=1),
                        op=OP.add)
                    ex_t = smp.tile([P, H * H], f16, tag="ex", name="ex")
                    nc.scalar.activation(out=ex_t[:], in_=s_t[:], func=AF.Exp,
                                         scale=rsc[:, t:t + 1])
                    den = smp.tile([P, H], f32, tag="den", name="den")
                    nc.vector.tensor_reduce(
                        out=den[:], in_=ex_t.rearrange("p (h g) -> p h g", g=H),
                        axis=AX.X, op=OP.add)
                    rden = smp.tile([P, H], f32, tag="rden", name="rden")
                    nc.vector.reciprocal(out=rden[:], in_=den[:])
                    a_t = smp.tile([P, H * H], f16, tag="a", name="a")
                    nc.vector.tensor_tensor(
                        out=bc(a_t, [(8, 8), (1, 8)]),
                        in0=bc(ex_t, [(8, 8), (1, 8)]),
                        in1=bc(rden, [(1, 8), (0, 8)]),
                        op=OP.mult)
                    tmp2 = tmpp.tile([P, H * H * D], f16, tag="sdpa2", bufs=2,
                                     name="sdpa2")
                    av_eng = nc.gpsimd if (t % POOL_AV_MOD == POOL_AV_PHASE) \
                        else nc.vector
                    av_eng.tensor_tensor(
                        out=bc(tmp2, [(512, 8), (8, 64), (1, 8)]),
                        in0=bc(a_t, [(8, 8), (0, 64), (1, 8)]),
                        in1=bc(v_t, [(0, 8), (8, 64), (1, 8)]),
                        op=OP.mult)
                    for gg in (4, 2):
                        nc.vector.tensor_tensor(
                            out=bc(tmp2, [(8, 512), (1, gg)]),
                            in0=bc(tmp2, [(8, 512), (1, gg)]),
                            in1=bc(tmp2, [(8, 512), (1, gg)], off=gg),
                            op=OP.add)
                    if first:
                        nc.vector.tensor_tensor(
                            out=branch_att[t][:],
                            in0=bc(tmp2, [(8, 512)]),
                            in1=bc(tmp2, [(8, 512)], off=1),
                            op=OP.add)
                    else:
                        rt = smp.tile([P, E], f16, tag="avred", bufs=1,
                                      name="avred")
                        nc.vector.tensor_tensor(
                            out=rt[:],
                            in0=bc(tmp2, [(8, 512)]),
                            in1=bc(tmp2, [(8, 512)], off=1),
                            op=OP.add)
                        nc.vector.tensor_tensor(out=branch_att[t][:],
                                                in0=branch_att[t][:],
                                                in1=rt[:], op=OP.add)
                    if tile_cb is not None:
                        tile_cb(t)

            def ffn_wload(w1_dram, w2_dram):
                """Weight tiles + DMAs; emit early to hide the transfers."""
                w1_ts = []
                for half in range(2):
                    HW1 = FF // 2
                    w1_t = fwts.tile([P, 4 * HW1], bf16, tag=f"w1_{half}",
                                     name=f"w1_{half}")
                    nc.sync.dma_start(
                        out=w1_t.rearrange("p (fc e) -> p fc e", e=HW1),
                        in_=w1_dram[:, half * HW1:(half + 1) * HW1].rearrange(
                            "(fc p) e -> p fc e", p=P))
                    w1_ts.append(w1_t)
                return (w1_ts, w2_dram)

            def ffn(branch_att, wtiles, b1_t, b2_t, out_dram):
                w1_ts, w2_dram = wtiles
                with tc.tile_pool(name="ffn_sb", bufs=1) as fsb:
                    w2_t = fsb.tile([P, 16 * E], f16, tag="w2", name="w2")
                    nc.sync.dma_start(
                        out=w2_t.rearrange("p (fc e) -> p fc e", e=E),
                        in_=w2_dram[:, :].rearrange("(fc p) e -> p fc e", p=P))
                    # rmsnorm2: stats + rsqrt + normalize (f16); squares on
                    # DVE (idle here) so module 7's softmax exps aren't stuck
                    # behind them in the ACT queue
                    ssq2 = fsb.tile([P, LT], f32, tag="fss", name="fss")
                    for t in range(LT):
                        scr = sqscp.tile([P, E], f16, tag="sq", bufs=1,
                                         name="fsq")
                        nc.vector.scalar_tensor_tensor(
                            out=scr[:], in0=branch_att[t][:], scalar=0.0,
                            in1=branch_att[t][:], op0=OP.add, op1=OP.mult,
                            accum_out=ssq2[:, t:t + 1])
                    ln2 = fsb.tile([P, LT], f32, tag="fln", name="fln")
                    nc.scalar.activation(out=ln2[:], in_=ssq2[:], func=AF.Ln,
                                         scale=1.0 / E, bias=eps_t[:])
                    rs2 = fsb.tile([P, LT], f32, tag="frs", name="frs")
                    nc.scalar.activation(out=rs2[:], in_=ln2[:], func=AF.Exp,
                                         scale=-0.5)
                    xn_tiles = []
                    for t in range(LT):
                        xt = fsb.tile([P, E], f16, tag=f"fx{t}", name=f"fx{t}")
                        nc.gpsimd.tensor_scalar_mul(xt[:], branch_att[t][:],
                                                    rs2[:, t:t + 1])
                        xn_tiles.append(xt)
                    xnT = []
                    for fc in range(4):
                        ps = psp.tile([P, RPC], f16, tag="agps0", name="ftr")
                        for t in range(4):
                            nc.tensor.transpose(ps[:, t * P:(t + 1) * P],
                                                xn_tiles[t][:, fc * P:(fc + 1) * P],
                                                ident[:])
                        xt = fsb.tile([P, RPC], bf16, tag=f"fxT{fc}",
                                      name=f"fxT{fc}")
                        nc.scalar.copy(xt[:], ps[:])
                        xnT.append(xt)
                    g1 = []
                    HW1 = FF // 2
                    for half in range(2):
                        w1_t = w1_ts[half]
                        for fb in range(HW1 // P):
                            ffb = half * (HW1 // P) + fb
                            ps = psp.tile([P, RPC], f32, tag=f"agps{1 + ffb % 2}",
                                          name="fps1")
                            for fc in range(4):
                                nc.tensor.matmul(
                                    ps[:],
                                    lhsT=w1_t[:, fc * HW1 + fb * P:
                                              fc * HW1 + (fb + 1) * P],
                                    rhs=xnT[fc][:],
                                    start=(fc == 0), stop=(fc == 3))
                            gt = fsb.tile([P, RPC], f16, tag=f"g1_{ffb}",
                                          name=f"g1_{ffb}")
                            nc.scalar.activation(out=gt[:], in_=ps[:],
                                                 func=AF.Gelu,
                                                 bias=b1_t[:, ffb:ffb + 1],
                                                 scale=1.0)
                            g1.append(gt)
                    for b in range(LT):
                        ps = psp.tile([P, E], f32, tag="agps3", name="fps2")
                        for ffc in range(FF // P):
                            nc.tensor.matmul(
                                ps[:],
                                lhsT=g1[ffc][:, b * P:(b + 1) * P],
                                rhs=w2_t[:, ffc * E:(ffc + 1) * E],
                                start=(ffc == 0), stop=False)
                        nc.tensor.matmul(ps[:], lhsT=ones1[:], rhs=b2_t[:],
                                         start=False, stop=True)
                        ob = fsb.tile([P, E], f32, tag="fo", bufs=2, name="fo")
                        nc.vector.tensor_scalar_mul(ob[:], ps[:], 1.0)
                        nc.sync.dma_start(
                            out=out_dram[b * P:(b + 1) * P, :], in_=ob[:])

            def mk_ffn_pipe(branch_att, w1_ts, w2_dram, b1q, b2_t, out_dram,
                            fsb):
                """Per-tile pipelined FFN: the returned callback is invoked
                inside the final module after each tile's SDPA, so nearly the
                whole FFN runs under the module's remaining SDPA work.  Node
                block t is pushed through rmsnorm2 -> transpose -> w1 -> gelu
                -> w2 as soon as its attention accumulator is final; only
                block 3's chain is exposed at the end."""
                HW1 = FF // 2
                st = {
                    "ssq2": fsb.tile([P, LT], f32, tag="fss", name="fss"),
                    "ln2": fsb.tile([P, LT], f32, tag="fln", name="fln"),
                    "rs2": fsb.tile([P, LT], f32, tag="frs", name="frs"),
                    "psT": [psp.tile([P, RPC], f16, tag=f"agps{fc}",
                                     name=f"eftr{fc}") for fc in range(4)],
                    "xnT": [fsb.tile([P, RPC], bf16, tag=f"fxT{fc}",
                                     name=f"fxT{fc}") for fc in range(4)],
                    "g1g": [fsb.tile([P, 4 * RPC], f16, tag=f"g1g_{g}",
                                     name=f"g1g_{g}") for g in range(4)],
                    "w2": fsb.tile([P, 16 * E], f16, tag="w2", name="w2"),
                    "ps2": [None] * LT,
                }

                def out_block(b):
                    ob = fsb.tile([P, E], f32, tag="fo", bufs=2, name="fo")
                    nc.vector.tensor_scalar_mul(ob[:], st["ps2"][b][:], 1.0)
                    nc.sync.dma_start(out=out_dram[b * P:(b + 1) * P, :],
                                      in_=ob[:])

                def block_chain(b):
                    """w1 -> gelu -> w2 for node block b (emitted one SDPA
                    tile later so its ACT ops sit behind the next exp in the
                    queue and can't stall the softmax chain)."""
                    lo, hi = b * P, (b + 1) * P
                    if b > 0:
                        out_block(b - 1)
                    for g in range(4):
                        ps4 = psp.tile([P, 4 * P], f32, tag="projps", bufs=4,
                                       name="efps1")
                        # the bias matmul OPENS the bank (start=True zeroes
                        # the whole 2KB zero-region, so per-quarter starts
                        # would wipe sibling quarters); quarters accumulate
                        nc.tensor.matmul(ps4[:],
                                         lhsT=b1q[0:4, g * P:(g + 1) * P],
                                         rhs=ind4[:], start=True, stop=False,
                                         skip_group_check=True)
                        for qq in range(4):
                            ffb = g * 4 + qq
                            w1_t = w1_ts[ffb // 8]
                            fb = ffb % 8
                            for fc in range(4):
                                nc.tensor.matmul(
                                    ps4[:, qq * P:(qq + 1) * P],
                                    lhsT=w1_t[:, fc * HW1 + fb * P:
                                              fc * HW1 + (fb + 1) * P],
                                    rhs=st["xnT"][fc][:, lo:hi],
                                    start=False,
                                    stop=(qq == 3 and fc == 3),
                                    skip_group_check=True)
                        nc.scalar.activation(
                            out=bc(st["g1g"][g], [(RPC, 4), (1, P)], off=lo),
                            in_=ps4[:], func=AF.Gelu, scale=1.0)
                    ps2 = psp.tile([P, E], f32, tag="projps", bufs=4,
                                    name="efps2")
                    for ffc in range(FF // P):
                        nc.tensor.matmul(
                            ps2[:],
                            lhsT=st["g1g"][ffc // 4][:, (ffc % 4) * RPC + lo:
                                                     (ffc % 4) * RPC + hi],
                            rhs=st["w2"][:, ffc * E:(ffc + 1) * E],
                            start=(ffc == 0), stop=False)
                    nc.tensor.matmul(ps2[:], lhsT=ones1[:], rhs=b2_t[:],
                                     start=False, stop=True)
                    st["ps2"][b] = ps2

                def cb(t):
                    if t == 0:
                        nc.sync.dma_start(
                            out=st["w2"].rearrange("p (fc e) -> p fc e", e=E),
                            in_=w2_dram[:, :].rearrange("(fc p) e -> p fc e",
                                                        p=P))
                    lo, hi = t * P, (t + 1) * P
                    scr = sqscp.tile([P, E], f16, tag="sq", bufs=1, name="esq")
                    nc.vector.scalar_tensor_tensor(
                        out=scr[:], in0=branch_att[t][:], scalar=0.0,
                        in1=branch_att[t][:], op0=OP.add, op1=OP.mult,
                        accum_out=st["ssq2"][:, t:t + 1])
                    nc.scalar.activation(out=st["ln2"][:, t:t + 1],
                                         in_=st["ssq2"][:, t:t + 1],
                                         func=AF.Ln, scale=1.0 / E,
                                         bias=eps_t[:])
                    nc.scalar.activation(out=st["rs2"][:, t:t + 1],
                                         in_=st["ln2"][:, t:t + 1],
                                         func=AF.Exp, scale=-0.5)
                    xt = fsb.tile([P, E], f16, tag="fx", bufs=2, name="fx")
                    nc.gpsimd.tensor_scalar_mul(xt[:], branch_att[t][:],
                                                st["rs2"][:, t:t + 1])
                    for fc in range(4):
                        nc.tensor.transpose(st["psT"][fc][:, lo:hi],
                                            xt[:, fc * P:(fc + 1) * P],
                                            ident[:])
                        nc.scalar.copy(st["xnT"][fc][:, lo:hi],
                                       st["psT"][fc][:, lo:hi])
                    if t > 0:
                        block_chain(t - 1)

                def tail():
                    block_chain(LT - 1)
                    out_block(LT - 1)

                return cb, tail

            # ======== emission order (the schedule) ========
            agg12_stack = ExitStack()
            agg12p = agg12_stack.enter_context(tc.tile_pool(name="agg12", bufs=1))
            agg34_stack = ExitStack()
            agg34p = agg34_stack.enter_context(tc.tile_pool(name="agg34", bufs=1))
            e_stack = ExitStack()
            epool = e_stack.enter_context(tc.tile_pool(name="epool", bufs=1))
            h_stack = ExitStack()
            hpool = h_stack.enter_context(tc.tile_pool(name="hpool", bufs=1))
            agg0_stack = ExitStack()
            agg0p = agg0_stack.enter_context(tc.tile_pool(name="agg0p", bufs=1))

            h_sb = hpool.tile([P, NT * E], f8, tag="h8", name="h8")
            e_sb = epool.tile([P, NT * E], f8, tag="e8", name="e8")

            # local tiles first (rsqrt cols 0-3 feed the exp scales);
            # xe first: modules 1/5 need re^2 + xeTl before anything else
            load_group(xe_d, e_sb, ssq_e, re_t, 0, dve_stats=True)
            nc.scalar.activation(out=re2_t[:], in_=re_t[:, 0:LT],
                                 func=AF.Square)

            # modules 1 and 5 need no aggregates - start DVE early
            # (xn group 0's ACT squares go after module 5 so they cannot
            # delay module 5's softmax exps in the ACT queue)
            # first two modules' weights via the ACT DGE queue so they don't
            # sit behind the body's x-tile loads in the SP queue at startup
            module(1, xeTl, xeTl, acc_h, re2_t, first=True, wdma=nc.scalar)
            module(5, xeTl, xeTl, acc_e, re2_t, first=True, wdma=nc.scalar)
            load_group(xn_d, h_sb, ssq_h, rh_t, 0)

            for g0 in range(4, NT, 4):
                load_group(xn_d, h_sb, ssq_h, rh_t, g0)
            for g0 in range(4, NT // 2, 4):
                load_group(xe_d, e_sb, ssq_e, re_t, g0)

            nc.sync.dma_start(out=b1h_t,
                                in_=b1h_d[:].rearrange("(c p) -> p c", p=P))
            nc.sync.dma_start(out=b1e_t,
                                in_=b1e_d[:].rearrange("(c p) -> p c", p=P))
            nc.gpsimd.dma_start(out=b2h_t,
                                in_=b2h_d[:].rearrange("(a e) -> a e", a=1))
            nc.gpsimd.dma_start(out=b2e_t,
                                in_=b2e_d[:].rearrange("(a e) -> a e", a=1))
            nc.gpsimd.dma_start(
                out=b1eq.rearrange("q (g p) -> q g p", p=P),
                in_=b1e_d[:].rearrange("(g q p) -> q g p", q=4, p=P))

            agg0 = aggregate(0, h_sb, agg0p)
            module(0, agg0, xnTl, acc_h, rh_t, first=False)
            module(4, agg0, xnTl, acc_e, rh_t, first=False)

            agg1 = aggregate(1, h_sb, agg12p)
            agg2 = aggregate(2, h_sb, agg12p)
            agg0_stack.close()

            module(2, xeTl, agg1, acc_h, re_t, first=False)
            h_stack.close()

            for g0 in range(NT // 2, NT, 4):
                load_group(xe_d, e_sb, ssq_e, re_t, g0)
            agg3 = aggregate(3, e_sb, agg34p)
            wt_h = ffn_wload(w1h_d, w2h_d)
            module(3, xnTl, agg3, acc_h, rh_t, first=False)

            agg4 = aggregate(4, e_sb, agg34p)
            e_stack.close()

            ffn(acc_h, wt_h, b1h_t, b2h_t, outh_d)
            wt_e = ffn_wload(w1e_d, w2e_d)
            module(7, xnTl, agg4, acc_e, rh_t, first=False, warmln=True)
            agg34_stack.close()
            with tc.tile_pool(name="ffnE_sb", bufs=1) as fsbE:
                ecb, etail = mk_ffn_pipe(acc_e, wt_e[0], wt_e[1], b1eq,
                                         b2e_t, oute_d, fsbE)
                module(6, xeTl, agg2, acc_e, re_t, first=False, tile_cb=ecb)
                etail()
            agg12_stack.close()

        for _ in range(repeat):
            body()

    _split_big_waits(nc, mybir)
    return nc


def _get_program():
    if "nc" not in _PROGRAM_CACHE:
        _PROGRAM_CACHE["nc"] = _build_program()
    return _PROGRAM_CACHE["nc"]


def _prep_inputs(x_node, x_edge, adj, Wq, Wk, Wv,
                 proj_he_h, proj_eh_h, proj_he_e, proj_eh_e,
                 rms1_h, rms1_e, rms2_h,
                 w1_h, b1_h, w2_h, b2_h, w1_e, b1_e, w2_e, b2_e):
    """Per-core input dicts. Weight folding + row rotation happen here."""
    from ml_dtypes import bfloat16, float8_e4m3
    f = np.float32
    bf = bfloat16
    f16 = np.float16
    f8 = float8_e4m3
    # mats quantized to fp8e4m3, pre-scaled into a good fp8 range; the
    # inverse scale is folded into the wq/wk of the module consuming the
    # aggregate (mat0=adj -> q of modules 0/4; mat1..4 -> k of 2,6,3,7).
    MSC = [4096.0, 64.0, 64.0, 64.0, 64.0]
    qsc = [1.0 / MSC[0], 1, 1, 1, 1.0 / MSC[0], 1, 1, 1]
    ksc = [1, 1, 1.0 / MSC[1], 1.0 / MSC[3], 1, 1, 1.0 / MSC[2], 1.0 / MSC[4]]
    wsrc_q = [rms1_h, rms1_e, rms1_e, rms1_h, rms1_h, rms1_e, rms1_e, rms1_h]
    wsrc_k = [rms1_h, rms1_e, rms1_h, rms1_e, rms1_h, rms1_e, rms1_h, rms1_e]
    wqT = np.stack([(Wq[m].T * wsrc_q[m][:, None]) * (0.125 * qsc[m])
                    for m in range(H)])
    wkT = np.stack([(Wk[m].T * wsrc_k[m][:, None]) * ksc[m] for m in range(H)])
    wvT = np.stack([Wv[m].T * rms1_h[:, None] for m in range(H)])
    w1hT = np.ascontiguousarray((w1_h * rms2_h[None, :]).T.astype(bf))
    w1eT = np.ascontiguousarray((w1_e * rms2_h[None, :]).T.astype(bf))
    w2hT = np.ascontiguousarray(w2_h.T.astype(f16))
    w2eT = np.ascontiguousarray(w2_e.T.astype(f16))
    mats = [adj, proj_eh_h, proj_eh_e, proj_he_h, proj_he_e]

    shared = dict(wqT=np.ascontiguousarray(wqT.astype(bf)),
                  wkT=np.ascontiguousarray(wkT.astype(bf)),
                  wvT=np.ascontiguousarray(wvT.astype(bf)),
                  w1hT=w1hT, w2hT=w2hT, w1eT=w1eT, w2eT=w2eT,
                  b1h=b1_h.astype(f), b2h=b2_h.astype(f),
                  b1e=b1_e.astype(f), b2e=b2_e.astype(f))
    xn_bf = x_node.astype(bf)
    xe_bf = x_edge.astype(bf)
    in_maps = []
    for c in range(NCORES):
        r0 = c * RPC
        m = dict(shared)
        m["xn"] = np.ascontiguousarray(np.roll(xn_bf, -r0, axis=0))
        m["xe"] = np.ascontiguousarray(np.roll(xe_bf, -r0, axis=0))
        m["xnT"] = np.ascontiguousarray(xn_bf[r0:r0 + RPC].T)
        m["xeT"] = np.ascontiguousarray(xe_bf[r0:r0 + RPC].T)
        for i, mat in enumerate(mats):
            mt = np.ascontiguousarray(
                (mat[r0:r0 + RPC].T * MSC[i]).astype(f8))  # [N, RPC]
            m[f"mat{i}"] = np.ascontiguousarray(np.roll(mt, -r0, axis=0))
        in_maps.append(m)
    return in_maps


def kernel(**inputs):
    from concourse.bass_utils import run_bass_kernel_spmd
    nc = _get_program()
    in_maps = _prep_inputs(**{k: np.asarray(v) for k, v in inputs.items()})
    res = run_bass_kernel_spmd(nc, in_maps, list(range(NCORES))).results
    x_h = np.concatenate([res[c]["outh"] for c in range(NCORES)], axis=0)
    x_e = np.concatenate([res[c]["oute"] for c in range(NCORES)], axis=0)
    return (x_h, x_e)



# revision 30
# speedup vs baseline: 8.2220x; 5.5637x over previous
"""Trainium2 Bass kernel for the gnn_message_passing block (8 NeuronCores).

Strategy (per core c, owning 512 global rows r = c*512..(c+1)*512):
  - Host rotates x rows by -r0 (owned rows first), pre-transposes the owned
    x block (raw feature-major), and converts all large tensors to 16-bit
    (bf16 weights/mats/x, f16 w2) -- halves HBM traffic and removes all
    casting DMAs (pure HWDGE byte copies).
  - rmsnorm scale-invariance tricks: the per-node inverse-rms r[n] is
    needed exactly (a) multiplied into the node-major aggregation inputs
    h_sb/e_sb and (b) as the exp() scale of the per-node softmax for the
    raw (non-aggregated) q/k sources.  The v path needs NO normalization:
    v = h@Wv for every module, so x_att is uniformly scaled by r_h[n],
    which the (scale-invariant) second rmsnorm removes exactly.  The
    feature-major q/k/v stationary sources are therefore RAW transposed x
    from the host -- no on-chip transposes for them.
  - rsqrt = exp(-0.5*ln(.)): ln+exp live in ONE activation table set
    (natural_log_exp_and_others) together with square/copy, so the whole
    kernel needs only ~4 ACT table loads (vs ~100 when mixing sqrt):
    nl_exp era -> gelu(FFN h) -> nl_exp era -> gelu(FFN e).
  - The five N x N aggregations (adj@h shared by modules 0/4, four proj@k)
    run in fp8e4m3 DoubleRow mode (0.5 PE cycles/row): mats are pre-scaled
    host-side into fp8 range (adj*4096, proj*64; inverse folded into the
    consuming module's wq/wk) and the normalized h/e aggregation inputs are
    quantized to fp8 mega-tiles.  DoubleRow dst is ISA-limited to psum
    partitions 0-63, so each 64-feature odd half accumulates in a scratch
    bank and is rebased to partitions 64-127 via a cheap identity matmul.
    Costs ~6e-3 extra rel-err (1e-2 total vs the 2e-2 gate), halves
    aggregation PE time AND mat HBM traffic.  512x512 projections bf16.
  - Per-node 8-head SDPA on DVE in f16 2x mode: broadcast-AP multiplies +
    halving-tree reduces (measured: TT=2x, TensorReduce/Pool/TTR=1x, so
    trees beat single reduces).  Softmax exp on ACT with per-node scale.
    GpSimd offload of the av-multiply is wired behind POOL_AV_MOD but OFF:
    the cost model charges Pool TT at 0.83ns/elem while real Q7 hardware
    runs 2-input ops ~2.2ns/elem, so the offload only looks good in sim.
  - Module processing interleaves with aggregations so PE and DVE overlap:
    loads | mod 1,5 | agg0 | mod 0,4 | agg1,2 | mod 2 | agg3 | mod 3 |
    agg4 | FFN-h | mod 7 | mod 6 + pipelined FFN-e
  - FFN-e is fully pipelined per node-block via a tile callback inside the
    last module: each 128-node block runs rmsnorm2 -> transpose -> w1 ->
    gelu -> w2 -> DMA while the remaining SDPA tiles are still on DVE.
    The per-ffb gelu bias is applied by one 4-deep indicator matmul per
    4-ffb group so the gelu runs 512 wide (4 ACT ops/block instead of 16).
    FFN-h stays serial (pipelining it into module 7 would thrash the
    gelu/exp ACT table sets) but its rmsnorm2 squares and out-copies run
    on DVE, which is idle there, so module 7's softmax exps aren't queued
    behind them on ACT.
  - DMA queues: x/mats/outputs/weights on SP (HWDGE), modules 1/5 weights
    on the ACT queue (ahead of the SP load burst at body start); only the
    casting bias loads remain on POOL SWDGE.  POOL otherwise does the
    cheap per-node normalization multiplies (1-input ops are line-rate on
    real Q7; 2-input ops are not).
  - CoreSim: 343.4us (DVE-bound: DVE busy ~307us, PE ~220, ACT ~200) vs
    350.8us for the bf16 predecessor; measured HW (repeat-differential)
    331us for the predecessor.  HW-verified rel-err 9.44e-3.
"""
import numpy as np

N = 4096
E = 512
H = 8
D = 64
FF = 2048
P = 128
NCORES = 8
RPC = N // NCORES  # 512 rows per core
NT = N // P        # 32 tiles over all nodes
LT = RPC // P      # 4 local tiles
EPS = float(np.finfo(np.float32).eps)
# run the av broadcast-multiply on GpSimd for tiles with
# t % POOL_AV_MOD == POOL_AV_PHASE (engine-balance knob; real-HW GpSimd
# tensor_tensor is ~4x slower than the cost model says, so keep this off)
POOL_AV_MOD = 1000
POOL_AV_PHASE = 1

_PROGRAM_CACHE = {}


def _split_big_waits(nc, mybir):
    """walrus in this toolchain rejects multi-wait instructions; cap at 1
    (2 for EventSemaphore), chaining the excess as EventSemaphores."""
    for f in nc.m.functions:
        for bb in f.blocks:
            insts = list(bb.instructions)
            out = []
            changed = False
            for inst in insts:
                si = inst.sync_info
                waits = list(si.on_wait) if si and si.on_wait else []
                cap = 2 if isinstance(inst, mybir.InstEventSemaphore) else 1
                if len(waits) > cap:
                    extra, keep = waits[:-cap], waits[-cap:]
                    for ci in range(0, len(extra), 2):
                        ev = mybir.InstEventSemaphore(name=f"{inst.name}-evw{ci}")
                        ev.engine = inst.engine
                        ev.sync_info = mybir.SyncInfo(on_wait=extra[ci:ci + 2],
                                                      on_update=[])
                        out.append(ev)
                    si.on_wait = keep
                    changed = True
                out.append(inst)
            if changed:
                bb.instructions[:] = out


def _build_program(repeat=1):
    import concourse.bass as bass
    import concourse.tile as tile
    from concourse import mybir
    from concourse.masks import make_identity
    from contextlib import ExitStack

    f32 = mybir.dt.float32
    f32r = mybir.dt.float32r
    f16 = mybir.dt.float16
    bf16 = mybir.dt.bfloat16
    f8 = mybir.dt.float8e4
    AF = mybir.ActivationFunctionType
    OP = mybir.AluOpType
    AX = mybir.AxisListType
    DR = mybir.MatmulPerfMode.DoubleRow

    def bc(t, dims, off=0):
        return bass.AP(tensor=t.tensor, offset=t.offset + off,
                       ap=[list(t.ap[0])] + [[s, c] for (s, c) in dims])

    nc = bass.Bass()

    xn_d = nc.declare_dram_parameter("xn", [N, E], bf16, isOutput=False)
    xe_d = nc.declare_dram_parameter("xe", [N, E], bf16, isOutput=False)
    xnT_d = nc.declare_dram_parameter("xnT", [E, RPC], bf16, isOutput=False)
    xeT_d = nc.declare_dram_parameter("xeT", [E, RPC], bf16, isOutput=False)
    mat_d = [nc.declare_dram_parameter(f"mat{i}", [N, RPC], f8, isOutput=False)
             for i in range(5)]
    wq_d = nc.declare_dram_parameter("wqT", [H, E, E], bf16, isOutput=False)
    wk_d = nc.declare_dram_parameter("wkT", [H, E, E], bf16, isOutput=False)
    wv_d = nc.declare_dram_parameter("wvT", [H, E, E], bf16, isOutput=False)
    w1h_d = nc.declare_dram_parameter("w1hT", [E, FF], bf16, isOutput=False)
    w2h_d = nc.declare_dram_parameter("w2hT", [FF, E], f16, isOutput=False)
    w1e_d = nc.declare_dram_parameter("w1eT", [E, FF], bf16, isOutput=False)
    w2e_d = nc.declare_dram_parameter("w2eT", [FF, E], f16, isOutput=False)
    b1h_d = nc.declare_dram_parameter("b1h", [FF], f32, isOutput=False)
    b2h_d = nc.declare_dram_parameter("b2h", [E], f32, isOutput=False)
    b1e_d = nc.declare_dram_parameter("b1e", [FF], f32, isOutput=False)
    b2e_d = nc.declare_dram_parameter("b2e", [E], f32, isOutput=False)
    outh_d = nc.declare_dram_parameter("outh", [RPC, E], f32, isOutput=True)
    oute_d = nc.declare_dram_parameter("oute", [RPC, E], f32, isOutput=True)

    with tile.TileContext(nc, pool_alloc_mode="queue") as tc, ExitStack() as ctx:
        consts = ctx.enter_context(tc.tile_pool(name="consts", bufs=1))
        eps_t = consts.tile([P, 1], f32)
        nc.vector.memset(eps_t, EPS)
        lnwarm = consts.tile([P, 1], f32)
        # first ACT op: pins the natural_log_exp_and_others table set
        nc.scalar.activation(out=lnwarm[:], in_=eps_t[:], func=AF.Ln)
        ident = consts.tile([P, P], f16)
        make_identity(nc, ident)
        ones1f = consts.tile([1, P], f32)
        nc.gpsimd.memset(ones1f, 1.0)
        ones1 = consts.tile([1, P], f32r)
        nc.scalar.copy(ones1[:], ones1f[:])
        b1h_t = consts.tile([P, FF // P], f32)
        b1e_t = consts.tile([P, FF // P], f32)
        b2h_t = consts.tile([1, E], f32r)
        b2e_t = consts.tile([1, E], f32r)
        # indicator eye(4) (x) ones(128) + b1 rows: lets the pipelined FFN
        # apply the per-ffb gelu bias via one 4-deep matmul so 4 ffb blocks
        # share a single wide gelu
        ind4f = consts.tile([4, E], f32)
        nc.gpsimd.memset(ind4f, 0.0)
        for q in range(4):
            # partition-offset writes need a DMA (engines can't start at
            # partition q); tiny one-time init copies
            nc.sync.dma_start(out=ind4f[q:q + 1, q * P:(q + 1) * P],
                              in_=ones1f[0:1, :])
        ind4 = consts.tile([4, E], f32r)
        nc.scalar.copy(ind4[:], ind4f[:])
        b1eq = consts.tile([4, 4 * P], f32r)

        # whole-program pools
        statp = ctx.enter_context(tc.tile_pool(name="stat", bufs=1))
        rawp = ctx.enter_context(tc.tile_pool(name="raw", bufs=1))
        sqscp = ctx.enter_context(tc.tile_pool(name="sqsc", bufs=1))
        xTlp = ctx.enter_context(tc.tile_pool(name="xTl", bufs=1))
        wpool = ctx.enter_context(tc.tile_pool(name="wts", bufs=1))
        qkvp = ctx.enter_context(tc.tile_pool(name="qkv", bufs=1))
        tmpp = ctx.enter_context(tc.tile_pool(name="sdtmp", bufs=1))
        smp = ctx.enter_context(tc.tile_pool(name="sdsm", bufs=2))
        accp = ctx.enter_context(tc.tile_pool(name="acc", bufs=1))
        matgp = ctx.enter_context(tc.tile_pool(name="matg", bufs=2))
        fwts = ctx.enter_context(tc.tile_pool(name="fwts", bufs=1))
        psp = ctx.enter_context(tc.tile_pool(name="ps", bufs=1, space="PSUM"))

        def body():
            # per-iteration stat tiles
            ssq_h = statp.tile([P, NT], f32, tag="ssqh", name="ssqh")
            ssq_e = statp.tile([P, NT], f32, tag="ssqe", name="ssqe")
            lnsc = statp.tile([P, NT], f32, tag="lnsc", name="lnsc")
            rh_t = statp.tile([P, NT], f32, tag="rh", name="rh")
            re_t = statp.tile([P, NT], f32, tag="re", name="re")
            re2_t = statp.tile([P, LT], f32, tag="re2", name="re2")

            xnTl = [xTlp.tile([P, RPC], bf16, tag=f"xnT{fc}", name=f"xnT{fc}")
                    for fc in range(4)]
            xeTl = [xTlp.tile([P, RPC], bf16, tag=f"xeT{fc}", name=f"xeT{fc}")
                    for fc in range(4)]
            for fc in range(4):
                nc.sync.dma_start(out=xeTl[fc][:],
                                  in_=xeT_d[fc * P:(fc + 1) * P, :])
            for fc in range(4):
                nc.sync.dma_start(out=xnTl[fc][:],
                                  in_=xnT_d[fc * P:(fc + 1) * P, :])

            acc_h = [accp.tile([P, E], f16, tag=f"ah{t}", name=f"ah{t}")
                     for t in range(LT)]
            acc_e = [accp.tile([P, E], f16, tag=f"ae{t}", name=f"ae{t}")
                     for t in range(LT)]

            def load_group(x_d, sb8, ssq, rdst, g0, dve_stats=False):
                """Load 4 node-major tiles, square-accum stats, rsqrt the 4
                columns, normalize into the fp8 aggregation mega-tile sb8.
                dve_stats puts the squares on DVE (for the first group, while
                DVE is idle and ACT latency gates the first module)."""
                xg = rawp.tile([P, 4 * E], bf16, tag="xraw", bufs=2,
                               name="xraw")
                nc.sync.dma_start(
                    out=xg.rearrange("p (t e) -> p t e", e=E),
                    in_=x_d[g0 * P:(g0 + 4) * P, :].rearrange(
                        "(t p) e -> p t e", p=P))
                for t in range(4):
                    ti = g0 + t
                    scr = sqscp.tile([P, E], f16, tag="sq", bufs=1, name="sq")
                    if dve_stats:
                        nc.vector.scalar_tensor_tensor(
                            out=scr[:], in0=xg[:, t * E:(t + 1) * E],
                            scalar=0.0, in1=xg[:, t * E:(t + 1) * E],
                            op0=OP.add, op1=OP.mult,
                            accum_out=ssq[:, ti:ti + 1])
                    else:
                        nc.scalar.activation(out=scr[:],
                                             in_=xg[:, t * E:(t + 1) * E],
                                             func=AF.Square,
                                             accum_out=ssq[:, ti:ti + 1])
                nc.scalar.activation(out=lnsc[:, g0:g0 + 4],
                                     in_=ssq[:, g0:g0 + 4], func=AF.Ln,
                                     scale=1.0 / E, bias=eps_t[:])
                nc.scalar.activation(out=rdst[:, g0:g0 + 4],
                                     in_=lnsc[:, g0:g0 + 4], func=AF.Exp,
                                     scale=-0.5)
                for t in range(4):
                    ti = g0 + t
                    nc.gpsimd.tensor_scalar_mul(sb8[:, ti * E:(ti + 1) * E],
                                                xg[:, t * E:(t + 1) * E],
                                                rdst[:, ti:ti + 1])

            def aggregate(mi, src8, aggpool):
                """4 feature-major bf16 [128, 512] blocks of mat_mi @ x.

                fp8e4 DoubleRow matmuls: both operands fp8 (mats pre-scaled
                host-side; inverse scale folded into wq/wk), 2 node-tiles
                (256 contraction rows) per pass at 0.5 cycles/row.  Output
                partition limit is 64, so each psum bank holds a pair of
                64-feature blocks at partition offsets 0/64 and the bf16
                copy-out still reads one [128, 512] bank."""
                pss = [psp.tile([P, E], f32, tag=f"agps{b}", name=f"agps{b}")
                       for b in range(4)]
                scr = [psp.tile([64, E], f32, tag="projps", bufs=4,
                                name=f"agsc{b}") for b in range(4)]
                for g in range(8):
                    mt = matgp.tile([P, 4 * RPC], f8, tag="matg", name="matg")
                    nc.sync.dma_start(
                        out=mt.rearrange("p (t e) -> p t e", e=RPC),
                        in_=mat_d[mi][g * 4 * P:(g + 1) * 4 * P, :].rearrange(
                            "(t p) e -> p t e", p=P))
                    for tp in range(2):
                        pair = g * 2 + tp
                        for b in range(4):
                            # DoubleRow dst is ISA-limited to partitions
                            # 0-63: even feature-half accumulates in
                            # pss[b][0:64], odd half in a scratch bank and
                            # is rebased to [64:128] afterwards.
                            for half in range(2):
                                dst = pss[b][0:64, :] if half == 0 else scr[b][:]
                                nc.tensor.matmul(
                                    dst,
                                    lhsT=bc(src8, [(E, 2), (1, 64)],
                                            off=pair * 2 * E + (2 * b + half) * 64),
                                    rhs=bc(mt, [(RPC, 2), (1, RPC)],
                                           off=tp * 2 * RPC),
                                    start=(pair == 0), stop=(pair == 15),
                                    perf_mode=DR,
                                    tile_position=(0, 0),
                                    skip_group_check=True)
                outt = []
                for b in range(4):
                    t64 = sqscp.tile([64, E], f16, tag="agt64", bufs=2,
                                     name="agt64")
                    nc.scalar.copy(t64[:], scr[b][:])
                    nc.tensor.matmul(pss[b][64:128, :],
                                     lhsT=ident[0:64, 0:64], rhs=t64[:],
                                     start=True, stop=True,
                                     tile_position=(0, 64),
                                     skip_group_check=True)
                    at = aggpool.tile([P, E], bf16, tag=f"ag{mi}_{b}",
                                      name=f"ag{mi}_{b}")
                    nc.scalar.copy(at[:], pss[b][:])
                    outt.append(at)
                return outt

            def module(m, qsrcT, ksrcT, branch_att, rsc, first, warmln=False,
                       tile_cb=None, wdma=None):
                w_ts = {}
                for (dram, nm) in ((wq_d, "wq"), (wk_d, "wk"), (wv_d, "wv")):
                    wt = wpool.tile([P, 4 * E], bf16, tag=nm,
                                    bufs=(1 if nm == "wv" else 2),
                                    name=f"w_{nm}")
                    (wdma or nc.sync).dma_start(
                        out=wt.rearrange("p (fc e) -> p fc e", e=E),
                        in_=dram[m].rearrange("(fc p) e -> p fc e", p=P))
                    w_ts[nm] = wt
                if warmln:
                    # re-pin the ln+exp table set after a gelu era
                    nc.scalar.activation(out=lnwarm[:], in_=eps_t[:],
                                         func=AF.Ln)

                # per-tile interleave (q_b, k_b, v_b) so tile 0's SDPA can
                # start after 12 matmuls instead of 36
                q_sb, k_sb, v_sb = [], [], []
                for b in range(LT):
                    for (srcT, wnm, lst) in ((qsrcT, "wq", q_sb),
                                             (ksrcT, "wk", k_sb),
                                             (xnTl, "wv", v_sb)):
                        wt = w_ts[wnm]
                        ps = psp.tile([P, E], f32, tag="projps", bufs=4,
                                      name="projps")
                        for fc in range(4):
                            nc.tensor.matmul(
                                ps[:],
                                lhsT=srcT[fc][:, b * P:(b + 1) * P],
                                rhs=wt[:, fc * E:(fc + 1) * E],
                                start=(fc == 0), stop=(fc == 3))
                        dt = qkvp.tile([P, E], f16, tag=f"{wnm}_{b}",
                                       bufs=(2 if wnm == "wq" else 1),
                                       name=f"qkv{b}")
                        if wnm == "wv":
                            nc.scalar.copy(bc(dt, [(1, 8), (8, 64)]), ps[:])
                        else:
                            nc.scalar.copy(dt[:], ps[:])
                        lst.append(dt)

                for t in range(LT):
                    q_t, k_t, v_t = q_sb[t], k_sb[t], v_sb[t]
                    tmp = tmpp.tile([P, H * H * D], f16, tag="sdpa", bufs=1,
                                    name="sdpa")
                    nc.vector.tensor_tensor(
                        out=bc(tmp, [(512, 8), (64, 8), (1, 64)]),
                        in0=bc(q_t, [(64, 8), (0, 8), (1, 64)]),
                        in1=bc(k_t, [(0, 8), (64, 8), (1, 64)]),
                        op=OP.mult)
                    for dd in (32, 16, 8, 4, 2):
                        nc.vector.tensor_tensor(
                            out=bc(tmp, [(64, 64), (1, dd)]),
                            in0=bc(tmp, [(64, 64), (1, dd)]),
                            in1=bc(tmp, [(64, 64), (1, dd)], off=dd),
                            op=OP.add)
                    s_t = smp.tile([P, H * H], f16, tag="s", name="s")
                    nc.vector.tensor_tensor(
                        out=s_t[:],
                        in0=bc(tmp, [(64, 64)]),
                        in1=bc(tmp, [(64, 64)], off=1),
                        op=OP.add)
                    ex_t = smp.tile([P, H * H], f16, tag="ex", name="ex")
                    nc.scalar.activation(out=ex_t[:], in_=s_t[:], func=AF.Exp,
                                         scale=rsc[:, t:t + 1])
                    den = smp.tile([P, H], f32, tag="den", name="den")
                    nc.vector.tensor_reduce(
                        out=den[:], in_=ex_t.rearrange("p (h g) -> p h g", g=H),
                        axis=AX.X, op=OP.add)
                    rden = smp.tile([P, H], f32, tag="rden", name="rden")
                    nc.vector.reciprocal(out=rden[:], in_=den[:])
                    a_t = smp.tile([P, H * H], f16, tag="a", name="a")
                    nc.vector.tensor_tensor(
                        out=bc(a_t, [(8, 8), (1, 8)]),
                        in0=bc(ex_t, [(8, 8), (1, 8)]),
                        in1=bc(rden, [(1, 8), (0, 8)]),
                        op=OP.mult)
                    tmp2 = tmpp.tile([P, H * H * D], f16, tag="sdpa2", bufs=2,
                                     name="sdpa2")
                    av_eng = nc.gpsimd if (t % POOL_AV_MOD == POOL_AV_PHASE) \
                        else nc.vector
                    av_eng.tensor_tensor(
                        out=bc(tmp2, [(512, 8), (8, 64), (1, 8)]),
                        in0=bc(a_t, [(8, 8), (0, 64), (1, 8)]),
                        in1=bc(v_t, [(0, 8), (8, 64), (1, 8)]),
                        op=OP.mult)
                    for gg in (4, 2):
                        nc.vector.tensor_tensor(
                            out=bc(tmp2, [(8, 512), (1, gg)]),
                            in0=bc(tmp2, [(8, 512), (1, gg)]),
                            in1=bc(tmp2, [(8, 512), (1, gg)], off=gg),
                            op=OP.add)
                    if first:
                        nc.vector.tensor_tensor(
                            out=branch_att[t][:],
                            in0=bc(tmp2, [(8, 512)]),
                            in1=bc(tmp2, [(8, 512)], off=1),
                            op=OP.add)
                    else:
                        rt = smp.tile([P, E], f16, tag="avred", bufs=1,
                                      name="avred")
                        nc.vector.tensor_tensor(
                            out=rt[:],
                            in0=bc(tmp2, [(8, 512)]),
                            in1=bc(tmp2, [(8, 512)], off=1),
                            op=OP.add)
                        nc.vector.tensor_tensor(out=branch_att[t][:],
                                                in0=branch_att[t][:],
                                                in1=rt[:], op=OP.add)
                    if tile_cb is not None:
                        tile_cb(t)

            def ffn_wload(w1_dram, w2_dram):
                """Weight tiles + DMAs; emit early to hide the transfers."""
                w1_ts = []
                for half in range(2):
                    HW1 = FF // 2
                    w1_t = fwts.tile([P, 4 * HW1], bf16, tag=f"w1_{half}",
                                     name=f"w1_{half}")
                    nc.sync.dma_start(
                        out=w1_t.rearrange("p (fc e) -> p fc e", e=HW1),
                        in_=w1_dram[:, half * HW1:(half + 1) * HW1].rearrange(
                            "(fc p) e -> p fc e", p=P))
                    w1_ts.append(w1_t)
                return (w1_ts, w2_dram)

            def ffn(branch_att, wtiles, b1_t, b2_t, out_dram):
                w1_ts, w2_dram = wtiles
                with tc.tile_pool(name="ffn_sb", bufs=1) as fsb:
                    w2_t = fsb.tile([P, 16 * E], f16, tag="w2", name="w2")
                    nc.sync.dma_start(
                        out=w2_t.rearrange("p (fc e) -> p fc e", e=E),
                        in_=w2_dram[:, :].rearrange("(fc p) e -> p fc e", p=P))
                    # rmsnorm2: stats + rsqrt + normalize (f16); squares on
                    # DVE (idle here) so module 7's softmax exps aren't stuck
                    # behind them in the ACT queue
                    ssq2 = fsb.tile([P, LT], f32, tag="fss", name="fss")
                    for t in range(LT):
                        scr = sqscp.tile([P, E], f16, tag="sq", bufs=1,
                                         name="fsq")
                        nc.vector.scalar_tensor_tensor(
                            out=scr[:], in0=branch_att[t][:], scalar=0.0,
                            in1=branch_att[t][:], op0=OP.add, op1=OP.mult,
                            accum_out=ssq2[:, t:t + 1])
                    ln2 = fsb.tile([P, LT], f32, tag="fln", name="fln")
                    nc.scalar.activation(out=ln2[:], in_=ssq2[:], func=AF.Ln,
                                         scale=1.0 / E, bias=eps_t[:])
                    rs2 = fsb.tile([P, LT], f32, tag="frs", name="frs")
                    nc.scalar.activation(out=rs2[:], in_=ln2[:], func=AF.Exp,
                                         scale=-0.5)
                    xn_tiles = []
                    for t in range(LT):
                        xt = fsb.tile([P, E], f16, tag=f"fx{t}", name=f"fx{t}")
                        nc.gpsimd.tensor_scalar_mul(xt[:], branch_att[t][:],
                                                    rs2[:, t:t + 1])
                        xn_tiles.append(xt)
                    xnT = []
                    for fc in range(4):
                        ps = psp.tile([P, RPC], f16, tag="agps0", name="ftr")
                        for t in range(4):
                            nc.tensor.transpose(ps[:, t * P:(t + 1) * P],
                                                xn_tiles[t][:, fc * P:(fc + 1) * P],
                                                ident[:])
                        xt = fsb.tile([P, RPC], bf16, tag=f"fxT{fc}",
                                      name=f"fxT{fc}")
                        nc.scalar.copy(xt[:], ps[:])
                        xnT.append(xt)
                    g1 = []
                    HW1 = FF // 2
                    for half in range(2):
                        w1_t = w1_ts[half]
                        for fb in range(HW1 // P):
                            ffb = half * (HW1 // P) + fb
                            ps = psp.tile([P, RPC], f32, tag=f"agps{1 + ffb % 2}",
                                          name="fps1")
                            for fc in range(4):
                                nc.tensor.matmul(
                                    ps[:],
                                    lhsT=w1_t[:, fc * HW1 + fb * P:
                                              fc * HW1 + (fb + 1) * P],
                                    rhs=xnT[fc][:],
                                    start=(fc == 0), stop=(fc == 3))
                            gt = fsb.tile([P, RPC], f16, tag=f"g1_{ffb}",
                                          name=f"g1_{ffb}")
                            nc.scalar.activation(out=gt[:], in_=ps[:],
                                                 func=AF.Gelu,
                                                 bias=b1_t[:, ffb:ffb + 1],
                                                 scale=1.0)
                            g1.append(gt)
                    for b in range(LT):
                        ps = psp.tile([P, E], f32, tag="agps3", name="fps2")
                        for ffc in range(FF // P):
                            nc.tensor.matmul(
                                ps[:],
                                lhsT=g1[ffc][:, b * P:(b + 1) * P],
                                rhs=w2_t[:, ffc * E:(ffc + 1) * E],
                                start=(ffc == 0), stop=False)
                        nc.tensor.matmul(ps[:], lhsT=ones1[:], rhs=b2_t[:],
                                         start=False, stop=True)
                        ob = fsb.tile([P, E], f32, tag="fo", bufs=2, name="fo")
                        nc.vector.tensor_scalar_mul(ob[:], ps[:], 1.0)
                        nc.sync.dma_start(
                            out=out_dram[b * P:(b + 1) * P, :], in_=ob[:])

            def mk_ffn_pipe(branch_att, w1_ts, w2_dram, b1q, b2_t, out_dram,
                            fsb):
                """Per-tile pipelined FFN: the returned callback is invoked
                inside the final module after each tile's SDPA, so nearly the
                whole FFN runs under the module's remaining SDPA work.  Node
                block t is pushed through rmsnorm2 -> transpose -> w1 -> gelu
                -> w2 as soon as its attention accumulator is final; only
                block 3's chain is exposed at the end."""
                HW1 = FF // 2
                st = {
                    "ssq2": fsb.tile([P, LT], f32, tag="fss", name="fss"),
                    "ln2": fsb.tile([P, LT], f32, tag="fln", name="fln"),
                    "rs2": fsb.tile([P, LT], f32, tag="frs", name="frs"),
                    "psT": [psp.tile([P, RPC], f16, tag=f"agps{fc}",
                                     name=f"eftr{fc}") for fc in range(4)],
                    "xnT": [fsb.tile([P, RPC], bf16, tag=f"fxT{fc}",
                                     name=f"fxT{fc}") for fc in range(4)],
                    "g1g": [fsb.tile([P, 4 * RPC], f16, tag=f"g1g_{g}",
                                     name=f"g1g_{g}") for g in range(4)],
                    "w2": fsb.tile([P, 16 * E], f16, tag="w2", name="w2"),
                    "ps2": [None] * LT,
                }

                def out_block(b):
                    ob = fsb.tile([P, E], f32, tag="fo", bufs=2, name="fo")
                    nc.vector.tensor_scalar_mul(ob[:], st["ps2"][b][:], 1.0)
                    nc.sync.dma_start(out=out_dram[b * P:(b + 1) * P, :],
                                      in_=ob[:])

                def block_chain(b):
                    """w1 -> gelu -> w2 for node block b (emitted one SDPA
                    tile later so its ACT ops sit behind the next exp in the
                    queue and can't stall the softmax chain)."""
                    lo, hi = b * P, (b + 1) * P
                    if b > 0:
                        out_block(b - 1)
                    for g in range(4):
                        ps4 = psp.tile([P, 4 * P], f32, tag="projps", bufs=4,
                                       name="efps1")
                        # the bias matmul OPENS the bank (start=True zeroes
                        # the whole 2KB zero-region, so per-quarter starts
                        # would wipe sibling quarters); quarters accumulate
                        nc.tensor.matmul(ps4[:],
                                         lhsT=b1q[0:4, g * P:(g + 1) * P],
                                         rhs=ind4[:], start=True, stop=False,
                                         skip_group_check=True)
                        for qq in range(4):
                            ffb = g * 4 + qq
                            w1_t = w1_ts[ffb // 8]
                            fb = ffb % 8
                            for fc in range(4):
                                nc.tensor.matmul(
                                    ps4[:, qq * P:(qq + 1) * P],
                                    lhsT=w1_t[:, fc * HW1 + fb * P:
                                              fc * HW1 + (fb + 1) * P],
                                    rhs=st["xnT"][fc][:, lo:hi],
                                    start=False,
                                    stop=(qq == 3 and fc == 3),
                                    skip_group_check=True)
                        nc.scalar.activation(
                            out=bc(st["g1g"][g], [(RPC, 4), (1, P)], off=lo),
                            in_=ps4[:], func=AF.Gelu, scale=1.0)
                    ps2 = psp.tile([P, E], f32, tag="projps", bufs=4,
                                    name="efps2")
                    for ffc in range(FF // P):
                        nc.tensor.matmul(
                            ps2[:],
                            lhsT=st["g1g"][ffc // 4][:, (ffc % 4) * RPC + lo:
                                                     (ffc % 4) * RPC + hi],
                            rhs=st["w2"][:, ffc * E:(ffc + 1) * E],
                            start=(ffc == 0), stop=False)
                    nc.tensor.matmul(ps2[:], lhsT=ones1[:], rhs=b2_t[:],
                                     start=False, stop=True)
                    st["ps2"][b] = ps2

                def cb(t):
                    if t == 0:
                        nc.sync.dma_start(
                            out=st["w2"].rearrange("p (fc e) -> p fc e", e=E),
                            in_=w2_dram[:, :].rearrange("(fc p) e -> p fc e",
                                                        p=P))
                    lo, hi = t * P, (t + 1) * P
                    scr = sqscp.tile([P, E], f16, tag="sq", bufs=1, name="esq")
                    nc.vector.scalar_tensor_tensor(
                        out=scr[:], in0=branch_att[t][:], scalar=0.0,
                        in1=branch_att[t][:], op0=OP.add, op1=OP.mult,
                        accum_out=st["ssq2"][:, t:t + 1])
                    nc.scalar.activation(out=st["ln2"][:, t:t + 1],
                                         in_=st["ssq2"][:, t:t + 1],
                                         func=AF.Ln, scale=1.0 / E,
                                         bias=eps_t[:])
                    nc.scalar.activation(out=st["rs2"][:, t:t + 1],
                                         in_=st["ln2"][:, t:t + 1],
                                         func=AF.Exp, scale=-0.5)
                    xt = fsb.tile([P, E], f16, tag="fx", bufs=2, name="fx")
                    nc.gpsimd.tensor_scalar_mul(xt[:], branch_att[t][:],
                                                st["rs2"][:, t:t + 1])
                    for fc in range(4):
                        nc.tensor.transpose(st["psT"][fc][:, lo:hi],
                                            xt[:, fc * P:(fc + 1) * P],
                                            ident[:])
                        nc.scalar.copy(st["xnT"][fc][:, lo:hi],
                                       st["psT"][fc][:, lo:hi])
                    if t > 0:
                        block_chain(t - 1)

                def tail():
                    block_chain(LT - 1)
                    out_block(LT - 1)

                return cb, tail

            # ======== emission order (the schedule) ========
            agg12_stack = ExitStack()
            agg12p = agg12_stack.enter_context(tc.tile_pool(name="agg12", bufs=1))
            agg34_stack = ExitStack()
            agg34p = agg34_stack.enter_context(tc.tile_pool(name="agg34", bufs=1))
            e_stack = ExitStack()
            epool = e_stack.enter_context(tc.tile_pool(name="epool", bufs=1))
            h_stack = ExitStack()
            hpool = h_stack.enter_context(tc.tile_pool(name="hpool", bufs=1))
            agg0_stack = ExitStack()
            agg0p = agg0_stack.enter_context(tc.tile_pool(name="agg0p", bufs=1))

            h_sb = hpool.tile([P, NT * E], f8, tag="h8", name="h8")
            e_sb = epool.tile([P, NT * E], f8, tag="e8", name="e8")

            # local tiles first (rsqrt cols 0-3 feed the exp scales);
            # xe first: modules 1/5 need re^2 + xeTl before anything else
            load_group(xe_d, e_sb, ssq_e, re_t, 0, dve_stats=True)
            nc.scalar.activation(out=re2_t[:], in_=re_t[:, 0:LT],
                                 func=AF.Square)

            # modules 1 and 5 need no aggregates - start DVE early
            # (xn group 0's ACT squares go after module 5 so they cannot
            # delay module 5's softmax exps in the ACT queue)
            # first two modules' weights via the ACT DGE queue so they don't
            # sit behind the body's x-tile loads in the SP queue at startup
            module(1, xeTl, xeTl, acc_h, re2_t, first=True, wdma=nc.scalar)
            module(5, xeTl, xeTl, acc_e, re2_t, first=True, wdma=nc.scalar)
            load_group(xn_d, h_sb, ssq_h, rh_t, 0)

            for g0 in range(4, NT, 4):
                load_group(xn_d, h_sb, ssq_h, rh_t, g0)
            for g0 in range(4, NT // 2, 4):
                load_group(xe_d, e_sb, ssq_e, re_t, g0)

            nc.sync.dma_start(out=b1h_t,
                                in_=b1h_d[:].rearrange("(c p) -> p c", p=P))
            nc.sync.dma_start(out=b1e_t,
                                in_=b1e_d[:].rearrange("(c p) -> p c", p=P))
            nc.gpsimd.dma_start(out=b2h_t,
                                in_=b2h_d[:].rearrange("(a e) -> a e", a=1))
            nc.gpsimd.dma_start(out=b2e_t,
                                in_=b2e_d[:].rearrange("(a e) -> a e", a=1))
            nc.gpsimd.dma_start(
                out=b1eq.rearrange("q (g p) -> q g p", p=P),
                in_=b1e_d[:].rearrange("(g q p) -> q g p", q=4, p=P))

            agg0 = aggregate(0, h_sb, agg0p)
            module(0, agg0, xnTl, acc_h, rh_t, first=False)
            module(4, agg0, xnTl, acc_e, rh_t, first=False)

            agg1 = aggregate(1, h_sb, agg12p)
            agg2 = aggregate(2, h_sb, agg12p)
            agg0_stack.close()

            module(2, xeTl, agg1, acc_h, re_t, first=False)
            h_stack.close()

            for g0 in range(NT // 2, NT, 4):
                load_group(xe_d, e_sb, ssq_e, re_t, g0)
            agg3 = aggregate(3, e_sb, agg34p)
            wt_h = ffn_wload(w1h_d, w2h_d)
            module(3, xnTl, agg3, acc_h, rh_t, first=False)

            agg4 = aggregate(4, e_sb, agg34p)
            e_stack.close()

            ffn(acc_h, wt_h, b1h_t, b2h_t, outh_d)
            wt_e = ffn_wload(w1e_d, w2e_d)
            module(7, xnTl, agg4, acc_e, rh_t, first=False, warmln=True)
            agg34_stack.close()
            with tc.tile_pool(name="ffnE_sb", bufs=1) as fsbE:
                ecb, etail = mk_ffn_pipe(acc_e, wt_e[0], wt_e[1], b1eq,
                                         b2e_t, oute_d, fsbE)
                module(6, xeTl, agg2, acc_e, re_t, first=False, tile_cb=ecb)
                etail()
            agg12_stack.close()

        for _ in range(repeat):
            body()

    _split_big_waits(nc, mybir)
    return nc


def _get_program():
    if "nc" not in _PROGRAM_CACHE:
        _PROGRAM_CACHE["nc"] = _build_program()
    return _PROGRAM_CACHE["nc"]


def _prep_inputs(x_node, x_edge, adj, Wq, Wk, Wv,
                 proj_he_h, proj_eh_h, proj_he_e, proj_eh_e,
                 rms1_h, rms1_e, rms2_h,
                 w1_h, b1_h, w2_h, b2_h, w1_e, b1_e, w2_e, b2_e):
    """Per-core input dicts. Weight folding + row rotation happen here."""
    from ml_dtypes import bfloat16, float8_e4m3
    f = np.float32
    bf = bfloat16
    f16 = np.float16
    f8 = float8_e4m3
    # mats quantized to fp8e4m3, pre-scaled into a good fp8 range; the
    # inverse scale is folded into the wq/wk of the module consuming the
    # aggregate (mat0=adj -> q of modules 0/4; mat1..4 -> k of 2,6,3,7).
    MSC = [4096.0, 64.0, 64.0, 64.0, 64.0]
    qsc = [1.0 / MSC[0], 1, 1, 1, 1.0 / MSC[0], 1, 1, 1]
    ksc = [1, 1, 1.0 / MSC[1], 1.0 / MSC[3], 1, 1, 1.0 / MSC[2], 1.0 / MSC[4]]
    wsrc_q = [rms1_h, rms1_e, rms1_e, rms1_h, rms1_h, rms1_e, rms1_e, rms1_h]
    wsrc_k = [rms1_h, rms1_e, rms1_h, rms1_e, rms1_h, rms1_e, rms1_h, rms1_e]
    wqT = np.stack([(Wq[m].T * wsrc_q[m][:, None]) * (0.125 * qsc[m])
                    for m in range(H)])
    wkT = np.stack([(Wk[m].T * wsrc_k[m][:, None]) * ksc[m] for m in range(H)])
    wvT = np.stack([Wv[m].T * rms1_h[:, None] for m in range(H)])
    w1hT = np.ascontiguousarray((w1_h * rms2_h[None, :]).T.astype(bf))
    w1eT = np.ascontiguousarray((w1_e * rms2_h[None, :]).T.astype(bf))
    w2hT = np.ascontiguousarray(w2_h.T.astype(f16))
    w2eT = np.ascontiguousarray(w2_e.T.astype(f16))
    mats = [adj, proj_eh_h, proj_eh_e, proj_he_h, proj_he_e]

    shared = dict(wqT=np.ascontiguousarray(wqT.astype(bf)),
                  wkT=np.ascontiguousarray(wkT.astype(bf)),
                  wvT=np.ascontiguousarray(wvT.astype(bf)),
                  w1hT=w1hT, w2hT=w2hT, w1eT=w1eT, w2eT=w2eT,
                  b1h=b1_h.astype(f), b2h=b2_h.astype(f),
                  b1e=b1_e.astype(f), b2e=b2_e.astype(f))
    xn_bf = x_node.astype(bf)
    xe_bf = x_edge.astype(bf)
    in_maps = []
    for c in range(NCORES):
        r0 = c * RPC
        m = dict(shared)
        m["xn"] = np.ascontiguousarray(np.roll(xn_bf, -r0, axis=0))
        m["xe"] = np.ascontiguousarray(np.roll(xe_bf, -r0, axis=0))
        m["xnT"] = np.ascontiguousarray(xn_bf[r0:r0 + RPC].T)
        m["xeT"] = np.ascontiguousarray(xe_bf[r0:r0 + RPC].T)
        for i, mat in enumerate(mats):
            mt = np.ascontiguousarray(
                (mat[r0:r0 + RPC].T * MSC[i]).astype(f8))  # [N, RPC]
            m[f"mat{i}"] = np.ascontiguousarray(np.roll(mt, -r0, axis=0))
        in_maps.append(m)
    return in_maps


def kernel(**inputs):
    from concourse.bass_utils import run_bass_kernel_spmd
    nc = _get_program()
    in_maps = _prep_inputs(**{k: np.asarray(v) for k, v in inputs.items()})
    res = run_bass_kernel_spmd(nc, in_maps, list(range(NCORES))).results
    x_h = np.concatenate([res[c]["outh"] for c in range(NCORES)], axis=0)
    x_e = np.concatenate([res[c]["oute"] for c in range(NCORES)], axis=0)
    return (x_h, x_e)



# revision 49
# speedup vs baseline: 8.3509x; 1.0157x over previous
"""Trainium2 Bass kernel for the gnn_message_passing block (8 NeuronCores).

Strategy (per core c, owning 512 global rows r = c*512..(c+1)*512):
  - Host rotates x rows by -r0 (owned rows first), pre-transposes the owned
    x block (raw feature-major), and converts all large tensors to 16-bit
    (bf16 weights/mats/x, f16 w2) -- halves HBM traffic and removes all
    casting DMAs (pure HWDGE byte copies).
  - rmsnorm scale-invariance tricks: the per-node inverse-rms r[n] is
    needed exactly (a) multiplied into the node-major aggregation inputs
    h_sb/e_sb and (b) as the exp() scale of the per-node softmax for the
    raw (non-aggregated) q/k sources.  The v path needs NO normalization:
    v = h@Wv for every module, so x_att is uniformly scaled by r_h[n],
    which the (scale-invariant) second rmsnorm removes exactly.  The
    feature-major q/k/v stationary sources are therefore RAW transposed x
    from the host -- no on-chip transposes for them.
  - rsqrt = exp(-0.5*ln(.)): ln+exp live in ONE activation table set
    (natural_log_exp_and_others) together with square/copy, so the whole
    kernel needs only ~4 ACT table loads (vs ~100 when mixing sqrt):
    nl_exp era -> gelu(FFN h) -> nl_exp era -> gelu(FFN e).
  - The five N x N aggregations (adj@h shared by modules 0/4, four proj@k)
    run in fp8e4m3 DoubleRow mode (0.5 PE cycles/row): mats are pre-scaled
    host-side into fp8 range (adj*4096, proj*64; inverse folded into the
    consuming module's wq/wk) and the normalized h/e aggregation inputs are
    quantized to fp8 mega-tiles.  DoubleRow dst is ISA-limited to psum
    partitions 0-63, so each 64-feature odd half accumulates in a scratch
    bank and is rebased to partitions 64-127 via a cheap identity matmul.
    Costs ~6e-3 extra rel-err (1e-2 total vs the 2e-2 gate), halves
    aggregation PE time AND mat HBM traffic.  512x512 projections bf16.
  - Per-node 8-head SDPA on DVE in f16 2x mode: broadcast-AP multiplies +
    halving-tree reduces (measured: TT=2x, TensorReduce/Pool/TTR=1x, so
    trees beat single reduces).  Softmax exp on ACT with per-node scale.
    GpSimd offload of the av-multiply is wired behind POOL_AV_MOD but OFF:
    the cost model charges Pool TT at 0.83ns/elem while real Q7 hardware
    runs 2-input ops ~2.2ns/elem, so the offload only looks good in sim.
  - Module processing interleaves with aggregations so PE and DVE overlap:
    loads | mod 1,5 | agg0 | mod 0,4 | agg1,2 | mod 2 | agg3 | mod 3 |
    agg4 | FFN-h | mod 7 | mod 6 + pipelined FFN-e
  - FFN-e is fully pipelined per node-block via a tile callback inside the
    last module: each 128-node block runs rmsnorm2 -> transpose -> w1 ->
    gelu -> w2 -> DMA while the remaining SDPA tiles are still on DVE.
    The per-ffb gelu bias is applied by one 4-deep indicator matmul per
    4-ffb group so the gelu runs 512 wide (4 ACT ops/block instead of 16).
    FFN-h stays serial (pipelining it into module 7 would thrash the
    gelu/exp ACT table sets) but its rmsnorm2 squares and out-copies run
    on DVE, which is idle there, so module 7's softmax exps aren't queued
    behind them on ACT.
  - DMA queues: x/mats/outputs/weights on SP (HWDGE), modules 1/5 weights
    on the ACT queue (ahead of the SP load burst at body start); only the
    casting bias loads remain on POOL SWDGE.  POOL otherwise does the
    cheap per-node normalization multiplies (1-input ops are line-rate on
    real Q7; 2-input ops are not).
  - Schedule details: the first xe node-major load is issued on SP BEFORE
    the eight transposed-x loads so the DVE stats fill starts ~3us earlier;
    a 28-transpose PE warm-up chain covers the p-state ramp (matmuls run at
    1.2GHz for the first 3us after idle, 2.4GHz after) so the first
    projections run at full clock; the FFN-e callback's rmsnorm squares run
    on ACT (which has slack inside module 6) rather than DVE.
  - CoreSim: 338.1us (DVE-bound: DVE busy ~305us, PE ~224, ACT ~210) vs
    350.8us for the bf16 predecessor; measured HW (repeat-differential)
    331us for the predecessor.  HW-verified rel-err 9.44e-3.
"""
import numpy as np

N = 4096
E = 512
H = 8
D = 64
FF = 2048
P = 128
NCORES = 8
RPC = N // NCORES  # 512 rows per core
NT = N // P        # 32 tiles over all nodes
LT = RPC // P      # 4 local tiles
EPS = float(np.finfo(np.float32).eps)
# run the av broadcast-multiply on GpSimd for tiles with
# t % POOL_AV_MOD == POOL_AV_PHASE (engine-balance knob; real-HW GpSimd
# tensor_tensor is ~4x slower than the cost model says, so keep this off)
POOL_AV_MOD = 1000
POOL_AV_PHASE = 1

_PROGRAM_CACHE = {}


def _split_big_waits(nc, mybir):
    """walrus in this toolchain rejects multi-wait instructions; cap at 1
    (2 for EventSemaphore), chaining the excess as EventSemaphores."""
    for f in nc.m.functions:
        for bb in f.blocks:
            insts = list(bb.instructions)
            out = []
            changed = False
            for inst in insts:
                si = inst.sync_info
                waits = list(si.on_wait) if si and si.on_wait else []
                cap = 2 if isinstance(inst, mybir.InstEventSemaphore) else 1
                if len(waits) > cap:
                    extra, keep = waits[:-cap], waits[-cap:]
                    for ci in range(0, len(extra), 2):
                        ev = mybir.InstEventSemaphore(name=f"{inst.name}-evw{ci}")
                        ev.engine = inst.engine
                        ev.sync_info = mybir.SyncInfo(on_wait=extra[ci:ci + 2],
                                                      on_update=[])
                        out.append(ev)
                    si.on_wait = keep
                    changed = True
                out.append(inst)
            if changed:
                bb.instructions[:] = out


def _build_program(repeat=1):
    import concourse.bass as bass
    import concourse.tile as tile
    from concourse import mybir
    from concourse.masks import make_identity
    from contextlib import ExitStack

    f32 = mybir.dt.float32
    f32r = mybir.dt.float32r
    f16 = mybir.dt.float16
    bf16 = mybir.dt.bfloat16
    f8 = mybir.dt.float8e4
    AF = mybir.ActivationFunctionType
    OP = mybir.AluOpType
    AX = mybir.AxisListType
    DR = mybir.MatmulPerfMode.DoubleRow

    def bc(t, dims, off=0):
        return bass.AP(tensor=t.tensor, offset=t.offset + off,
                       ap=[list(t.ap[0])] + [[s, c] for (s, c) in dims])

    nc = bass.Bass()

    xn_d = nc.declare_dram_parameter("xn", [N, E], bf16, isOutput=False)
    xe_d = nc.declare_dram_parameter("xe", [N, E], bf16, isOutput=False)
    xnT_d = nc.declare_dram_parameter("xnT", [E, RPC], bf16, isOutput=False)
    xeT_d = nc.declare_dram_parameter("xeT", [E, RPC], bf16, isOutput=False)
    mat_d = [nc.declare_dram_parameter(f"mat{i}", [N, RPC], f8, isOutput=False)
             for i in range(5)]
    wq_d = nc.declare_dram_parameter("wqT", [H, E, E], bf16, isOutput=False)
    wk_d = nc.declare_dram_parameter("wkT", [H, E, E], bf16, isOutput=False)
    wv_d = nc.declare_dram_parameter("wvT", [H, E, E], bf16, isOutput=False)
    w1h_d = nc.declare_dram_parameter("w1hT", [E, FF], bf16, isOutput=False)
    w2h_d = nc.declare_dram_parameter("w2hT", [FF, E], f16, isOutput=False)
    w1e_d = nc.declare_dram_parameter("w1eT", [E, FF], bf16, isOutput=False)
    w2e_d = nc.declare_dram_parameter("w2eT", [FF, E], f16, isOutput=False)
    b1h_d = nc.declare_dram_parameter("b1h", [FF], f32, isOutput=False)
    b2h_d = nc.declare_dram_parameter("b2h", [E], f32, isOutput=False)
    b1e_d = nc.declare_dram_parameter("b1e", [FF], f32, isOutput=False)
    b2e_d = nc.declare_dram_parameter("b2e", [E], f32, isOutput=False)
    outh_d = nc.declare_dram_parameter("outh", [RPC, E], f32, isOutput=True)
    oute_d = nc.declare_dram_parameter("oute", [RPC, E], f32, isOutput=True)

    with tile.TileContext(nc, pool_alloc_mode="queue") as tc, ExitStack() as ctx:
        consts = ctx.enter_context(tc.tile_pool(name="consts", bufs=1))
        eps_t = consts.tile([P, 1], f32)
        nc.vector.memset(eps_t, EPS)
        lnwarm = consts.tile([P, 1], f32)
        # first ACT op: pins the natural_log_exp_and_others table set
        nc.scalar.activation(out=lnwarm[:], in_=eps_t[:], func=AF.Ln)
        ident = consts.tile([P, P], f16)
        make_identity(nc, ident)
        ones1f = consts.tile([1, P], f32)
        nc.gpsimd.memset(ones1f, 1.0)
        ones1 = consts.tile([1, P], f32r)
        nc.scalar.copy(ones1[:], ones1f[:])
        b1h_t = consts.tile([P, FF // P], f32)
        b1e_t = consts.tile([P, FF // P], f32)
        b2h_t = consts.tile([1, E], f32r)
        b2e_t = consts.tile([1, E], f32r)
        # indicator eye(4) (x) ones(128) + b1 rows: lets the pipelined FFN
        # apply the per-ffb gelu bias via one 4-deep matmul so 4 ffb blocks
        # share a single wide gelu
        ind4f = consts.tile([4, E], f32)
        nc.gpsimd.memset(ind4f, 0.0)
        for q in range(4):
            # partition-offset writes need a DMA (engines can't start at
            # partition q); tiny one-time init copies
            nc.sync.dma_start(out=ind4f[q:q + 1, q * P:(q + 1) * P],
                              in_=ones1f[0:1, :])
        ind4 = consts.tile([4, E], f32r)
        nc.scalar.copy(ind4[:], ind4f[:])

        b1eq = consts.tile([4, 4 * P], f32r)

        # whole-program pools
        statp = ctx.enter_context(tc.tile_pool(name="stat", bufs=1))
        rawp = ctx.enter_context(tc.tile_pool(name="raw", bufs=1))
        sqscp = ctx.enter_context(tc.tile_pool(name="sqsc", bufs=1))
        xTlp = ctx.enter_context(tc.tile_pool(name="xTl", bufs=1))
        wpool = ctx.enter_context(tc.tile_pool(name="wts", bufs=1))
        qkvp = ctx.enter_context(tc.tile_pool(name="qkv", bufs=1))
        tmpp = ctx.enter_context(tc.tile_pool(name="sdtmp", bufs=1))
        smp = ctx.enter_context(tc.tile_pool(name="sdsm", bufs=2))
        accp = ctx.enter_context(tc.tile_pool(name="acc", bufs=1))
        matgp = ctx.enter_context(tc.tile_pool(name="matg", bufs=2))
        fwts = ctx.enter_context(tc.tile_pool(name="fwts", bufs=1))
        psp = ctx.enter_context(tc.tile_pool(name="ps", bufs=1, space="PSUM"))

        # PE p-state warm-up: dummy transpose chain covering the first ~3us
        # so the p-state ramp completes before the first real projections
        warmps = psp.tile([P, P], f16, tag="projps", bufs=4, name="warm")
        for _ in range(28):
            nc.tensor.transpose(warmps[:], ident[:], ident[:])

        def body():
            # per-iteration stat tiles
            ssq_h = statp.tile([P, NT], f32, tag="ssqh", name="ssqh")
            ssq_e = statp.tile([P, NT], f32, tag="ssqe", name="ssqe")
            lnsc = statp.tile([P, NT], f32, tag="lnsc", name="lnsc")
            rh_t = statp.tile([P, NT], f32, tag="rh", name="rh")
            re_t = statp.tile([P, NT], f32, tag="re", name="re")
            re2_t = statp.tile([P, LT], f32, tag="re2", name="re2")

            xnTl = [xTlp.tile([P, RPC], bf16, tag=f"xnT{fc}", name=f"xnT{fc}")
                    for fc in range(4)]
            xeTl = [xTlp.tile([P, RPC], bf16, tag=f"xeT{fc}", name=f"xeT{fc}")
                    for fc in range(4)]
            def load_xT():
                for fc in range(4):
                    nc.sync.dma_start(out=xeTl[fc][:],
                                      in_=xeT_d[fc * P:(fc + 1) * P, :])
                for fc in range(4):
                    nc.sync.dma_start(out=xnTl[fc][:],
                                      in_=xnT_d[fc * P:(fc + 1) * P, :])

            acc_h = [accp.tile([P, E], f16, tag=f"ah{t}", name=f"ah{t}")
                     for t in range(LT)]
            acc_e = [accp.tile([P, E], f16, tag=f"ae{t}", name=f"ae{t}")
                     for t in range(LT)]

            def load_group(x_d, sb8, ssq, rdst, g0, dve_stats=False):
                """Load 4 node-major tiles, square-accum stats, rsqrt the 4
                columns, normalize into the fp8 aggregation mega-tile sb8.
                dve_stats puts the squares on DVE (for the first group, while
                DVE is idle and ACT latency gates the first module)."""
                xg = rawp.tile([P, 4 * E], bf16, tag="xraw", bufs=2,
                               name="xraw")
                nc.sync.dma_start(
                    out=xg.rearrange("p (t e) -> p t e", e=E),
                    in_=x_d[g0 * P:(g0 + 4) * P, :].rearrange(
                        "(t p) e -> p t e", p=P))
                for t in range(4):
                    ti = g0 + t
                    scr = sqscp.tile([P, E], f16, tag="sq", bufs=1, name="sq")
                    if dve_stats:
                        nc.vector.scalar_tensor_tensor(
                            out=scr[:], in0=xg[:, t * E:(t + 1) * E],
                            scalar=0.0, in1=xg[:, t * E:(t + 1) * E],
                            op0=OP.add, op1=OP.mult,
                            accum_out=ssq[:, ti:ti + 1])
                    else:
                        nc.scalar.activation(out=scr[:],
                                             in_=xg[:, t * E:(t + 1) * E],
                                             func=AF.Square,
                                             accum_out=ssq[:, ti:ti + 1])
                nc.scalar.activation(out=lnsc[:, g0:g0 + 4],
                                     in_=ssq[:, g0:g0 + 4], func=AF.Ln,
                                     scale=1.0 / E, bias=eps_t[:])
                nc.scalar.activation(out=rdst[:, g0:g0 + 4],
                                     in_=lnsc[:, g0:g0 + 4], func=AF.Exp,
                                     scale=-0.5)
                for t in range(4):
                    ti = g0 + t
                    nc.gpsimd.tensor_scalar_mul(sb8[:, ti * E:(ti + 1) * E],
                                                xg[:, t * E:(t + 1) * E],
                                                rdst[:, ti:ti + 1])

            def aggregate(mi, src8, aggpool):
                """4 feature-major bf16 [128, 512] blocks of mat_mi @ x.

                fp8e4 DoubleRow matmuls: both operands fp8 (mats pre-scaled
                host-side; inverse scale folded into wq/wk), 2 node-tiles
                (256 contraction rows) per pass at 0.5 cycles/row.  Output
                partition limit is 64, so each psum bank holds a pair of
                64-feature blocks at partition offsets 0/64 and the bf16
                copy-out still reads one [128, 512] bank."""
                pss = [psp.tile([P, E], f32, tag=f"agps{b}", name=f"agps{b}")
                       for b in range(4)]
                scr = [psp.tile([64, E], f32, tag="projps", bufs=4,
                                name=f"agsc{b}") for b in range(4)]
                for g in range(8):
                    mt = matgp.tile([P, 4 * RPC], f8, tag="matg", name="matg")
                    nc.sync.dma_start(
                        out=mt.rearrange("p (t e) -> p t e", e=RPC),
                        in_=mat_d[mi][g * 4 * P:(g + 1) * 4 * P, :].rearrange(
                            "(t p) e -> p t e", p=P))
                    for tp in range(2):
                        pair = g * 2 + tp
                        for b in range(4):
                            # DoubleRow dst is ISA-limited to partitions
                            # 0-63: even feature-half accumulates in
                            # pss[b][0:64], odd half in a scratch bank and
                            # is rebased to [64:128] afterwards.
                            for half in range(2):
                                dst = pss[b][0:64, :] if half == 0 else scr[b][:]
                                nc.tensor.matmul(
                                    dst,
                                    lhsT=bc(src8, [(E, 2), (1, 64)],
                                            off=pair * 2 * E + (2 * b + half) * 64),
                                    rhs=bc(mt, [(RPC, 2), (1, RPC)],
                                           off=tp * 2 * RPC),
                                    start=(pair == 0), stop=(pair == 15),
                                    perf_mode=DR,
                                    tile_position=(0, 0),
                                    skip_group_check=True)
                outt = []
                for b in range(4):
                    t64 = sqscp.tile([64, E], f16, tag="agt64", bufs=2,
                                     name="agt64")
                    nc.scalar.copy(t64[:], scr[b][:])
                    nc.tensor.matmul(pss[b][64:128, :],
                                     lhsT=ident[0:64, 0:64], rhs=t64[:],
                                     start=True, stop=True,
                                     tile_position=(0, 64),
                                     skip_group_check=True)
                    at = aggpool.tile([P, E], bf16, tag=f"ag{mi}_{b}",
                                      name=f"ag{mi}_{b}")
                    nc.scalar.copy(at[:], pss[b][:])
                    outt.append(at)
                return outt

            def module(m, qsrcT, ksrcT, branch_att, rsc, first, warmln=False,
                       tile_cb=None, wdma=None):
                w_ts = {}
                for (dram, nm) in ((wq_d, "wq"), (wk_d, "wk"), (wv_d, "wv")):
                    wt = wpool.tile([P, 4 * E], bf16, tag=nm,
                                    bufs=(1 if nm == "wv" else 2),
                                    name=f"w_{nm}")
                    (wdma or nc.sync).dma_start(
                        out=wt.rearrange("p (fc e) -> p fc e", e=E),
                        in_=dram[m].rearrange("(fc p) e -> p fc e", p=P))
                    w_ts[nm] = wt
                if warmln:
                    # re-pin the ln+exp table set after a gelu era
                    nc.scalar.activation(out=lnwarm[:], in_=eps_t[:],
                                         func=AF.Ln)

                # pair-wide SDPA (two tiles per DVE instruction) was tried
                # and REGRESSED (CoreSim 353.5 vs 343.4): the per-op saving is
                # only ~60ns while the coarser q/k/v pair dependencies open a
                # ~10us module-boundary stall; keep per-tile issue
                wide = False

                # per-tile interleave (q_b, k_b, v_b) so tile 0's SDPA can
                # start after 12 matmuls instead of 36
                q_sb, k_sb, v_sb = [], [], []
                for b in range(LT):
                    for (srcT, wnm, lst) in ((qsrcT, "wq", q_sb),
                                             (ksrcT, "wk", k_sb),
                                             (xnTl, "wv", v_sb)):
                        wt = w_ts[wnm]
                        ps = psp.tile([P, E], f32, tag="projps", bufs=4,
                                      name="projps")
                        for fc in range(4):
                            nc.tensor.matmul(
                                ps[:],
                                lhsT=srcT[fc][:, b * P:(b + 1) * P],
                                rhs=wt[:, fc * E:(fc + 1) * E],
                                start=(fc == 0), stop=(fc == 3))
                        dt = qkvp.tile([P, E], f16, tag=f"{wnm}_{b}",
                                       bufs=(2 if wnm == "wq" else 1),
                                       name=f"qkv{b}")
                        lst.append(dt)
                        if wnm == "wv":
                            nc.scalar.copy(bc(dt, [(1, 8), (8, 64)]), ps[:])
                        else:
                            nc.scalar.copy(dt[:], ps[:])

                if wide:
                    W = H * H * D  # 4096: one tile's score/av scratch
                    pair_st = {}

                    def phase_a(pb):
                        q2, k2, v2 = q_sb[pb], k_sb[pb], v_sb[pb]
                        tmp = tmpp.tile([P, 2 * W], f16, tag="sdpa", bufs=2,
                                        name="sdpa")
                        nc.vector.tensor_tensor(
                            out=bc(tmp, [(W, 2), (512, 8), (64, 8), (1, 64)]),
                            in0=bc(q2, [(E, 2), (64, 8), (0, 8), (1, 64)]),
                            in1=bc(k2, [(E, 2), (0, 8), (64, 8), (1, 64)]),
                            op=OP.mult)
                        for dd in (32, 16, 8, 4, 2):
                            nc.vector.tensor_tensor(
                                out=bc(tmp, [(W, 2), (64, 64), (1, dd)]),
                                in0=bc(tmp, [(W, 2), (64, 64), (1, dd)]),
                                in1=bc(tmp, [(W, 2), (64, 64), (1, dd)],
                                       off=dd),
                                op=OP.add)
                        s2 = smp.tile([P, 2 * H * H], f16, tag="sw", name="sw")
                        nc.vector.tensor_tensor(
                            out=bc(s2, [(64, 2), (1, 64)]),
                            in0=bc(tmp, [(W, 2), (64, 64)]),
                            in1=bc(tmp, [(W, 2), (64, 64)], off=1),
                            op=OP.add)
                        ex2 = smp.tile([P, 2 * H * H], f16, tag="exw",
                                       name="exw")
                        for t2 in range(2):
                            t = 2 * pb + t2
                            nc.scalar.activation(
                                out=ex2[:, t2 * 64:(t2 + 1) * 64],
                                in_=s2[:, t2 * 64:(t2 + 1) * 64],
                                func=AF.Exp, scale=rsc[:, t:t + 1])
                        pair_st[pb] = (ex2, v2)

                    def phase_b(pb):
                        ex2, v2 = pair_st[pb]
                        den2 = smp.tile([P, 2 * H], f32, tag="denw",
                                        name="denw")
                        nc.vector.tensor_reduce(
                            out=den2[:],
                            in_=ex2.rearrange("p (x g) -> p x g", g=H),
                            axis=AX.X, op=OP.add)
                        rden2 = smp.tile([P, 2 * H], f32, tag="rdenw",
                                         name="rdenw")
                        nc.vector.reciprocal(out=rden2[:], in_=den2[:])
                        a2 = smp.tile([P, 2 * H * H], f16, tag="aw", name="aw")
                        nc.vector.tensor_tensor(
                            out=bc(a2, [(64, 2), (8, 8), (1, 8)]),
                            in0=bc(ex2, [(64, 2), (8, 8), (1, 8)]),
                            in1=bc(rden2, [(8, 2), (1, 8), (0, 8)]),
                            op=OP.mult)
                        tmp2 = tmpp.tile([P, 2 * W], f16, tag="sdpa2", bufs=1,
                                         name="sdpa2")
                        nc.vector.tensor_tensor(
                            out=bc(tmp2, [(W, 2), (512, 8), (8, 64), (1, 8)]),
                            in0=bc(a2, [(64, 2), (8, 8), (0, 64), (1, 8)]),
                            in1=bc(v2, [(E, 2), (0, 8), (8, 64), (1, 8)]),
                            op=OP.mult)
                        for gg in (4, 2):
                            nc.vector.tensor_tensor(
                                out=bc(tmp2, [(W, 2), (8, 512), (1, gg)]),
                                in0=bc(tmp2, [(W, 2), (8, 512), (1, gg)]),
                                in1=bc(tmp2, [(W, 2), (8, 512), (1, gg)],
                                       off=gg),
                                op=OP.add)
                        for t2 in range(2):
                            t = 2 * pb + t2
                            ofs = t2 * W
                            if first:
                                nc.vector.tensor_tensor(
                                    out=branch_att[t][:],
                                    in0=bc(tmp2, [(8, 512)], off=ofs),
                                    in1=bc(tmp2, [(8, 512)], off=ofs + 1),
                                    op=OP.add)
                            else:
                                rt = smp.tile([P, E], f16, tag="avred",
                                              bufs=1, name="avred")
                                nc.vector.tensor_tensor(
                                    out=rt[:],
                                    in0=bc(tmp2, [(8, 512)], off=ofs),
                                    in1=bc(tmp2, [(8, 512)], off=ofs + 1),
                                    op=OP.add)
                                nc.vector.tensor_tensor(out=branch_att[t][:],
                                                        in0=branch_att[t][:],
                                                        in1=rt[:], op=OP.add)

                    # software-pipeline: both pairs' score+exp first, so the
                    # second pair's DVE work hides the first pair's ACT
                    # round-trip before the softmax/AV half needs ex2
                    phase_a(0)
                    phase_a(1)
                    phase_b(0)
                    phase_b(1)
                    return

                for t in range(LT):
                    q_t, k_t, v_t = q_sb[t], k_sb[t], v_sb[t]
                    tmp = tmpp.tile([P, H * H * D], f16, tag="sdpan",
                                    bufs=1, name="sdpan")
                    nc.vector.tensor_tensor(
                        out=bc(tmp, [(512, 8), (64, 8), (1, 64)]),
                        in0=bc(q_t, [(64, 8), (0, 8), (1, 64)]),
                        in1=bc(k_t, [(0, 8), (64, 8), (1, 64)]),
                        op=OP.mult)
                    for dd in (32, 16, 8, 4, 2):
                        nc.vector.tensor_tensor(
                            out=bc(tmp, [(64, 64), (1, dd)]),
                            in0=bc(tmp, [(64, 64), (1, dd)]),
                            in1=bc(tmp, [(64, 64), (1, dd)], off=dd),
                            op=OP.add)
                    s_t = smp.tile([P, H * H], f16, tag="s", name="s")
                    nc.vector.tensor_tensor(
                        out=s_t[:],
                        in0=bc(tmp, [(64, 64)]),
                        in1=bc(tmp, [(64, 64)], off=1),
                        op=OP.add)
                    ex_t = smp.tile([P, H * H], f16, tag="ex", name="ex")
                    nc.scalar.activation(out=ex_t[:], in_=s_t[:], func=AF.Exp,
                                         scale=rsc[:, t:t + 1])
                    den = smp.tile([P, H], f32, tag="den", name="den")
                    nc.vector.tensor_reduce(
                        out=den[:], in_=ex_t.rearrange("p (h g) -> p h g", g=H),
                        axis=AX.X, op=OP.add)
                    rden = smp.tile([P, H], f32, tag="rden", name="rden")
                    nc.vector.reciprocal(out=rden[:], in_=den[:])
                    a_t = smp.tile([P, H * H], f16, tag="a", name="a")
                    nc.vector.tensor_tensor(
                        out=bc(a_t, [(8, 8), (1, 8)]),
                        in0=bc(ex_t, [(8, 8), (1, 8)]),
                        in1=bc(rden, [(1, 8), (0, 8)]),
                        op=OP.mult)
                    tmp2 = tmpp.tile([P, H * H * D], f16, tag="sdpa2n",
                                     bufs=2, name="sdpa2n")
                    nc.vector.tensor_tensor(
                        out=bc(tmp2, [(512, 8), (8, 64), (1, 8)]),
                        in0=bc(a_t, [(8, 8), (0, 64), (1, 8)]),
                        in1=bc(v_t, [(0, 8), (8, 64), (1, 8)]),
                        op=OP.mult)
                    for gg in (4, 2):
                        nc.vector.tensor_tensor(
                            out=bc(tmp2, [(8, 512), (1, gg)]),
                            in0=bc(tmp2, [(8, 512), (1, gg)]),
                            in1=bc(tmp2, [(8, 512), (1, gg)], off=gg),
                            op=OP.add)
                    if first:
                        nc.vector.tensor_tensor(
                            out=branch_att[t][:],
                            in0=bc(tmp2, [(8, 512)]),
                            in1=bc(tmp2, [(8, 512)], off=1),
                            op=OP.add)
                    else:
                        rt = smp.tile([P, E], f16, tag="avred", bufs=1,
                                      name="avred")
                        nc.vector.tensor_tensor(
                            out=rt[:],
                            in0=bc(tmp2, [(8, 512)]),
                            in1=bc(tmp2, [(8, 512)], off=1),
                            op=OP.add)
                        nc.vector.tensor_tensor(out=branch_att[t][:],
                                                in0=branch_att[t][:],
                                                in1=rt[:], op=OP.add)
                    if tile_cb is not None:
                        tile_cb(t)

            def ffn_wload(w1_dram, w2_dram):
                """Weight tiles + DMAs; emit early to hide the transfers."""
                w1_ts = []
                for half in range(2):
                    HW1 = FF // 2
                    w1_t = fwts.tile([P, 4 * HW1], bf16, tag=f"w1_{half}",
                                     name=f"w1_{half}")
                    nc.sync.dma_start(
                        out=w1_t.rearrange("p (fc e) -> p fc e", e=HW1),
                        in_=w1_dram[:, half * HW1:(half + 1) * HW1].rearrange(
                            "(fc p) e -> p fc e", p=P))
                    w1_ts.append(w1_t)
                return (w1_ts, w2_dram)

            def ffn(branch_att, wtiles, b1_t, b2_t, out_dram):
                w1_ts, w2_dram = wtiles
                with tc.tile_pool(name="ffn_sb", bufs=1) as fsb:
                    w2_t = fsb.tile([P, 16 * E], f16, tag="w2", name="w2")
                    nc.sync.dma_start(
                        out=w2_t.rearrange("p (fc e) -> p fc e", e=E),
                        in_=w2_dram[:, :].rearrange("(fc p) e -> p fc e", p=P))
                    # rmsnorm2: stats + rsqrt + normalize (f16); squares on
                    # DVE (idle here) so module 7's softmax exps aren't stuck
                    # behind them in the ACT queue
                    ssq2 = statp.tile([P, LT], f32, tag="fss", name="fss")
                    for t in range(LT):
                        scr = sqscp.tile([P, E], f16, tag="sq", bufs=1,
                                         name="fsq")
                        nc.vector.scalar_tensor_tensor(
                            out=scr[:], in0=branch_att[t][:], scalar=0.0,
                            in1=branch_att[t][:], op0=OP.add, op1=OP.mult,
                            accum_out=ssq2[:, t:t + 1])
                    ln2 = statp.tile([P, LT], f32, tag="fln", name="fln")
                    nc.scalar.activation(out=ln2[:], in_=ssq2[:], func=AF.Ln,
                                         scale=1.0 / E, bias=eps_t[:])
                    rs2 = statp.tile([P, LT], f32, tag="frs", name="frs")
                    nc.scalar.activation(out=rs2[:], in_=ln2[:], func=AF.Exp,
                                         scale=-0.5)
                    xn_tiles = []
                    for t in range(LT):
                        xt = sqscp.tile([P, E], f16, tag=f"ffx{t}",
                                        bufs=1, name=f"fx{t}")
                        nc.gpsimd.tensor_scalar_mul(xt[:], branch_att[t][:],
                                                    rs2[:, t:t + 1])
                        xn_tiles.append(xt)
                    xnT = []
                    for fc in range(4):
                        ps = psp.tile([P, RPC], f16, tag="agps0", name="ftr")
                        for t in range(4):
                            nc.tensor.transpose(ps[:, t * P:(t + 1) * P],
                                                xn_tiles[t][:, fc * P:(fc + 1) * P],
                                                ident[:])
                        xt = fsb.tile([P, RPC], bf16, tag=f"fxT{fc}",
                                      name=f"fxT{fc}")
                        nc.scalar.copy(xt[:], ps[:])
                        xnT.append(xt)
                    g1 = []
                    HW1 = FF // 2
                    for half in range(2):
                        w1_t = w1_ts[half]
                        for fb in range(HW1 // P):
                            ffb = half * (HW1 // P) + fb
                            ps = psp.tile([P, RPC], f32, tag=f"agps{1 + ffb % 2}",
                                          name="fps1")
                            for fc in range(4):
                                nc.tensor.matmul(
                                    ps[:],
                                    lhsT=w1_t[:, fc * HW1 + fb * P:
                                              fc * HW1 + (fb + 1) * P],
                                    rhs=xnT[fc][:],
                                    start=(fc == 0), stop=(fc == 3))
                            gt = fsb.tile([P, RPC], f16, tag=f"g1_{ffb}",
                                          name=f"g1_{ffb}")
                            nc.scalar.activation(out=gt[:], in_=ps[:],
                                                 func=AF.Gelu,
                                                 bias=b1_t[:, ffb:ffb + 1],
                                                 scale=1.0)
                            g1.append(gt)
                    for b in range(LT):
                        ps = psp.tile([P, E], f32, tag="agps3", name="fps2")
                        for ffc in range(FF // P):
                            nc.tensor.matmul(
                                ps[:],
                                lhsT=g1[ffc][:, b * P:(b + 1) * P],
                                rhs=w2_t[:, ffc * E:(ffc + 1) * E],
                                start=(ffc == 0), stop=False)
                        nc.tensor.matmul(ps[:], lhsT=ones1[:], rhs=b2_t[:],
                                         start=False, stop=True)
                        ob = fsb.tile([P, E], f32, tag="fo", bufs=2, name="fo")
                        nc.vector.tensor_scalar_mul(ob[:], ps[:], 1.0)
                        nc.sync.dma_start(
                            out=out_dram[b * P:(b + 1) * P, :], in_=ob[:])

            def mk_ffn_pipe(branch_att, w1_ts, w2_dram, b1q, b2_t, out_dram,
                            fsb):
                """Per-tile pipelined FFN: the returned callback is invoked
                inside the final module after each tile's SDPA, so nearly the
                whole FFN runs under the module's remaining SDPA work.  Node
                block t is pushed through rmsnorm2 -> transpose -> w1 -> gelu
                -> w2 as soon as its attention accumulator is final; only
                block 3's chain is exposed at the end."""
                HW1 = FF // 2
                st = {
                    "ssq2": fsb.tile([P, LT], f32, tag="fss", name="fss"),
                    "ln2": fsb.tile([P, LT], f32, tag="fln", name="fln"),
                    "rs2": fsb.tile([P, LT], f32, tag="frs", name="frs"),
                    "psT": [psp.tile([P, RPC], f16, tag=f"agps{fc}",
                                     name=f"eftr{fc}") for fc in range(4)],
                    "xnT": [fsb.tile([P, RPC], bf16, tag=f"fxT{fc}",
                                     name=f"fxT{fc}") for fc in range(4)],
                    "g1g": [fsb.tile([P, 4 * RPC], f16, tag=f"g1g_{g}",
                                     name=f"g1g_{g}") for g in range(4)],
                    "w2": fsb.tile([P, 16 * E], f16, tag="w2", name="w2"),
                    "ps2": [None] * LT,
                }

                def out_block(b):
                    ob = fsb.tile([P, E], f32, tag="fo", bufs=2, name="fo")
                    nc.vector.tensor_scalar_mul(ob[:], st["ps2"][b][:], 1.0)
                    nc.sync.dma_start(out=out_dram[b * P:(b + 1) * P, :],
                                      in_=ob[:])

                def block_chain(b):
                    """w1 -> gelu -> w2 for node block b (emitted one SDPA
                    tile later so its ACT ops sit behind the next exp in the
                    queue and can't stall the softmax chain)."""
                    lo, hi = b * P, (b + 1) * P
                    if b > 0:
                        out_block(b - 1)
                    for g in range(4):
                        ps4 = psp.tile([P, 4 * P], f32, tag="projps", bufs=4,
                                       name="efps1")
                        # the bias matmul OPENS the bank (start=True zeroes
                        # the whole 2KB zero-region, so per-quarter starts
                        # would wipe sibling quarters); quarters accumulate
                        nc.tensor.matmul(ps4[:],
                                         lhsT=b1q[0:4, g * P:(g + 1) * P],
                                         rhs=ind4[:], start=True, stop=False,
                                         skip_group_check=True)
                        for qq in range(4):
                            ffb = g * 4 + qq
                            w1_t = w1_ts[ffb // 8]
                            fb = ffb % 8
                            for fc in range(4):
                                nc.tensor.matmul(
                                    ps4[:, qq * P:(qq + 1) * P],
                                    lhsT=w1_t[:, fc * HW1 + fb * P:
                                              fc * HW1 + (fb + 1) * P],
                                    rhs=st["xnT"][fc][:, lo:hi],
                                    start=False,
                                    stop=(qq == 3 and fc == 3),
                                    skip_group_check=True)
                        nc.scalar.activation(
                            out=bc(st["g1g"][g], [(RPC, 4), (1, P)], off=lo),
                            in_=ps4[:], func=AF.Gelu, scale=1.0)
                    ps2 = psp.tile([P, E], f32, tag="projps", bufs=4,
                                    name="efps2")
                    for ffc in range(FF // P):
                        nc.tensor.matmul(
                            ps2[:],
                            lhsT=st["g1g"][ffc // 4][:, (ffc % 4) * RPC + lo:
                                                     (ffc % 4) * RPC + hi],
                            rhs=st["w2"][:, ffc * E:(ffc + 1) * E],
                            start=(ffc == 0), stop=False)
                    nc.tensor.matmul(ps2[:], lhsT=ones1[:], rhs=b2_t[:],
                                     start=False, stop=True)
                    st["ps2"][b] = ps2

                def cb(t):
                    if t == 0:
                        nc.sync.dma_start(
                            out=st["w2"].rearrange("p (fc e) -> p fc e", e=E),
                            in_=w2_dram[:, :].rearrange("(fc p) e -> p fc e",
                                                        p=P))
                    lo, hi = t * P, (t + 1) * P
                    scr = sqscp.tile([P, E], f16, tag="sq", bufs=1, name="esq")
                    nc.scalar.activation(out=scr[:], in_=branch_att[t][:],
                                         func=AF.Square,
                                         accum_out=st["ssq2"][:, t:t + 1])
                    nc.scalar.activation(out=st["ln2"][:, t:t + 1],
                                         in_=st["ssq2"][:, t:t + 1],
                                         func=AF.Ln, scale=1.0 / E,
                                         bias=eps_t[:])
                    nc.scalar.activation(out=st["rs2"][:, t:t + 1],
                                         in_=st["ln2"][:, t:t + 1],
                                         func=AF.Exp, scale=-0.5)
                    xt = fsb.tile([P, E], f16, tag="fx", bufs=2, name="fx")
                    nc.gpsimd.tensor_scalar_mul(xt[:], branch_att[t][:],
                                                st["rs2"][:, t:t + 1])
                    for fc in range(4):
                        nc.tensor.transpose(st["psT"][fc][:, lo:hi],
                                            xt[:, fc * P:(fc + 1) * P],
                                            ident[:])
                        nc.scalar.copy(st["xnT"][fc][:, lo:hi],
                                       st["psT"][fc][:, lo:hi])
                    if t > 0:
                        block_chain(t - 1)

                def tail():
                    block_chain(LT - 1)
                    out_block(LT - 1)

                return cb, tail

            # ======== emission order (the schedule) ========
            agg12_stack = ExitStack()
            agg12p = agg12_stack.enter_context(tc.tile_pool(name="agg12", bufs=1))
            agg34_stack = ExitStack()
            agg34p = agg34_stack.enter_context(tc.tile_pool(name="agg34", bufs=1))
            e_stack = ExitStack()
            epool = e_stack.enter_context(tc.tile_pool(name="epool", bufs=1))
            h_stack = ExitStack()
            hpool = h_stack.enter_context(tc.tile_pool(name="hpool", bufs=1))
            agg0_stack = ExitStack()
            agg0p = agg0_stack.enter_context(tc.tile_pool(name="agg0p", bufs=1))

            h_sb = hpool.tile([P, NT * E], f8, tag="h8", name="h8")
            e_sb = epool.tile([P, NT * E], f8, tag="e8", name="e8")

            # local tiles first (rsqrt cols 0-3 feed the exp scales);
            # xe first: modules 1/5 need re^2 + xeTl before anything else
            load_group(xe_d, e_sb, ssq_e, re_t, 0, dve_stats=True)
            load_xT()
            nc.scalar.activation(out=re2_t[:], in_=re_t[:, 0:LT],
                                 func=AF.Square)

            # modules 1 and 5 need no aggregates - start DVE early
            # (xn group 0's ACT squares go after module 5 so they cannot
            # delay module 5's softmax exps in the ACT queue)
            # first two modules' weights via the ACT DGE queue so they don't
            # sit behind the body's x-tile loads in the SP queue at startup
            module(1, xeTl, xeTl, acc_h, re2_t, first=True, wdma=nc.scalar)
            module(5, xeTl, xeTl, acc_e, re2_t, first=True, wdma=nc.scalar)
            load_group(xn_d, h_sb, ssq_h, rh_t, 0)

            for g0 in range(4, NT, 4):
                load_group(xn_d, h_sb, ssq_h, rh_t, g0)
            for g0 in range(4, NT // 2, 4):
                load_group(xe_d, e_sb, ssq_e, re_t, g0)

            nc.sync.dma_start(out=b1h_t,
                                in_=b1h_d[:].rearrange("(c p) -> p c", p=P))
            nc.sync.dma_start(out=b1e_t,
                                in_=b1e_d[:].rearrange("(c p) -> p c", p=P))
            nc.gpsimd.dma_start(out=b2h_t,
                                in_=b2h_d[:].rearrange("(a e) -> a e", a=1))
            nc.gpsimd.dma_start(out=b2e_t,
                                in_=b2e_d[:].rearrange("(a e) -> a e", a=1))
            nc.gpsimd.dma_start(
                out=b1eq.rearrange("q (g p) -> q g p", p=P),
                in_=b1e_d[:].rearrange("(g q p) -> q g p", q=4, p=P))

            agg0 = aggregate(0, h_sb, agg0p)
            module(0, agg0, xnTl, acc_h, rh_t, first=False)
            module(4, agg0, xnTl, acc_e, rh_t, first=False)

            agg1 = aggregate(1, h_sb, agg12p)
            agg2 = aggregate(2, h_sb, agg12p)
            agg0_stack.close()

            module(2, xeTl, agg1, acc_h, re_t, first=False)
            h_stack.close()

            for g0 in range(NT // 2, NT, 4):
                load_group(xe_d, e_sb, ssq_e, re_t, g0)
            agg3 = aggregate(3, e_sb, agg34p)
            wt_h = ffn_wload(w1h_d, w2h_d)
            module(3, xnTl, agg3, acc_h, rh_t, first=False)

            agg4 = aggregate(4, e_sb, agg34p)
            e_stack.close()

            ffn(acc_h, wt_h, b1h_t, b2h_t, outh_d)
            wt_e = ffn_wload(w1e_d, w2e_d)
            module(7, xnTl, agg4, acc_e, rh_t, first=False, warmln=True)
            agg34_stack.close()
            with tc.tile_pool(name="ffnE_sb", bufs=1) as fsbE:
                ecb, etail = mk_ffn_pipe(acc_e, wt_e[0], wt_e[1], b1eq,
                                         b2e_t, oute_d, fsbE)
                module(6, xeTl, agg2, acc_e, re_t, first=False, tile_cb=ecb)
                etail()
            agg12_stack.close()

        for _ in range(repeat):
            body()

    _split_big_waits(nc, mybir)
    return nc


def _get_program():
    if "nc" not in _PROGRAM_CACHE:
        _PROGRAM_CACHE["nc"] = _build_program()
    return _PROGRAM_CACHE["nc"]


def _prep_inputs(x_node, x_edge, adj, Wq, Wk, Wv,
                 proj_he_h, proj_eh_h, proj_he_e, proj_eh_e,
                 rms1_h, rms1_e, rms2_h,
                 w1_h, b1_h, w2_h, b2_h, w1_e, b1_e, w2_e, b2_e):
    """Per-core input dicts. Weight folding + row rotation happen here."""
    from ml_dtypes import bfloat16, float8_e4m3
    f = np.float32
    bf = bfloat16
    f16 = np.float16
    f8 = float8_e4m3
    # mats quantized to fp8e4m3, pre-scaled into a good fp8 range; the
    # inverse scale is folded into the wq/wk of the module consuming the
    # aggregate (mat0=adj -> q of modules 0/4; mat1..4 -> k of 2,6,3,7).
    MSC = [4096.0, 64.0, 64.0, 64.0, 64.0]
    qsc = [1.0 / MSC[0], 1, 1, 1, 1.0 / MSC[0], 1, 1, 1]
    ksc = [1, 1, 1.0 / MSC[1], 1.0 / MSC[3], 1, 1, 1.0 / MSC[2], 1.0 / MSC[4]]
    wsrc_q = [rms1_h, rms1_e, rms1_e, rms1_h, rms1_h, rms1_e, rms1_e, rms1_h]
    wsrc_k = [rms1_h, rms1_e, rms1_h, rms1_e, rms1_h, rms1_e, rms1_h, rms1_e]
    wqT = np.stack([(Wq[m].T * wsrc_q[m][:, None]) * (0.125 * qsc[m])
                    for m in range(H)])
    wkT = np.stack([(Wk[m].T * wsrc_k[m][:, None]) * ksc[m] for m in range(H)])
    wvT = np.stack([Wv[m].T * rms1_h[:, None] for m in range(H)])
    w1hT = np.ascontiguousarray((w1_h * rms2_h[None, :]).T.astype(bf))
    w1eT = np.ascontiguousarray((w1_e * rms2_h[None, :]).T.astype(bf))
    w2hT = np.ascontiguousarray(w2_h.T.astype(f16))
    w2eT = np.ascontiguousarray(w2_e.T.astype(f16))
    mats = [adj, proj_eh_h, proj_eh_e, proj_he_h, proj_he_e]

    shared = dict(wqT=np.ascontiguousarray(wqT.astype(bf)),
                  wkT=np.ascontiguousarray(wkT.astype(bf)),
                  wvT=np.ascontiguousarray(wvT.astype(bf)),
                  w1hT=w1hT, w2hT=w2hT, w1eT=w1eT, w2eT=w2eT,
                  b1h=b1_h.astype(f), b2h=b2_h.astype(f),
                  b1e=b1_e.astype(f), b2e=b2_e.astype(f))
    xn_bf = x_node.astype(bf)
    xe_bf = x_edge.astype(bf)
    in_maps = []
    for c in range(NCORES):
        r0 = c * RPC
        m = dict(shared)
        m["xn"] = np.ascontiguousarray(np.roll(xn_bf, -r0, axis=0))
        m["xe"] = np.ascontiguousarray(np.roll(xe_bf, -r0, axis=0))
        m["xnT"] = np.ascontiguousarray(xn_bf[r0:r0 + RPC].T)
        m["xeT"] = np.ascontiguousarray(xe_bf[r0:r0 + RPC].T)
        for i, mat in enumerate(mats):
            mt = np.ascontiguousarray(
                (mat[r0:r0 + RPC].T * MSC[i]).astype(f8))  # [N, RPC]
            m[f"mat{i}"] = np.ascontiguousarray(np.roll(mt, -r0, axis=0))
        in_maps.append(m)
    return in_maps


def kernel(**inputs):
    from concourse.bass_utils import run_bass_kernel_spmd
    nc = _get_program()
    in_maps = _prep_inputs(**{k: np.asarray(v) for k, v in inputs.items()})
    res = run_bass_kernel_spmd(nc, in_maps, list(range(NCORES))).results
    x_h = np.concatenate([res[c]["outh"] for c in range(NCORES)], axis=0)
    x_e = np.concatenate([res[c]["oute"] for c in range(NCORES)], axis=0)
    return (x_h, x_e)



# revision 51
# speedup vs baseline: 8.5644x; 1.0256x over previous
"""Trainium2 Bass kernel for the gnn_message_passing block (8 NeuronCores).

Strategy (per core c, owning 512 global rows r = c*512..(c+1)*512):
  - Host rotates x rows by -r0 (owned rows first), pre-transposes the owned
    x block (raw feature-major), and converts all large tensors to 16-bit
    (bf16 weights/mats/x, f16 w2) -- halves HBM traffic and removes all
    casting DMAs (pure HWDGE byte copies).
  - rmsnorm scale-invariance tricks: the per-node inverse-rms r[n] is
    needed exactly (a) multiplied into the node-major aggregation inputs
    h_sb/e_sb and (b) as the exp() scale of the per-node softmax for the
    raw (non-aggregated) q/k sources.  The v path needs NO normalization:
    v = h@Wv for every module, so x_att is uniformly scaled by r_h[n],
    which the (scale-invariant) second rmsnorm removes exactly.  The
    feature-major q/k/v stationary sources are therefore RAW transposed x
    from the host -- no on-chip transposes for them.
  - rsqrt = exp(-0.5*ln(.)): ln+exp live in ONE activation table set
    (natural_log_exp_and_others) together with square/copy, so the whole
    kernel needs only ~4 ACT table loads (vs ~100 when mixing sqrt):
    nl_exp era -> gelu(FFN h) -> nl_exp era -> gelu(FFN e).
  - The five N x N aggregations (adj@h shared by modules 0/4, four proj@k)
    run in fp8e4m3 DoubleRow mode (0.5 PE cycles/row): mats are pre-scaled
    host-side into fp8 range (adj*4096, proj*64; inverse folded into the
    consuming module's wq/wk) and the normalized h/e aggregation inputs are
    quantized to fp8 mega-tiles.  DoubleRow dst is ISA-limited to psum
    partitions 0-63, so each 64-feature odd half accumulates in a scratch
    bank and is rebased to partitions 64-127 via a cheap identity matmul.
    Costs ~6e-3 extra rel-err (1e-2 total vs the 2e-2 gate), halves
    aggregation PE time AND mat HBM traffic.  512x512 projections bf16.
  - Per-node 8-head SDPA on DVE in f16 2x mode: broadcast-AP multiplies +
    halving-tree reduces (measured: TT=2x, TensorReduce/Pool/TTR=1x, so
    trees beat single reduces).  Softmax exp on ACT with per-node scale.
    GpSimd offload of the av-multiply is wired behind POOL_AV_MOD but OFF:
    the cost model charges Pool TT at 0.83ns/elem while real Q7 hardware
    runs 2-input ops ~2.2ns/elem, so the offload only looks good in sim.
  - Module processing interleaves with aggregations so PE and DVE overlap:
    loads | mod 1,5 | agg0 | mod 0,4 | agg1,2 | mod 2 | agg3 | mod 3 |
    agg4 | FFN-h | mod 7 | mod 6 + pipelined FFN-e
  - FFN-e is fully pipelined per node-block via a tile callback inside the
    last module: each 128-node block runs rmsnorm2 -> transpose -> w1 ->
    gelu -> w2 -> DMA while the remaining SDPA tiles are still on DVE.
    The per-ffb gelu bias is applied by one 4-deep indicator matmul per
    4-ffb group so the gelu runs 512 wide (4 ACT ops/block instead of 16).
    FFN-h stays serial (pipelining it into module 7 would thrash the
    gelu/exp ACT table sets) but its rmsnorm2 squares and out-copies run
    on DVE, which is idle there, so module 7's softmax exps aren't queued
    behind them on ACT.
  - DMA queues: x/mats/outputs/weights on SP (HWDGE), modules 1/5 weights
    on the ACT queue (ahead of the SP load burst at body start); only the
    casting bias loads remain on POOL SWDGE.  POOL otherwise does the
    cheap per-node normalization multiplies (1-input ops are line-rate on
    real Q7; 2-input ops are not).
  - Schedule details: the first xe node-major load is issued on SP BEFORE
    the eight transposed-x loads so the DVE stats fill starts ~3us earlier;
    a 28-transpose PE warm-up chain covers the p-state ramp (matmuls run at
    1.2GHz for the first 3us after idle, 2.4GHz after) so the first
    projections run at full clock; the FFN-e callback's rmsnorm squares run
    on ACT (which has slack inside module 6) rather than DVE.
  - CoreSim: 338.1us (DVE-bound: DVE busy ~305us, PE ~224, ACT ~210) vs
    350.8us for the bf16 predecessor; measured HW (repeat-differential)
    331us for the predecessor.  HW-verified rel-err 9.44e-3.
"""
import numpy as np

N = 4096
E = 512
H = 8
D = 64
FF = 2048
P = 128
NCORES = 8
RPC = N // NCORES  # 512 rows per core
NT = N // P        # 32 tiles over all nodes
LT = RPC // P      # 4 local tiles
EPS = float(np.finfo(np.float32).eps)
# run the av broadcast-multiply on GpSimd for tiles with
# t % POOL_AV_MOD == POOL_AV_PHASE (engine-balance knob; real-HW GpSimd
# tensor_tensor is ~4x slower than the cost model says, so keep this off)
POOL_AV_MOD = 1000
POOL_AV_PHASE = 1

_PROGRAM_CACHE = {}


def _split_big_waits(nc, mybir):
    """walrus in this toolchain rejects multi-wait instructions; cap at 1
    (2 for EventSemaphore), chaining the excess as EventSemaphores."""
    for f in nc.m.functions:
        for bb in f.blocks:
            insts = list(bb.instructions)
            out = []
            changed = False
            for inst in insts:
                si = inst.sync_info
                waits = list(si.on_wait) if si and si.on_wait else []
                cap = 2 if isinstance(inst, mybir.InstEventSemaphore) else 1
                if len(waits) > cap:
                    extra, keep = waits[:-cap], waits[-cap:]
                    for ci in range(0, len(extra), 2):
                        ev = mybir.InstEventSemaphore(name=f"{inst.name}-evw{ci}")
                        ev.engine = inst.engine
                        ev.sync_info = mybir.SyncInfo(on_wait=extra[ci:ci + 2],
                                                      on_update=[])
                        out.append(ev)
                    si.on_wait = keep
                    changed = True
                out.append(inst)
            if changed:
                bb.instructions[:] = out


def _build_program(repeat=1):
    import concourse.bass as bass
    import concourse.tile as tile
    from concourse import mybir
    from concourse.masks import make_identity
    from contextlib import ExitStack

    f32 = mybir.dt.float32
    f32r = mybir.dt.float32r
    f16 = mybir.dt.float16
    bf16 = mybir.dt.bfloat16
    f8 = mybir.dt.float8e4
    AF = mybir.ActivationFunctionType
    OP = mybir.AluOpType
    AX = mybir.AxisListType
    DR = mybir.MatmulPerfMode.DoubleRow

    def bc(t, dims, off=0):
        return bass.AP(tensor=t.tensor, offset=t.offset + off,
                       ap=[list(t.ap[0])] + [[s, c] for (s, c) in dims])

    nc = bass.Bass()

    xn_d = nc.declare_dram_parameter("xn", [N, E], bf16, isOutput=False)
    xe_d = nc.declare_dram_parameter("xe", [N, E], bf16, isOutput=False)
    xnT_d = nc.declare_dram_parameter("xnT", [E, RPC], bf16, isOutput=False)
    xeT_d = nc.declare_dram_parameter("xeT", [E, RPC], bf16, isOutput=False)
    mat_d = [nc.declare_dram_parameter(f"mat{i}", [N, RPC], f8, isOutput=False)
             for i in range(5)]
    wq_d = nc.declare_dram_parameter("wqT", [H, E, E], bf16, isOutput=False)
    wk_d = nc.declare_dram_parameter("wkT", [H, E, E], bf16, isOutput=False)
    wv_d = nc.declare_dram_parameter("wvT", [H, E, E], bf16, isOutput=False)
    w1h_d = nc.declare_dram_parameter("w1hT", [E, FF], bf16, isOutput=False)
    w2h_d = nc.declare_dram_parameter("w2hT", [FF, E], f16, isOutput=False)
    w1e_d = nc.declare_dram_parameter("w1eT", [E, FF], bf16, isOutput=False)
    w2e_d = nc.declare_dram_parameter("w2eT", [FF, E], f16, isOutput=False)
    b1h_d = nc.declare_dram_parameter("b1h", [FF], f32, isOutput=False)
    b2h_d = nc.declare_dram_parameter("b2h", [E], f32, isOutput=False)
    b1e_d = nc.declare_dram_parameter("b1e", [FF], f32, isOutput=False)
    b2e_d = nc.declare_dram_parameter("b2e", [E], f32, isOutput=False)
    outh_d = nc.declare_dram_parameter("outh", [RPC, E], f32, isOutput=True)
    oute_d = nc.declare_dram_parameter("oute", [RPC, E], f32, isOutput=True)

    with tile.TileContext(nc, pool_alloc_mode="queue") as tc, ExitStack() as ctx:
        consts = ctx.enter_context(tc.tile_pool(name="consts", bufs=1))
        eps_t = consts.tile([P, 1], f32)
        nc.vector.memset(eps_t, EPS)
        lnwarm = consts.tile([P, 1], f32)
        # first ACT op: pins the natural_log_exp_and_others table set
        nc.scalar.activation(out=lnwarm[:], in_=eps_t[:], func=AF.Ln)
        ident = consts.tile([P, P], f16)
        make_identity(nc, ident)
        ones1f = consts.tile([1, P], f32)
        nc.gpsimd.memset(ones1f, 1.0)
        ones1 = consts.tile([1, P], f32r)
        nc.scalar.copy(ones1[:], ones1f[:])
        b1h_t = consts.tile([P, FF // P], f32)
        b1e_t = consts.tile([P, FF // P], f32)
        b2h_t = consts.tile([1, E], f32r)
        b2e_t = consts.tile([1, E], f32r)
        # indicator eye(4) (x) ones(128) + b1 rows: lets the pipelined FFN
        # apply the per-ffb gelu bias via one 4-deep matmul so 4 ffb blocks
        # share a single wide gelu
        ind4f = consts.tile([4, E], f32)
        nc.gpsimd.memset(ind4f, 0.0)
        for q in range(4):
            # partition-offset writes need a DMA (engines can't start at
            # partition q); tiny one-time init copies
            nc.sync.dma_start(out=ind4f[q:q + 1, q * P:(q + 1) * P],
                              in_=ones1f[0:1, :])
        ind4 = consts.tile([4, E], f32r)
        nc.scalar.copy(ind4[:], ind4f[:])

        b1eq = consts.tile([4, 4 * P], f32r)

        # whole-program pools
        statp = ctx.enter_context(tc.tile_pool(name="stat", bufs=1))
        rawp = ctx.enter_context(tc.tile_pool(name="raw", bufs=1))
        sqscp = ctx.enter_context(tc.tile_pool(name="sqsc", bufs=1))
        xTlp = ctx.enter_context(tc.tile_pool(name="xTl", bufs=1))
        wpool = ctx.enter_context(tc.tile_pool(name="wts", bufs=1))
        qkvp = ctx.enter_context(tc.tile_pool(name="qkv", bufs=1))
        tmpp = ctx.enter_context(tc.tile_pool(name="sdtmp", bufs=1))
        smp = ctx.enter_context(tc.tile_pool(name="sdsm", bufs=2))
        accp = ctx.enter_context(tc.tile_pool(name="acc", bufs=1))
        matgp = ctx.enter_context(tc.tile_pool(name="matg", bufs=2))
        fwts = ctx.enter_context(tc.tile_pool(name="fwts", bufs=1))
        psp = ctx.enter_context(tc.tile_pool(name="ps", bufs=1, space="PSUM"))

        # PE p-state warm-up: dummy transpose chain covering the first ~3us
        # so the p-state ramp completes before the first real projections
        warmps = psp.tile([P, P], f16, tag="projps", bufs=4, name="warm")
        for _ in range(28):
            nc.tensor.transpose(warmps[:], ident[:], ident[:])

        def body():
            # per-iteration stat tiles
            ssq_h = statp.tile([P, NT], f32, tag="ssqh", name="ssqh")
            ssq_e = statp.tile([P, NT], f32, tag="ssqe", name="ssqe")
            lnsc = statp.tile([P, NT], f32, tag="lnsc", name="lnsc")
            rh_t = statp.tile([P, NT], f32, tag="rh", name="rh")
            re_t = statp.tile([P, NT], f32, tag="re", name="re")
            re2_t = statp.tile([P, LT], f32, tag="re2", name="re2")

            xnTl = [xTlp.tile([P, RPC], bf16, tag=f"xnT{fc}", name=f"xnT{fc}")
                    for fc in range(4)]
            xeTl = [xTlp.tile([P, RPC], bf16, tag=f"xeT{fc}", name=f"xeT{fc}")
                    for fc in range(4)]
            def load_xT():
                for fc in range(4):
                    nc.sync.dma_start(out=xeTl[fc][:],
                                      in_=xeT_d[fc * P:(fc + 1) * P, :])
                for fc in range(4):
                    nc.sync.dma_start(out=xnTl[fc][:],
                                      in_=xnT_d[fc * P:(fc + 1) * P, :])

            acc_h = [accp.tile([P, E], f16, tag=f"ah{t}", name=f"ah{t}")
                     for t in range(LT)]
            acc_e = [accp.tile([P, E], f16, tag=f"ae{t}", name=f"ae{t}")
                     for t in range(LT)]

            def load_group(x_d, sb8, ssq, rdst, g0, dve_stats=False):
                """Load 4 node-major tiles, square-accum stats, rsqrt the 4
                columns, normalize into the fp8 aggregation mega-tile sb8.
                dve_stats puts the squares on DVE (for the first group, while
                DVE is idle and ACT latency gates the first module)."""
                xg = rawp.tile([P, 4 * E], bf16, tag="xraw", bufs=2,
                               name="xraw")
                nc.sync.dma_start(
                    out=xg.rearrange("p (t e) -> p t e", e=E),
                    in_=x_d[g0 * P:(g0 + 4) * P, :].rearrange(
                        "(t p) e -> p t e", p=P))
                for t in range(4):
                    ti = g0 + t
                    scr = sqscp.tile([P, E], f16, tag="sq", bufs=1, name="sq")
                    if dve_stats:
                        nc.vector.scalar_tensor_tensor(
                            out=scr[:], in0=xg[:, t * E:(t + 1) * E],
                            scalar=0.0, in1=xg[:, t * E:(t + 1) * E],
                            op0=OP.add, op1=OP.mult,
                            accum_out=ssq[:, ti:ti + 1])
                    else:
                        nc.scalar.activation(out=scr[:],
                                             in_=xg[:, t * E:(t + 1) * E],
                                             func=AF.Square,
                                             accum_out=ssq[:, ti:ti + 1])
                nc.scalar.activation(out=lnsc[:, g0:g0 + 4],
                                     in_=ssq[:, g0:g0 + 4], func=AF.Ln,
                                     scale=1.0 / E, bias=eps_t[:])
                nc.scalar.activation(out=rdst[:, g0:g0 + 4],
                                     in_=lnsc[:, g0:g0 + 4], func=AF.Exp,
                                     scale=-0.5)
                for t in range(4):
                    ti = g0 + t
                    nc.gpsimd.tensor_scalar_mul(sb8[:, ti * E:(ti + 1) * E],
                                                xg[:, t * E:(t + 1) * E],
                                                rdst[:, ti:ti + 1])

            def aggregate(mi, src8, aggpool):
                """4 feature-major bf16 [128, 512] blocks of mat_mi @ x.

                fp8e4 DoubleRow matmuls: both operands fp8 (mats pre-scaled
                host-side; inverse scale folded into wq/wk), 2 node-tiles
                (256 contraction rows) per pass at 0.5 cycles/row.  Output
                partition limit is 64, so each psum bank holds a pair of
                64-feature blocks at partition offsets 0/64 and the bf16
                copy-out still reads one [128, 512] bank."""
                pss = [psp.tile([P, E], f32, tag=f"agps{b}", name=f"agps{b}")
                       for b in range(4)]
                scr = [psp.tile([64, E], f32, tag="projps", bufs=4,
                                name=f"agsc{b}") for b in range(4)]
                for g in range(8):
                    mt = matgp.tile([P, 4 * RPC], f8, tag="matg", name="matg")
                    nc.sync.dma_start(
                        out=mt.rearrange("p (t e) -> p t e", e=RPC),
                        in_=mat_d[mi][g * 4 * P:(g + 1) * 4 * P, :].rearrange(
                            "(t p) e -> p t e", p=P))
                    for tp in range(2):
                        pair = g * 2 + tp
                        for b in range(4):
                            # DoubleRow dst is ISA-limited to partitions
                            # 0-63: even feature-half accumulates in
                            # pss[b][0:64], odd half in a scratch bank and
                            # is rebased to [64:128] afterwards.
                            for half in range(2):
                                dst = pss[b][0:64, :] if half == 0 else scr[b][:]
                                nc.tensor.matmul(
                                    dst,
                                    lhsT=bc(src8, [(E, 2), (1, 64)],
                                            off=pair * 2 * E + (2 * b + half) * 64),
                                    rhs=bc(mt, [(RPC, 2), (1, RPC)],
                                           off=tp * 2 * RPC),
                                    start=(pair == 0), stop=(pair == 15),
                                    perf_mode=DR,
                                    tile_position=(0, 0),
                                    skip_group_check=True)
                outt = []
                for b in range(4):
                    t64 = sqscp.tile([64, E], f16, tag="agt64", bufs=2,
                                     name="agt64")
                    nc.scalar.copy(t64[:], scr[b][:])
                    nc.tensor.matmul(pss[b][64:128, :],
                                     lhsT=ident[0:64, 0:64], rhs=t64[:],
                                     start=True, stop=True,
                                     tile_position=(0, 64),
                                     skip_group_check=True)
                    at = aggpool.tile([P, E], bf16, tag=f"ag{mi}_{b}",
                                      name=f"ag{mi}_{b}")
                    nc.scalar.copy(at[:], pss[b][:])
                    outt.append(at)
                return outt

            def module(m, qsrcT, ksrcT, branch_att, rsc, first, warmln=False,
                       tile_cb=None, wdma=None):
                w_ts = {}
                for (dram, nm) in ((wq_d, "wq"), (wk_d, "wk"), (wv_d, "wv")):
                    wt = wpool.tile([P, 4 * E], bf16, tag=nm,
                                    bufs=(1 if nm == "wv" else 2),
                                    name=f"w_{nm}")
                    (wdma or nc.sync).dma_start(
                        out=wt.rearrange("p (fc e) -> p fc e", e=E),
                        in_=dram[m].rearrange("(fc p) e -> p fc e", p=P))
                    w_ts[nm] = wt
                if warmln:
                    # re-pin the ln+exp table set after a gelu era
                    nc.scalar.activation(out=lnwarm[:], in_=eps_t[:],
                                         func=AF.Ln)

                # pair-wide SDPA (two tiles per DVE instruction) was tried
                # and REGRESSED (CoreSim 353.5 vs 343.4): the per-op saving is
                # only ~60ns while the coarser q/k/v pair dependencies open a
                # ~10us module-boundary stall; keep per-tile issue
                wide = False

                # per-tile interleave (q_b, k_b, v_b) so tile 0's SDPA can
                # start after 12 matmuls instead of 36
                q_sb, k_sb, v_sb = [], [], []
                for b in range(LT):
                    for (srcT, wnm, lst) in ((qsrcT, "wq", q_sb),
                                             (ksrcT, "wk", k_sb),
                                             (xnTl, "wv", v_sb)):
                        wt = w_ts[wnm]
                        ps = psp.tile([P, E], f32, tag="projps", bufs=4,
                                      name="projps")
                        for fc in range(4):
                            nc.tensor.matmul(
                                ps[:],
                                lhsT=srcT[fc][:, b * P:(b + 1) * P],
                                rhs=wt[:, fc * E:(fc + 1) * E],
                                start=(fc == 0), stop=(fc == 3))
                        dt = qkvp.tile([P, E], f16, tag=f"{wnm}_{b}",
                                       bufs=(2 if wnm == "wq" else 1),
                                       name=f"qkv{b}")
                        lst.append(dt)
                        if wnm == "wv":
                            nc.scalar.copy(bc(dt, [(1, 8), (8, 64)]), ps[:])
                        else:
                            nc.scalar.copy(dt[:], ps[:])

                if wide:
                    W = H * H * D  # 4096: one tile's score/av scratch
                    pair_st = {}

                    def phase_a(pb):
                        q2, k2, v2 = q_sb[pb], k_sb[pb], v_sb[pb]
                        tmp = tmpp.tile([P, 2 * W], f16, tag="sdpa", bufs=2,
                                        name="sdpa")
                        nc.vector.tensor_tensor(
                            out=bc(tmp, [(W, 2), (512, 8), (64, 8), (1, 64)]),
                            in0=bc(q2, [(E, 2), (64, 8), (0, 8), (1, 64)]),
                            in1=bc(k2, [(E, 2), (0, 8), (64, 8), (1, 64)]),
                            op=OP.mult)
                        for dd in (32, 16, 8, 4, 2):
                            nc.vector.tensor_tensor(
                                out=bc(tmp, [(W, 2), (64, 64), (1, dd)]),
                                in0=bc(tmp, [(W, 2), (64, 64), (1, dd)]),
                                in1=bc(tmp, [(W, 2), (64, 64), (1, dd)],
                                       off=dd),
                                op=OP.add)
                        s2 = smp.tile([P, 2 * H * H], f16, tag="sw", name="sw")
                        nc.vector.tensor_tensor(
                            out=bc(s2, [(64, 2), (1, 64)]),
                            in0=bc(tmp, [(W, 2), (64, 64)]),
                            in1=bc(tmp, [(W, 2), (64, 64)], off=1),
                            op=OP.add)
                        ex2 = smp.tile([P, 2 * H * H], f16, tag="exw",
                                       name="exw")
                        for t2 in range(2):
                            t = 2 * pb + t2
                            nc.scalar.activation(
                                out=ex2[:, t2 * 64:(t2 + 1) * 64],
                                in_=s2[:, t2 * 64:(t2 + 1) * 64],
                                func=AF.Exp, scale=rsc[:, t:t + 1])
                        pair_st[pb] = (ex2, v2)

                    def phase_b(pb):
                        ex2, v2 = pair_st[pb]
                        den2 = smp.tile([P, 2 * H], f32, tag="denw",
                                        name="denw")
                        nc.vector.tensor_reduce(
                            out=den2[:],
                            in_=ex2.rearrange("p (x g) -> p x g", g=H),
                            axis=AX.X, op=OP.add)
                        rden2 = smp.tile([P, 2 * H], f32, tag="rdenw",
                                         name="rdenw")
                        nc.vector.reciprocal(out=rden2[:], in_=den2[:])
                        a2 = smp.tile([P, 2 * H * H], f16, tag="aw", name="aw")
                        nc.vector.tensor_tensor(
                            out=bc(a2, [(64, 2), (8, 8), (1, 8)]),
                            in0=bc(ex2, [(64, 2), (8, 8), (1, 8)]),
                            in1=bc(rden2, [(8, 2), (1, 8), (0, 8)]),
                            op=OP.mult)
                        tmp2 = tmpp.tile([P, 2 * W], f16, tag="sdpa2", bufs=1,
                                         name="sdpa2")
                        nc.vector.tensor_tensor(
                            out=bc(tmp2, [(W, 2), (512, 8), (8, 64), (1, 8)]),
                            in0=bc(a2, [(64, 2), (8, 8), (0, 64), (1, 8)]),
                            in1=bc(v2, [(E, 2), (0, 8), (8, 64), (1, 8)]),
                            op=OP.mult)
                        for gg in (4, 2):
                            nc.vector.tensor_tensor(
                                out=bc(tmp2, [(W, 2), (8, 512), (1, gg)]),
                                in0=bc(tmp2, [(W, 2), (8, 512), (1, gg)]),
                                in1=bc(tmp2, [(W, 2), (8, 512), (1, gg)],
                                       off=gg),
                                op=OP.add)
                        for t2 in range(2):
                            t = 2 * pb + t2
                            ofs = t2 * W
                            if first:
                                nc.vector.tensor_tensor(
                                    out=branch_att[t][:],
                                    in0=bc(tmp2, [(8, 512)], off=ofs),
                                    in1=bc(tmp2, [(8, 512)], off=ofs + 1),
                                    op=OP.add)
                            else:
                                rt = smp.tile([P, E], f16, tag="avred",
                                              bufs=1, name="avred")
                                nc.vector.tensor_tensor(
                                    out=rt[:],
                                    in0=bc(tmp2, [(8, 512)], off=ofs),
                                    in1=bc(tmp2, [(8, 512)], off=ofs + 1),
                                    op=OP.add)
                                nc.vector.tensor_tensor(out=branch_att[t][:],
                                                        in0=branch_att[t][:],
                                                        in1=rt[:], op=OP.add)

                    # software-pipeline: both pairs' score+exp first, so the
                    # second pair's DVE work hides the first pair's ACT
                    # round-trip before the softmax/AV half needs ex2
                    phase_a(0)
                    phase_a(1)
                    phase_b(0)
                    phase_b(1)
                    return

                for t in range(LT):
                    q_t, k_t, v_t = q_sb[t], k_sb[t], v_sb[t]
                    tmp = tmpp.tile([P, H * H * D], f16, tag="sdpan",
                                    bufs=1, name="sdpan")
                    nc.vector.tensor_tensor(
                        out=bc(tmp, [(512, 8), (64, 8), (1, 64)]),
                        in0=bc(q_t, [(64, 8), (0, 8), (1, 64)]),
                        in1=bc(k_t, [(0, 8), (64, 8), (1, 64)]),
                        op=OP.mult)
                    for dd in (32, 16, 8, 4, 2):
                        nc.vector.tensor_tensor(
                            out=bc(tmp, [(64, 64), (1, dd)]),
                            in0=bc(tmp, [(64, 64), (1, dd)]),
                            in1=bc(tmp, [(64, 64), (1, dd)], off=dd),
                            op=OP.add)
                    s_t = smp.tile([P, H * H], f16, tag="s", name="s")
                    nc.vector.tensor_tensor(
                        out=s_t[:],
                        in0=bc(tmp, [(64, 64)]),
                        in1=bc(tmp, [(64, 64)], off=1),
                        op=OP.add)
                    ex_t = smp.tile([P, H * H], f16, tag="ex", name="ex")
                    nc.scalar.activation(out=ex_t[:], in_=s_t[:], func=AF.Exp,
                                         scale=rsc[:, t:t + 1])
                    den = smp.tile([P, H], f32, tag="den", name="den")
                    nc.vector.tensor_reduce(
                        out=den[:], in_=ex_t.rearrange("p (h g) -> p h g", g=H),
                        axis=AX.X, op=OP.add)
                    rden = smp.tile([P, H], f32, tag="rden", name="rden")
                    nc.vector.reciprocal(out=rden[:], in_=den[:])
                    a_t = smp.tile([P, H * H], f16, tag="a", name="a")
                    nc.vector.tensor_tensor(
                        out=bc(a_t, [(8, 8), (1, 8)]),
                        in0=bc(ex_t, [(8, 8), (1, 8)]),
                        in1=bc(rden, [(1, 8), (0, 8)]),
                        op=OP.mult)
                    tmp2 = tmpp.tile([P, H * H * D], f16, tag="sdpa2n",
                                     bufs=2, name="sdpa2n")
                    nc.vector.tensor_tensor(
                        out=bc(tmp2, [(512, 8), (8, 64), (1, 8)]),
                        in0=bc(a_t, [(8, 8), (0, 64), (1, 8)]),
                        in1=bc(v_t, [(0, 8), (8, 64), (1, 8)]),
                        op=OP.mult)
                    for gg in (4, 2):
                        nc.vector.tensor_tensor(
                            out=bc(tmp2, [(8, 512), (1, gg)]),
                            in0=bc(tmp2, [(8, 512), (1, gg)]),
                            in1=bc(tmp2, [(8, 512), (1, gg)], off=gg),
                            op=OP.add)
                    if first:
                        nc.vector.tensor_tensor(
                            out=branch_att[t][:],
                            in0=bc(tmp2, [(8, 512)]),
                            in1=bc(tmp2, [(8, 512)], off=1),
                            op=OP.add)
                    else:
                        rt = smp.tile([P, E], f16, tag="avred", bufs=1,
                                      name="avred")
                        nc.vector.tensor_tensor(
                            out=rt[:],
                            in0=bc(tmp2, [(8, 512)]),
                            in1=bc(tmp2, [(8, 512)], off=1),
                            op=OP.add)
                        nc.vector.tensor_tensor(out=branch_att[t][:],
                                                in0=branch_att[t][:],
                                                in1=rt[:], op=OP.add)
                    if tile_cb is not None:
                        tile_cb(t)

            def ffn_wload(w1_dram, w2_dram):
                """Weight tiles + DMAs; emit early to hide the transfers."""
                w1_ts = []
                for half in range(2):
                    HW1 = FF // 2
                    w1_t = fwts.tile([P, 4 * HW1], bf16, tag=f"w1_{half}",
                                     name=f"w1_{half}")
                    nc.sync.dma_start(
                        out=w1_t.rearrange("p (fc e) -> p fc e", e=HW1),
                        in_=w1_dram[:, half * HW1:(half + 1) * HW1].rearrange(
                            "(fc p) e -> p fc e", p=P))
                    w1_ts.append(w1_t)
                return (w1_ts, w2_dram)

            def ffn(branch_att, wtiles, b1_t, b2_t, out_dram):
                w1_ts, w2_dram = wtiles
                with tc.tile_pool(name="ffn_sb", bufs=1) as fsb:
                    w2_t = fsb.tile([P, 16 * E], f16, tag="w2", name="w2")
                    nc.sync.dma_start(
                        out=w2_t.rearrange("p (fc e) -> p fc e", e=E),
                        in_=w2_dram[:, :].rearrange("(fc p) e -> p fc e", p=P))
                    # rmsnorm2: stats + rsqrt + normalize (f16); squares on
                    # DVE (idle here) so module 7's softmax exps aren't stuck
                    # behind them in the ACT queue
                    ssq2 = statp.tile([P, LT], f32, tag="fss", name="fss")
                    for t in range(LT):
                        scr = sqscp.tile([P, E], f16, tag="sq", bufs=1,
                                         name="fsq")
                        nc.vector.scalar_tensor_tensor(
                            out=scr[:], in0=branch_att[t][:], scalar=0.0,
                            in1=branch_att[t][:], op0=OP.add, op1=OP.mult,
                            accum_out=ssq2[:, t:t + 1])
                    ln2 = statp.tile([P, LT], f32, tag="fln", name="fln")
                    nc.scalar.activation(out=ln2[:], in_=ssq2[:], func=AF.Ln,
                                         scale=1.0 / E, bias=eps_t[:])
                    rs2 = statp.tile([P, LT], f32, tag="frs", name="frs")
                    nc.scalar.activation(out=rs2[:], in_=ln2[:], func=AF.Exp,
                                         scale=-0.5)
                    xn_tiles = []
                    for t in range(LT):
                        xt = sqscp.tile([P, E], f16, tag=f"ffx{t}",
                                        bufs=1, name=f"fx{t}")
                        nc.gpsimd.tensor_scalar_mul(xt[:], branch_att[t][:],
                                                    rs2[:, t:t + 1])
                        xn_tiles.append(xt)
                    xnT = []
                    for fc in range(4):
                        ps = psp.tile([P, RPC], f16, tag="agps0", name="ftr")
                        for t in range(4):
                            nc.tensor.transpose(ps[:, t * P:(t + 1) * P],
                                                xn_tiles[t][:, fc * P:(fc + 1) * P],
                                                ident[:])
                        xt = fsb.tile([P, RPC], bf16, tag=f"fxT{fc}",
                                      name=f"fxT{fc}")
                        nc.scalar.copy(xt[:], ps[:])
                        xnT.append(xt)
                    g1 = []
                    HW1 = FF // 2
                    for half in range(2):
                        w1_t = w1_ts[half]
                        for fb in range(HW1 // P):
                            ffb = half * (HW1 // P) + fb
                            ps = psp.tile([P, RPC], f32, tag=f"agps{1 + ffb % 2}",
                                          name="fps1")
                            for fc in range(4):
                                nc.tensor.matmul(
                                    ps[:],
                                    lhsT=w1_t[:, fc * HW1 + fb * P:
                                              fc * HW1 + (fb + 1) * P],
                                    rhs=xnT[fc][:],
                                    start=(fc == 0), stop=(fc == 3))
                            gt = fsb.tile([P, RPC], f16, tag=f"g1_{ffb}",
                                          name=f"g1_{ffb}")
                            nc.scalar.activation(out=gt[:], in_=ps[:],
                                                 func=AF.Gelu,
                                                 bias=b1_t[:, ffb:ffb + 1],
                                                 scale=1.0)
                            g1.append(gt)
                    for b in range(LT):
                        ps = psp.tile([P, E], f32, tag="agps3", name="fps2")
                        for ffc in range(FF // P):
                            nc.tensor.matmul(
                                ps[:],
                                lhsT=g1[ffc][:, b * P:(b + 1) * P],
                                rhs=w2_t[:, ffc * E:(ffc + 1) * E],
                                start=(ffc == 0), stop=False)
                        nc.tensor.matmul(ps[:], lhsT=ones1[:], rhs=b2_t[:],
                                         start=False, stop=True)
                        ob = fsb.tile([P, E], f32, tag="fo", bufs=2, name="fo")
                        nc.vector.tensor_scalar_mul(ob[:], ps[:], 1.0)
                        nc.sync.dma_start(
                            out=out_dram[b * P:(b + 1) * P, :], in_=ob[:])

            def mk_ffn_pipe(branch_att, w1_ts, w2_dram, b1q, b2_t, out_dram,
                            fsb):
                """Per-tile pipelined FFN: the returned callback is invoked
                inside the final module after each tile's SDPA, so nearly the
                whole FFN runs under the module's remaining SDPA work.  Node
                block t is pushed through rmsnorm2 -> transpose -> w1 -> gelu
                -> w2 as soon as its attention accumulator is final; only
                block 3's chain is exposed at the end."""
                HW1 = FF // 2
                st = {
                    "ssq2": fsb.tile([P, LT], f32, tag="fss", name="fss"),
                    "ln2": fsb.tile([P, LT], f32, tag="fln", name="fln"),
                    "rs2": fsb.tile([P, LT], f32, tag="frs", name="frs"),
                    "psT": [psp.tile([P, RPC], f16, tag=f"agps{fc}",
                                     name=f"eftr{fc}") for fc in range(4)],
                    "xnT": [fsb.tile([P, RPC], bf16, tag=f"fxT{fc}",
                                     name=f"fxT{fc}") for fc in range(4)],
                    "g1g": [fsb.tile([P, 4 * RPC], f16, tag=f"g1g_{g}",
                                     name=f"g1g_{g}") for g in range(4)],
                    "w2": fsb.tile([P, 16 * E], f16, tag="w2", name="w2"),
                    "ps2": [None] * LT,
                }

                def out_block(b):
                    ob = fsb.tile([P, E], f32, tag="fo", bufs=2, name="fo")
                    nc.vector.tensor_scalar_mul(ob[:], st["ps2"][b][:], 1.0)
                    nc.sync.dma_start(out=out_dram[b * P:(b + 1) * P, :],
                                      in_=ob[:])

                def block_chain(b):
                    """w1 -> gelu -> w2 for node block b (emitted one SDPA
                    tile later so its ACT ops sit behind the next exp in the
                    queue and can't stall the softmax chain)."""
                    lo, hi = b * P, (b + 1) * P
                    if b > 0:
                        out_block(b - 1)
                    for g in range(4):
                        ps4 = psp.tile([P, 4 * P], f32, tag="projps", bufs=4,
                                       name="efps1")
                        # the bias matmul OPENS the bank (start=True zeroes
                        # the whole 2KB zero-region, so per-quarter starts
                        # would wipe sibling quarters); quarters accumulate
                        nc.tensor.matmul(ps4[:],
                                         lhsT=b1q[0:4, g * P:(g + 1) * P],
                                         rhs=ind4[:], start=True, stop=False,
                                         skip_group_check=True)
                        for qq in range(4):
                            ffb = g * 4 + qq
                            w1_t = w1_ts[ffb // 8]
                            fb = ffb % 8
                            for fc in range(4):
                                nc.tensor.matmul(
                                    ps4[:, qq * P:(qq + 1) * P],
                                    lhsT=w1_t[:, fc * HW1 + fb * P:
                                              fc * HW1 + (fb + 1) * P],
                                    rhs=st["xnT"][fc][:, lo:hi],
                                    start=False,
                                    stop=(qq == 3 and fc == 3),
                                    skip_group_check=True)
                        nc.scalar.activation(
                            out=bc(st["g1g"][g], [(RPC, 4), (1, P)], off=lo),
                            in_=ps4[:], func=AF.Gelu, scale=1.0)
                    ps2 = psp.tile([P, E], f32, tag="projps", bufs=4,
                                    name="efps2")
                    for ffc in range(FF // P):
                        nc.tensor.matmul(
                            ps2[:],
                            lhsT=st["g1g"][ffc // 4][:, (ffc % 4) * RPC + lo:
                                                     (ffc % 4) * RPC + hi],
                            rhs=st["w2"][:, ffc * E:(ffc + 1) * E],
                            start=(ffc == 0), stop=False)
                    nc.tensor.matmul(ps2[:], lhsT=ones1[:], rhs=b2_t[:],
                                     start=False, stop=True)
                    st["ps2"][b] = ps2

                def cb(t):
                    if t == 0:
                        nc.sync.dma_start(
                            out=st["w2"].rearrange("p (fc e) -> p fc e", e=E),
                            in_=w2_dram[:, :].rearrange("(fc p) e -> p fc e",
                                                        p=P))
                    lo, hi = t * P, (t + 1) * P
                    scr = sqscp.tile([P, E], f16, tag="sq", bufs=1, name="esq")
                    nc.scalar.activation(out=scr[:], in_=branch_att[t][:],
                                         func=AF.Square,
                                         accum_out=st["ssq2"][:, t:t + 1])
                    nc.scalar.activation(out=st["ln2"][:, t:t + 1],
                                         in_=st["ssq2"][:, t:t + 1],
                                         func=AF.Ln, scale=1.0 / E,
                                         bias=eps_t[:])
                    nc.scalar.activation(out=st["rs2"][:, t:t + 1],
                                         in_=st["ln2"][:, t:t + 1],
                                         func=AF.Exp, scale=-0.5)
                    xt = fsb.tile([P, E], f16, tag="fx", bufs=2, name="fx")
                    nc.gpsimd.tensor_scalar_mul(xt[:], branch_att[t][:],
                                                st["rs2"][:, t:t + 1])
                    for fc in range(4):
                        nc.tensor.transpose(st["psT"][fc][:, lo:hi],
                                            xt[:, fc * P:(fc + 1) * P],
                                            ident[:])
                        nc.scalar.copy(st["xnT"][fc][:, lo:hi],
                                       st["psT"][fc][:, lo:hi])
                    if t > 0:
                        block_chain(t - 1)

                def tail():
                    block_chain(LT - 1)
                    out_block(LT - 1)

                return cb, tail

            # ======== emission order (the schedule) ========
            agg12_stack = ExitStack()
            agg12p = agg12_stack.enter_context(tc.tile_pool(name="agg12", bufs=1))
            agg34_stack = ExitStack()
            agg34p = agg34_stack.enter_context(tc.tile_pool(name="agg34", bufs=1))
            e_stack = ExitStack()
            epool = e_stack.enter_context(tc.tile_pool(name="epool", bufs=1))
            h_stack = ExitStack()
            hpool = h_stack.enter_context(tc.tile_pool(name="hpool", bufs=1))
            agg0_stack = ExitStack()
            agg0p = agg0_stack.enter_context(tc.tile_pool(name="agg0p", bufs=1))

            h_sb = hpool.tile([P, NT * E], f8, tag="h8", name="h8")
            e_sb = epool.tile([P, NT * E], f8, tag="e8", name="e8")

            # local tiles first (rsqrt cols 0-3 feed the exp scales);
            # xe first: modules 1/5 need re^2 + xeTl before anything else
            load_group(xe_d, e_sb, ssq_e, re_t, 0, dve_stats=True)
            load_xT()
            nc.scalar.activation(out=re2_t[:], in_=re_t[:, 0:LT],
                                 func=AF.Square)

            # modules 1 and 5 need no aggregates - start DVE early
            # (xn group 0's ACT squares go after module 5 so they cannot
            # delay module 5's softmax exps in the ACT queue)
            # first two modules' weights via the ACT DGE queue so they don't
            # sit behind the body's x-tile loads in the SP queue at startup
            module(1, xeTl, xeTl, acc_h, re2_t, first=True, wdma=nc.scalar)
            module(5, xeTl, xeTl, acc_e, re2_t, first=True, wdma=nc.scalar)
            load_group(xn_d, h_sb, ssq_h, rh_t, 0)

            for g0 in range(4, NT, 4):
                load_group(xn_d, h_sb, ssq_h, rh_t, g0)
            for g0 in range(4, NT // 2, 4):
                load_group(xe_d, e_sb, ssq_e, re_t, g0)

            nc.sync.dma_start(out=b1h_t,
                                in_=b1h_d[:].rearrange("(c p) -> p c", p=P))
            nc.sync.dma_start(out=b1e_t,
                                in_=b1e_d[:].rearrange("(c p) -> p c", p=P))
            nc.gpsimd.dma_start(out=b2h_t,
                                in_=b2h_d[:].rearrange("(a e) -> a e", a=1))
            nc.gpsimd.dma_start(out=b2e_t,
                                in_=b2e_d[:].rearrange("(a e) -> a e", a=1))
            nc.gpsimd.dma_start(
                out=b1eq.rearrange("q (g p) -> q g p", p=P),
                in_=b1e_d[:].rearrange("(g q p) -> q g p", q=4, p=P))

            agg0 = aggregate(0, h_sb, agg0p)
            module(0, agg0, xnTl, acc_h, rh_t, first=False)
            module(4, agg0, xnTl, acc_e, rh_t, first=False)

            agg1 = aggregate(1, h_sb, agg12p)
            agg2 = aggregate(2, h_sb, agg12p)
            agg0_stack.close()

            module(2, xeTl, agg1, acc_h, re_t, first=False)
            h_stack.close()

            for g0 in range(NT // 2, NT, 4):
                load_group(xe_d, e_sb, ssq_e, re_t, g0)
            agg3 = aggregate(3, e_sb, agg34p)
            wt_h = ffn_wload(w1h_d, w2h_d)
            module(3, xnTl, agg3, acc_h, rh_t, first=False)

            agg4 = aggregate(4, e_sb, agg34p)
            e_stack.close()

            ffn(acc_h, wt_h, b1h_t, b2h_t, outh_d)
            wt_e = ffn_wload(w1e_d, w2e_d)
            module(7, xnTl, agg4, acc_e, rh_t, first=False, warmln=True)
            agg34_stack.close()
            with tc.tile_pool(name="ffnE_sb", bufs=1) as fsbE:
                ecb, etail = mk_ffn_pipe(acc_e, wt_e[0], wt_e[1], b1eq,
                                         b2e_t, oute_d, fsbE)
                module(6, xeTl, agg2, acc_e, re_t, first=False, tile_cb=ecb)
                etail()
            agg12_stack.close()

        for _ in range(repeat):
            body()

    _split_big_waits(nc, mybir)
    return nc


def _get_program():
    if "nc" not in _PROGRAM_CACHE:
        _PROGRAM_CACHE["nc"] = _build_program()
    return _PROGRAM_CACHE["nc"]


def _prep_inputs(x_node, x_edge, adj, Wq, Wk, Wv,
                 proj_he_h, proj_eh_h, proj_he_e, proj_eh_e,
                 rms1_h, rms1_e, rms2_h,
                 w1_h, b1_h, w2_h, b2_h, w1_e, b1_e, w2_e, b2_e):
    """Per-core input dicts. Weight folding + row rotation happen here."""
    from ml_dtypes import bfloat16, float8_e4m3
    f = np.float32
    bf = bfloat16
    f16 = np.float16
    f8 = float8_e4m3
    # mats quantized to fp8e4m3, pre-scaled into a good fp8 range; the
    # inverse scale is folded into the wq/wk of the module consuming the
    # aggregate (mat0=adj -> q of modules 0/4; mat1..4 -> k of 2,6,3,7).
    MSC = [4096.0, 64.0, 64.0, 64.0, 64.0]
    qsc = [1.0 / MSC[0], 1, 1, 1, 1.0 / MSC[0], 1, 1, 1]
    ksc = [1, 1, 1.0 / MSC[1], 1.0 / MSC[3], 1, 1, 1.0 / MSC[2], 1.0 / MSC[4]]
    wsrc_q = [rms1_h, rms1_e, rms1_e, rms1_h, rms1_h, rms1_e, rms1_e, rms1_h]
    wsrc_k = [rms1_h, rms1_e, rms1_h, rms1_e, rms1_h, rms1_e, rms1_h, rms1_e]
    wqT = np.stack([(Wq[m].T * wsrc_q[m][:, None]) * (0.125 * qsc[m])
                    for m in range(H)])
    wkT = np.stack([(Wk[m].T * wsrc_k[m][:, None]) * ksc[m] for m in range(H)])
    wvT = np.stack([Wv[m].T * rms1_h[:, None] for m in range(H)])
    w1hT = np.ascontiguousarray((w1_h * rms2_h[None, :]).T.astype(bf))
    w1eT = np.ascontiguousarray((w1_e * rms2_h[None, :]).T.astype(bf))
    w2hT = np.ascontiguousarray(w2_h.T.astype(f16))
    w2eT = np.ascontiguousarray(w2_e.T.astype(f16))
    mats = [adj, proj_eh_h, proj_eh_e, proj_he_h, proj_he_e]

    shared = dict(wqT=np.ascontiguousarray(wqT.astype(bf)),
                  wkT=np.ascontiguousarray(wkT.astype(bf)),
                  wvT=np.ascontiguousarray(wvT.astype(bf)),
                  w1hT=w1hT, w2hT=w2hT, w1eT=w1eT, w2eT=w2eT,
                  b1h=b1_h.astype(f), b2h=b2_h.astype(f),
                  b1e=b1_e.astype(f), b2e=b2_e.astype(f))
    xn_bf = x_node.astype(bf)
    xe_bf = x_edge.astype(bf)
    in_maps = []
    for c in range(NCORES):
        r0 = c * RPC
        m = dict(shared)
        m["xn"] = np.ascontiguousarray(np.roll(xn_bf, -r0, axis=0))
        m["xe"] = np.ascontiguousarray(np.roll(xe_bf, -r0, axis=0))
        m["xnT"] = np.ascontiguousarray(xn_bf[r0:r0 + RPC].T)
        m["xeT"] = np.ascontiguousarray(xe_bf[r0:r0 + RPC].T)
        for i, mat in enumerate(mats):
            mt = np.ascontiguousarray(
                (mat[r0:r0 + RPC].T * MSC[i]).astype(f8))  # [N, RPC]
            m[f"mat{i}"] = np.ascontiguousarray(np.roll(mt, -r0, axis=0))
        in_maps.append(m)
    return in_maps


def kernel(**inputs):
    from concourse.bass_utils import run_bass_kernel_spmd
    nc = _get_program()
    in_maps = _prep_inputs(**{k: np.asarray(v) for k, v in inputs.items()})
    res = run_bass_kernel_spmd(nc, in_maps, list(range(NCORES))).results
    x_h = np.concatenate([res[c]["outh"] for c in range(NCORES)], axis=0)
    x_e = np.concatenate([res[c]["oute"] for c in range(NCORES)], axis=0)
    return (x_h, x_e)

